# revision 1
# baseline (speedup 1.0000x reference)
"""CDiT block kernel for 8 TRN2 NeuronCores.

Sharding: core c handles batch b=c//2, token half h=c%2 (512 of 1024 tokens).
Each core computes the full output for its (b, token-half) slice; K/V are
computed over the full T of the batch (duplicated within the pair), so no
cross-core collectives are needed.

Host folds adaLN modulation (scale/shift) and gates into the projection
weights/biases (per-batch constants), pre-transposes weights to [din, dout]
bf16, and pre-transposes x to feature-major [D, T] with the token axis
rolled so each core's own tokens are [0:512].

Device structure (no DMA transposes anywhere; engines kept decoupled):
- LayerNorm: stats via ones-matmuls into [1,512] psum rows; mean/var
  broadcast back with a ones matmul; rstd = exp(-0.5*ln(var+eps)) on the
  Act engine (ln/exp share one act table with softmax exp -> only two
  table loads in the whole kernel; DVE reciprocal is ~7 cyc/elem, avoid).
- Attention scores are computed K-MAJOR (stationary = host-stacked
  [Kr;Ki] tile per k-tile, moving = stacked-Q [Qr;-Qi]) in kt-PAIRS into
  [P,2,512] psums; one paired exp() activation per psum writes A tiles
  [k, kt, q] directly in the layout AV needs.
- Softmax denominators: z = ones-matmul over A k-tiles -> ln z (Act) ->
  broadcast via f32 ones-matmul -> S = exp(-bcast) = 1/z (Act).
- V is computed DIRECTLY k-major (stationary = LN'd activations as
  [c,t]-tiles, moving = host-repacked V weights) giving VA tiles
  [k, (head: vre|vim)]; the swapped copy [vim|vre] for the A_im half of
  AV is made per-head on GpSimd.
- Heads are software-pipelined: head h-1's AV matmuls are interleaved
  into head h's score stream (subtile deps let exp(h) overwrite A slices
  right after AV(h-1) reads them), so the PE fills act-wait gaps.
- AV epilogue on DVE: otmp_h = [(ps_a*S_re)+b -/+ (ps_b*S_im)] with the
  V bias applied via scalar_tensor_tensor; otmp_h [128=(re|im), q] feeds
  o-proj directly through head-stacked o weights (no reassembly).
- PSUM budget (8 banks): "sc2" [P,2,512] ring 2 (scores/projections/f2
  held psums/LN stats) + "avt" [P,2,512] ring 2 (AV psums, z rows+bcast,
  LN mean/var broadcast).
- Q-stack SBUF copies issued from the GpSimd DMA queue so the Sync queue
  keeps x chunk loads ahead of LayerNorm.
"""

import os
import sys
import numpy as np

for _p in ("/opt/trn_rl_repo",):
    if _p not in sys.path:
        sys.path.insert(0, _p)

import ml_dtypes

import concourse.bass as bass
import concourse.mybir as mybir
import concourse.tile as tile
from concourse.bass_utils import run_bass_kernel_spmd

B, T, D, H = 4, 1024, 512, 8
DH = D // H
MLP = 4 * D
EPS = 1e-6
P = 128
DT = D // P          # 4 feature tiles
TQ = T // 2          # own tokens per core
KT = T // P          # 8 k-tiles
NCORES = 8

F32 = mybir.dt.float32
BF16 = mybir.dt.bfloat16
AF = mybir.ActivationFunctionType
ALU = mybir.AluOpType
BF = ml_dtypes.bfloat16


# ----------------------------------------------------------------------------
# Host-side prep
# ----------------------------------------------------------------------------

def _feat_major(w_t):
    """[din, dout] -> [128, din//128 * dout] with din = kt*128 + p."""
    din, dout = w_t.shape
    return np.ascontiguousarray(
        w_t.reshape(din // P, P, dout).transpose(1, 0, 2).reshape(P, -1)
    )


def _col(v):
    """[dout] -> [128, dout//128] per-partition bias layout (d = o*128+p)."""
    return np.ascontiguousarray(v.reshape(-1, P).T)


def _silu(x):
    return x / (1.0 + np.exp(-x))


def _prep_core(inputs, b, half):
    """Build the per-core input map (numpy arrays, host precomputation)."""
    f = np.float32
    g = lambda n: np.asarray(inputs[n], dtype=f)

    # adaLN on host (tiny): complex silu -> complex linear -> 6 chunks
    sr, si = _silu(g('c_re')[b]), _silu(g('c_im')[b])
    aWr, aWi = g('ada_Wr'), g('ada_Wi')
    m_re = aWr @ sr - aWi @ si + (g('ada_br') - g('ada_bi'))
    m_im = aWr @ si + aWi @ sr + (g('ada_br') + g('ada_bi'))
    sh_ar, sc_ar, g_ar, sh_mr, sc_mr, g_mr = np.split(m_re, 6)
    sh_ai, sc_ai, g_ai, sh_mi, sc_mi, g_mi = np.split(m_im, 6)

    def fold_mod(Wr, Wi, br, bi, a, bb, shr, shi):
        """Fold complex modulate diag(a+ib)+shift into complex linear."""
        Mr = Wr * a[None, :] - Wi * bb[None, :]
        Mi = Wi * a[None, :] + Wr * bb[None, :]
        bre = (br - bi) + Wr @ shr - Wi @ shi
        bim = (br + bi) + Wi @ shr + Wr @ shi
        return Mr, Mi, bre, bim

    a1, b1 = 1.0 + sc_ar, sc_ai
    a2, b2 = 1.0 + sc_mr, sc_mi

    qMr, qMi, qbre, qbim = fold_mod(g('q_Wr'), g('q_Wi'), g('q_br'), g('q_bi'),
                                    a1, b1, sh_ar, sh_ai)
    kMr, kMi, kbre, kbim = fold_mod(g('k_Wr'), g('k_Wi'), g('k_br'), g('k_bi'),
                                    a1, b1, sh_ar, sh_ai)
    vMr, vMi, vbre, vbim = fold_mod(g('v_Wr'), g('v_Wi'), g('v_br'), g('v_bi'),
                                    a1, b1, sh_ar, sh_ai)
    scale = 1.0 / np.sqrt(np.float32(DH))
    qMr, qMi, qbre, qbim = qMr * scale, qMi * scale, qbre * scale, qbim * scale

    f1Mr, f1Mi, f1bre, f1bim = fold_mod(g('f1_Wr'), g('f1_Wi'),
                                        g('f1_br'), g('f1_bi'),
                                        a2, b2, sh_mr, sh_mi)

    # o-proj with attention gate folded (row scaling by complex g_a)
    oWr, oWi = g('o_Wr'), g('o_Wi')
    oGr = g_ar[:, None] * oWr - g_ai[:, None] * oWi
    oGi = g_ai[:, None] * oWr + g_ar[:, None] * oWi
    obre, obim = g('o_br') - g('o_bi'), g('o_br') + g('o_bi')
    ogbre = g_ar * obre - g_ai * obim
    ogbim = g_ai * obre + g_ar * obim

    # f2 with MLP gate folded
    fWr, fWi = g('f2_Wr'), g('f2_Wi')
    fGr = g_mr[:, None] * fWr - g_mi[:, None] * fWi
    fGi = g_mi[:, None] * fWr + g_mr[:, None] * fWi
    fbre, fbim = g('f2_br') - g('f2_bi'), g('f2_br') + g('f2_bi')
    fgbre = g_mr * fbre - g_mi * fbim
    fgbim = g_mi * fbre + g_mr * fbim

    # KA stacked weights: out rows = per head [Kr_h(64); Ki_h(64)]
    kA = np.empty((D * 2, D), f)   # rows for nr
    kB = np.empty((D * 2, D), f)   # rows for ni
    ka_b = np.empty(D * 2, f)
    for h in range(H):
        r = slice(h * DH, (h + 1) * DH)
        blk = slice(h * P, h * P + DH)
        blk2 = slice(h * P + DH, (h + 1) * P)
        kA[blk], kA[blk2] = kMr[r], kMi[r]
        kB[blk], kB[blk2] = -kMi[r], kMr[r]
        ka_b[blk], ka_b[blk2] = kbre[r], kbim[r]

    # AV epilogue bias: per head col [vbre-vbim ; vbre+vbim]
    av_b = np.empty(D * 2, f)
    for h in range(H):
        r = slice(h * DH, (h + 1) * DH)
        av_b[h * P: h * P + DH] = vbre[r] - vbim[r]
        av_b[h * P + DH: (h + 1) * P] = vbre[r] + vbim[r]

    bf = lambda w: _feat_major(w).astype(BF)

    wq = np.concatenate([bf(qMr.T), bf(qMi.T), bf(-qMi.T)], axis=1)
    wka0 = np.concatenate([bf(kA.T[:, 0:512]), bf(kB.T[:, 0:512])], axis=1)
    wka1 = np.concatenate([bf(kA.T[:, 512:1024]), bf(kB.T[:, 512:1024])],
                          axis=1)

    # V k-major pack: [ct*128+c, comp, (h, re|im, j)] -> [128, 2*4*1024]
    # comp0 (moving vs h_r stationary): re<-vMr, im<-vMi
    # comp1 (vs h_i): re<- -vMi, im<- vMr
    vMr_h = vMr.reshape(H, DH, D)            # [h, j, din]
    vMi_h = vMi.reshape(H, DH, D)
    wvk_np = np.empty((P, 2, DT, H, 2, DH), f)
    for ct in range(DT):
        cs = slice(ct * P, (ct + 1) * P)
        # [din_c, h, j]
        wvk_np[:, 0, ct, :, 0, :] = vMr_h[:, :, cs].transpose(2, 0, 1)
        wvk_np[:, 0, ct, :, 1, :] = vMi_h[:, :, cs].transpose(2, 0, 1)
        wvk_np[:, 1, ct, :, 0, :] = -vMi_h[:, :, cs].transpose(2, 0, 1)
        wvk_np[:, 1, ct, :, 1, :] = vMr_h[:, :, cs].transpose(2, 0, 1)
    wvk0 = np.ascontiguousarray(wvk_np[:, 0].reshape(P, -1)).astype(BF)
    wvk1 = np.ascontiguousarray(wvk_np[:, 1].reshape(P, -1)).astype(BF)

    # o-proj pack consuming head-stacked attn tiles:
    # wo[c(=head feat: j<64 re, j>=64 im), oc, h, do]
    # oc0 (x2r): j<64 -> oGr[do, h*64+j]; j>=64 -> -oGi[do, h*64+j-64]
    # oc1 (x2i): j<64 -> oGi[...];        j>=64 -> +oGr[...]
    oGr_h = oGr.reshape(D, H, DH)            # [do, h, j]
    oGi_h = oGi.reshape(D, H, DH)
    wo_np = np.empty((P, 2, H, D), f)
    wo_np[0:DH, 0] = oGr_h.transpose(2, 1, 0)      # [j, h, do]
    wo_np[DH:P, 0] = -oGi_h.transpose(2, 1, 0)
    wo_np[0:DH, 1] = oGi_h.transpose(2, 1, 0)
    wo_np[DH:P, 1] = oGr_h.transpose(2, 1, 0)
    wo0 = np.ascontiguousarray(wo_np[:, 0].reshape(P, -1)).astype(BF)
    wo1 = np.ascontiguousarray(wo_np[:, 1].reshape(P, -1)).astype(BF)

    wf1 = [np.concatenate([bf(f1Mr.T[:, j * 512:(j + 1) * 512]),
                           bf(f1Mi.T[:, j * 512:(j + 1) * 512]),
                           bf(-f1Mi.T[:, j * 512:(j + 1) * 512])], axis=1)
           for j in range(4)]
    wf2 = [np.concatenate([bf(fGr.T[j * 512:(j + 1) * 512]),
                           bf(fGi.T[j * 512:(j + 1) * 512]),
                           bf(-fGi.T[j * 512:(j + 1) * 512])], axis=1)
           for j in range(4)]

    smalls = np.concatenate([
        _col(qbre), _col(qbim), _col(-qbim),           # 0:4, 4:8, 8:12
        _col(ka_b),                                    # 12:20
        _col(av_b),                                    # 20:28
        _col(ogbre), _col(ogbim),                      # 28:32, 32:36
        _col(f1bre), _col(f1bim),                      # 36:52, 52:68
        _col(fgbre), _col(fgbim),                      # 68:72, 72:76
        np.full((P, 1), EPS, f),                       # 76
    ], axis=1)

    roll = lambda a: np.roll(a, -half * TQ, axis=0)
    xTr = np.ascontiguousarray(roll(g('x_re')[b]).T)
    xTi = np.ascontiguousarray(roll(g('x_im')[b]).T)

    im = {'xTr': xTr, 'xTi': xTi,
          'xbr': xTr.astype(BF), 'xbi': xTi.astype(BF),
          'wq': wq, 'wka0': wka0, 'wka1': wka1,
          'wvk0': wvk0, 'wvk1': wvk1, 'wo0': wo0, 'wo1': wo1,
          'smalls': smalls}
    for j in range(4):
        im[f'wf1_{j}'] = wf1[j]
        im[f'wf2_{j}'] = wf2[j]
    return im


# ----------------------------------------------------------------------------
# Device program
# ----------------------------------------------------------------------------

def build_nc(reps=1):
    nc = bass.Bass()

    xTr_d = nc.declare_dram_parameter("xTr", [D, T], F32, isOutput=False)
    xTi_d = nc.declare_dram_parameter("xTi", [D, T], F32, isOutput=False)
    xbr_d = nc.declare_dram_parameter("xbr", [D, T], BF16, isOutput=False)
    xbi_d = nc.declare_dram_parameter("xbi", [D, T], BF16, isOutput=False)
    wq_d = nc.declare_dram_parameter("wq", [P, 6144], BF16, isOutput=False)
    wka0_d = nc.declare_dram_parameter("wka0", [P, 4096], BF16, isOutput=False)
    wka1_d = nc.declare_dram_parameter("wka1", [P, 4096], BF16, isOutput=False)
    wvk_d = [nc.declare_dram_parameter(f"wvk{j}", [P, 4096], BF16,
                                       isOutput=False) for j in range(2)]
    wo_d = [nc.declare_dram_parameter(f"wo{j}", [P, 4096], BF16,
                                      isOutput=False) for j in range(2)]
    wf1_d = [nc.declare_dram_parameter(f"wf1_{j}", [P, 6144], BF16,
                                       isOutput=False) for j in range(4)]
    wf2_d = [nc.declare_dram_parameter(f"wf2_{j}", [P, 6144], BF16,
                                       isOutput=False) for j in range(4)]
    smalls_d = nc.declare_dram_parameter("smalls", [P, 77], F32, isOutput=False)
    out_d = nc.declare_dram_parameter("outT", [2, D, TQ], F32, isOutput=True)

    with tile.TileContext(nc) as tc:
        with (
            tc.tile_pool(name="persist", bufs=1) as pp,
            tc.tile_pool(name="acts", bufs=1) as ap_,
            tc.tile_pool(name="tmp", bufs=2) as tp,
            tc.tile_pool(name="attn", bufs=2) as atp,
            tc.tile_pool(name="psum", bufs=2, space="PSUM") as psp,
            tc.tile_pool(name="psum2", bufs=1, space="PSUM") as ps2,
        ):
            def emit():
                dma = nc.sync.dma_start

                smalls = pp.tile([P, 77], F32)
                dma(smalls, smalls_d.ap())
                b_qre, b_qim, b_nqim = smalls[:, 0:4], smalls[:, 4:8], smalls[:, 8:12]
                b_ka = smalls[:, 12:20]
                b_av = smalls[:, 20:28]
                b_ore, b_oim = smalls[:, 28:32], smalls[:, 32:36]
                b_f1re, b_f1im = smalls[:, 36:52], smalls[:, 52:68]
                b_f2re, b_f2im = smalls[:, 68:72], smalls[:, 72:76]
                eps = smalls[:, 76:77]

                ones = pp.tile([P, 1], BF16)
                nc.vector.memset(ones, 1.0)
                onesf = pp.tile([1, P], F32)
                nc.vector.memset(onesf, 1.0)
                onesb = pp.tile([1, P], BF16)
                nc.vector.memset(onesb, 1.0)
                m10 = pp.tile([P, 1], F32)
                nc.vector.memset(m10, -10.0)

                def loadpack(src, n, eng=None):
                    wpk = pp.tile([P, 6144], BF16, tag="wpk", bufs=3, name=n)
                    d_ = eng.dma_start if eng is not None else dma
                    d_(wpk[:, 0:src.shape[1]], src.ap())
                    return wpk

                def pair_ps(name):
                    return psp.tile([P, 2, 512], F32, tag="sc2", bufs=2,
                                    name=name)

                def big_ps(name):
                    return ps2.tile([P, 2, 512], F32, tag="avt", bufs=2,
                                    name=name)

                # ---------------- LayerNorm helper (per 512-token chunk) --------
                def ln_chunk(fetch, nout, tag, direct=False):
                    """direct: fetch(d, dst) DMAs bf16 into dst.
                    else: fetch(d) -> [P, 512] f32 AP (SBUF).
                    nout [P, DT, 512] bf16."""
                    xsq = tp.tile([P, DT, 2, 512], BF16, tag="xsq", bufs=1)
                    for d in range(DT):
                        if direct:
                            fetch(d, xsq[:, d, 0, :])
                        else:
                            nc.vector.tensor_copy(out=xsq[:, d, 0, :],
                                                  in_=fetch(d))
                        nc.vector.tensor_tensor(
                            out=xsq[:, d, 1, :], in0=xsq[:, d, 0, :],
                            in1=xsq[:, d, 0, :], op=ALU.mult)
                    st = pair_ps(f"st{tag}")
                    for s in range(2):
                        for d in range(DT):
                            nc.tensor.matmul(st[0:1, s, :], ones[:, 0:1],
                                             xsq[:, d, s, :],
                                             start=(d == 0), stop=(d == DT - 1))
                    strow = tp.tile([1, 2, 512], F32, tag="strow", bufs=1)
                    for s in range(2):
                        nc.scalar.activation(out=strow[:, s, :],
                                             in_=st[0:1, s, :], func=AF.Copy)
                    lnvp = big_ps(f"lnv{tag}")
                    for s in range(2):
                        nc.tensor.matmul(lnvp[:, s, :], onesf, strow[0:1, s, :],
                                         start=True, stop=True)
                    lnv = tp.tile([P, 2, 512], F32, tag="lnv", bufs=1)
                    nc.vector.tensor_copy(out=lnv, in_=lnvp)
                    mu, msq = lnv[:, 0, :], lnv[:, 1, :]
                    nc.scalar.mul(out=mu, in_=mu, mul=1.0 / D)
                    nc.scalar.mul(out=msq, in_=msq, mul=1.0 / D)
                    sc = tp.tile([P, 512], F32, tag="lnsc")
                    nc.vector.tensor_tensor(out=sc, in0=mu, in1=mu, op=ALU.mult)
                    nc.vector.tensor_tensor(out=msq, in0=msq, in1=sc,
                                            op=ALU.subtract)
                    # rstd = exp(-0.5 * ln(var + eps)); ln/exp share one table
                    nc.scalar.activation(out=msq, in_=msq, func=AF.Ln, bias=eps)
                    nc.scalar.activation(out=msq, in_=msq, func=AF.Exp,
                                         scale=-0.5)
                    nc.vector.tensor_tensor(out=mu, in0=mu, in1=msq,
                                            op=ALU.mult)            # mu*rstd
                    for d in range(DT):
                        sc2 = tp.tile([P, 512], F32, tag="lnsc")
                        nc.vector.tensor_tensor(out=sc2, in0=xsq[:, d, 0, :],
                                                in1=msq, op=ALU.mult)
                        nc.vector.tensor_tensor(out=nout[:, d, :], in0=sc2,
                                                in1=mu, op=ALU.subtract)

                def fetch_x(src_d, ch):
                    def fetch(d, dst):
                        xv = src_d.ap().rearrange("(o p) t -> p o t", p=P)
                        dma(dst, xv[:, d, ch * 512:(ch + 1) * 512])
                    return fetch

                # ---------------- projection helper -----------------------------
                def cgroups(mA, mB, mC):
                    """complex matmul groups: re=(A,nr),(C,ni); im=(B,nr),(A,ni)"""
                    return (((mA, 0), (mC, 1)), ((mB, 0), (mA, 1)))

                def run_group(ps_, pairs, rhs_re, rhs_im, mt, ch):
                    n = len(pairs) * DT
                    i = 0
                    for m_, which in pairs:
                        r_ = rhs_re if which == 0 else rhs_im
                        for d in range(DT):
                            nc.tensor.matmul(
                                ps_, m_[:, d, mt * P:(mt + 1) * P],
                                r_[:, d, ch * 512:(ch + 1) * 512],
                                start=(i == 0), stop=(i == n - 1))
                            i += 1

                def msec(pk, i, cols=512):
                    return pk[:, i * DT * cols:(i + 1) * DT * cols].rearrange(
                        "p (k n) -> p k n", k=DT)

                nrf = ap_.tile([P, DT, T], BF16, tag="nbig1")
                nif = ap_.tile([P, DT, T], BF16, tag="nbig2")

                # ---------------- LN1 chunk 0 (own tokens) ----------------------
                ln_chunk(fetch_x(xbr_d, 0), nrf[:, :, 0:512], "1r0",
                         direct=True)
                ln_chunk(fetch_x(xbi_d, 0), nif[:, :, 0:512], "1i0",
                         direct=True)

                # ---------------- Q (own half) + stacks, per dtile --------------
                wq = loadpack(wq_d, "wq")
                qa, qb_, qc = msec(wq, 0), msec(wq, 1), msec(wq, 2)
                QS = []   # (QC_h, QD_h) per head
                gre, gim = cgroups(qa, qb_, qc)
                for d in range(DT):
                    qre_t = atp.tile([P, 512], BF16, tag="qp", bufs=2, name="qre")
                    qim_t = atp.tile([P, 512], BF16, tag="qp", bufs=2, name="qim")
                    nqim_t = atp.tile([P, 512], BF16, tag="qp", bufs=2, name="nqim")
                    qps = pair_ps("psq")
                    run_group(qps[:, 0, :], gre, nrf, nif, d, 0)
                    nc.scalar.activation(out=qre_t, in_=qps[:, 0, :],
                                         func=AF.Identity,
                                         bias=b_qre[:, d:d + 1])
                    run_group(qps[:, 1, :], gim, nrf, nif, d, 0)
                    nc.scalar.activation(out=qim_t, in_=qps[:, 1, :],
                                         func=AF.Identity,
                                         bias=b_qim[:, d:d + 1])
                    nc.scalar.activation(out=nqim_t, in_=qps[:, 1, :],
                                         func=AF.Identity,
                                         scale=-1.0, bias=b_nqim[:, d:d + 1])
                    for hh in range(2):
                        h = 2 * d + hh
                        qc_h = atp.tile([P, 512], BF16, tag="qs", bufs=16,
                                        name=f"qc{h}")
                        qd_h = atp.tile([P, 512], BF16, tag="qs", bufs=16,
                                        name=f"qd{h}")
                        sl = slice(hh * DH, hh * DH + DH)
                        gdma = nc.gpsimd.dma_start
                        gdma(qc_h[0:DH, :], qre_t[sl, :])
                        gdma(qc_h[DH:P, :], nqim_t[sl, :])
                        gdma(qd_h[0:DH, :], qim_t[sl, :])
                        gdma(qd_h[DH:P, :], qre_t[sl, :])
                        QS.append((qc_h, qd_h))

                # ---------------- LN1 chunk 1 (other half; overlaps Q) ----------
                ln_chunk(fetch_x(xbr_d, 1), nrf[:, :, 512:1024], "1r1",
                         direct=True)
                ln_chunk(fetch_x(xbi_d, 1), nif[:, :, 512:1024], "1i1",
                         direct=True)

                # ---------------- V direct k-major -------------------------------
                # VA[k, kt, (h: vre|vim)]
                wvk0 = loadpack(wvk_d[0], "wvk0", eng=nc.gpsimd)
                wvk1 = loadpack(wvk_d[1], "wvk1", eng=nc.gpsimd)
                wvv = [w[:, 0:4096].rearrange("p (k n) -> p k n", k=DT)
                       for w in (wvk0, wvk1)]
                VA = ap_.tile([P, KT, 2 * D], BF16, tag="VAx")
                for kt in range(KT):
                    vps = pair_ps("psv")
                    for half in range(2):
                        i = 0
                        for comp, stat in ((0, nrf), (1, nif)):
                            for ct in range(DT):
                                nc.tensor.matmul(
                                    vps[:, half, :],
                                    stat[:, ct, kt * P:(kt + 1) * P],
                                    wvv[comp][:, ct,
                                              half * 512:(half + 1) * 512],
                                    start=(i == 0), stop=(i == 7))
                                i += 1
                    nc.scalar.activation(out=VA[:, kt, :], in_=vps,
                                         func=AF.Copy)
                VAv = VA.rearrange("p k (h s j) -> p k h s j", h=H, s=2)

                # ---------------- KA per head (full T) ---------------------------
                wka0 = loadpack(wka0_d, "wka0")
                wka1 = loadpack(wka1_d, "wka1")
                KAh = [atp.tile([P, T], BF16, tag="kah", bufs=8,
                                name=f"ka{h}") for h in range(H)]
                for ch in range(T // 512):
                    for hp in range(H // 2):
                        kps = pair_ps("psk")
                        for sl2 in range(2):
                            h = 2 * hp + sl2
                            pk = wka0 if h < 4 else wka1
                            hl = h % 4
                            kaA = pk[:, 0:2048].rearrange(
                                "p (k n) -> p k n", k=DT)
                            kaB = pk[:, 2048:4096].rearrange(
                                "p (k n) -> p k n", k=DT)
                            i = 0
                            for m_, r_ in ((kaA, nrf), (kaB, nif)):
                                for d in range(DT):
                                    nc.tensor.matmul(
                                        kps[:, sl2, :],
                                        m_[:, d, hl * P:(hl + 1) * P],
                                        r_[:, d, ch * 512:(ch + 1) * 512],
                                        start=(i == 0), stop=(i == 7))
                                    i += 1
                        for sl2 in range(2):
                            h = 2 * hp + sl2
                            nc.scalar.activation(
                                out=KAh[h][:, ch * 512:(ch + 1) * 512],
                                in_=kps[:, sl2, :], func=AF.Identity,
                                bias=b_ka[:, h:h + 1])

                # ---------------- attention (software-pipelined heads) ----------
                wo0 = loadpack(wo_d[0], "wo0", eng=nc.gpsimd)
                wo1 = loadpack(wo_d[1], "wo1", eng=nc.gpsimd)
                OT = [None] * H

                def emit_z(st):
                    """z row sums; S = exp(-ln z) broadcast, all on Act/PE
                    (no DVE reciprocal). Runs before next head's scores;
                    exps for st are already drained."""
                    zp = big_ps(f"zp{st['h']}")
                    st['zp'] = zp
                    st['lnz'] = []
                    for cn in range(2):
                        At = st['A'][cn]
                        for kt in range(KT):
                            nc.tensor.matmul(zp[0:1, cn, :], ones[:, 0:1],
                                             At[:, kt, :],
                                             start=(kt == 0),
                                             stop=(kt == KT - 1))
                        lnz = tp.tile([1, 512], F32, tag="rz", bufs=4,
                                      name="lnz")
                        nc.scalar.activation(out=lnz, in_=zp[0:1, cn, :],
                                             func=AF.Ln)
                        st['lnz'].append(lnz)
                    for cn in range(2):
                        nc.tensor.matmul(zp[:, cn, :], onesf, st['lnz'][cn],
                                         start=True, stop=True)

                def emit_epilogue(st):
                    """normalize + bias + combine into OT[h] (DVE)."""
                    h, avt = st['h'], st['avt']
                    SS = []
                    for cn in range(2):
                        S = atp.tile([P, 512], F32, tag="S", bufs=2,
                                     name=f"S{cn}")
                        nc.scalar.activation(out=S, in_=st['zp'][:, cn, :],
                                             func=AF.Exp, scale=-1.0)
                        SS.append(S)
                    t1 = tp.tile([P, 512], F32, tag="avt", bufs=2, name="t1")
                    t2 = tp.tile([P, 512], F32, tag="avt", bufs=2, name="t2")
                    nc.vector.tensor_tensor(out=t1, in0=avt[:, 0, :],
                                            in1=SS[0], op=ALU.mult)
                    nc.vector.tensor_tensor(out=t2, in0=avt[:, 1, :],
                                            in1=SS[1], op=ALU.mult)
                    otmp = atp.tile([P, 512], BF16, tag=f"ot{h}", bufs=1,
                                    name=f"ot{h}")
                    nc.vector.scalar_tensor_tensor(
                        out=otmp[0:DH, :], in0=t1[0:DH, :],
                        scalar=b_av[0:DH, h:h + 1], in1=t2[0:DH, :],
                        op0=ALU.add, op1=ALU.subtract)
                    nc.vector.scalar_tensor_tensor(
                        out=otmp[DH:P, :], in0=t1[DH:P, :],
                        scalar=b_av[DH:P, h:h + 1], in1=t2[DH:P, :],
                        op0=ALU.add, op1=ALU.add)
                    OT[h] = otmp

                prev = None
                for h in range(H):
                    qc_h, qd_h = QS[h]
                    ka_h = KAh[h]
                    # swapped V copy [vim|vre] for this head (GpSimd, idle)
                    vasw = atp.tile([P, KT, P], BF16, tag="vasw", bufs=2,
                                    name=f"vasw{h}")
                    vswv = vasw.rearrange("p k (s j) -> p k s j", s=2)
                    nc.gpsimd.tensor_copy(out=vswv[:, :, 0, :],
                                          in_=VAv[:, :, h, 1, :])
                    nc.gpsimd.tensor_copy(out=vswv[:, :, 1, :],
                                          in_=VAv[:, :, h, 0, :])

                    if prev is not None:
                        prev['avt'] = big_ps(f"av{prev['h']}")

                    A_re = atp.tile([P, KT, 512], BF16, tag="Are", bufs=1,
                                    name="Are")
                    A_im = atp.tile([P, KT, 512], BF16, tag="Aim", bufs=1,
                                    name="Aim")
                    for ci, (Qm, Atile) in enumerate(((qc_h, A_re),
                                                     (qd_h, A_im))):
                        for i in range(KT // 2):
                            if prev is not None:
                                # interleave prev head's AV matmuls
                                pav, ph = prev['avt'], prev['h']
                                pstat = (VAv[:, :, ph, :, :] if ci == 0
                                         else prev['vasw'])
                                for j in (2 * i, 2 * i + 1):
                                    lhs = (pstat[:, j, :, :] if ci == 0
                                           else pstat[:, j, :])
                                    nc.tensor.matmul(
                                        pav[:, ci, :], lhs,
                                        prev['A'][ci][:, j, :],
                                        start=(j == 0), stop=(j == KT - 1))
                            sp = pair_ps("pss")
                            for j2 in range(2):
                                kt = 2 * i + j2
                                nc.tensor.matmul(
                                    sp[:, j2, :],
                                    ka_h[:, kt * P:(kt + 1) * P], Qm,
                                    start=True, stop=True)
                            nc.scalar.activation(
                                out=Atile[:, 2 * i:2 * i + 2, :], in_=sp,
                                func=AF.Exp, bias=m10[:, 0:1])
                            if prev is not None and ci == 0 and i == 1:
                                # prev's z reduction fills the act-paced
                                # stalls mid-scores (its exps are drained)
                                emit_z(prev)
                    if prev is not None:
                        emit_epilogue(prev)
                    prev = {'h': h, 'A': (A_re, A_im), 'vasw': vasw}

                # drain the pipeline for the last head
                emit_z(prev)
                prev['avt'] = big_ps(f"av{prev['h']}")
                ph = prev['h']
                for ci in range(2):
                    pstat = VAv[:, :, ph, :, :] if ci == 0 else prev['vasw']
                    for j in range(KT):
                        lhs = pstat[:, j, :, :] if ci == 0 else pstat[:, j, :]
                        nc.tensor.matmul(prev['avt'][:, ci, :], lhs,
                                         prev['A'][ci][:, j, :],
                                         start=(j == 0), stop=(j == KT - 1))
                emit_epilogue(prev)

                # ---------------- o-proj (gated) + residual + LN2 ---------------
                wov = [w[:, 0:4096].rearrange("p (h n) -> p h n", h=H)
                       for w in (wo0, wo1)]
                x2 = ap_.tile([P, 2, DT, TQ], F32, tag="VAx")
                x2r, x2i = x2[:, 0], x2[:, 1]
                n2r = ap_.tile([P, DT, TQ], BF16, tag="n2r")
                n2i = ap_.tile([P, DT, TQ], BF16, tag="n2i")
                for oc, (dst, bias, src_d) in enumerate(
                        ((x2r, b_ore, xTr_d), (x2i, b_oim, xTi_d))):
                    xv = src_d.ap().rearrange("(o p) t -> p o t", p=P)
                    for mp in range(2):
                        ops = pair_ps("pso")
                        for sl2 in range(2):
                            mt = 2 * mp + sl2
                            for h in range(H):
                                nc.tensor.matmul(
                                    ops[:, sl2, :],
                                    wov[oc][:, h, mt * P:(mt + 1) * P], OT[h],
                                    start=(h == 0), stop=(h == H - 1))
                        for sl2 in range(2):
                            mt = 2 * mp + sl2
                            og = tp.tile([P, TQ], F32, tag="og", name="og")
                            nc.scalar.activation(out=og, in_=ops[:, sl2, :],
                                                 func=AF.Identity,
                                                 bias=bias[:, mt:mt + 1])
                            xres = tp.tile([P, TQ], F32, tag="xch", bufs=3,
                                           name="xres")
                            dma(xres, xv[:, mt, 0:TQ])
                            nc.vector.tensor_tensor(out=dst[:, mt, :], in0=og,
                                                    in1=xres, op=ALU.add)
                ln_chunk(lambda d: x2r[:, d, :], n2r, "2r")
                ln_chunk(lambda d: x2i[:, d, :], n2i, "2i")

                # ---------------- MLP -------------------------------------------
                g1r = ap_.tile([P, MLP // P, TQ], BF16, tag="nbig1")
                g1i = ap_.tile([P, MLP // P, TQ], BF16, tag="nbig2")
                for j in range(4):
                    pk = loadpack(wf1_d[j], f"wf1_{j}")
                    f1a, f1b, f1c = msec(pk, 0), msec(pk, 1), msec(pk, 2)
                    fre, fim = cgroups(f1a, f1b, f1c)
                    for gi, grp in enumerate((fre, fim)):
                        bias = b_f1re if gi == 0 else b_f1im
                        dst = g1r if gi == 0 else g1i
                        for mp in range(2):
                            fps = pair_ps("psf1")
                            for sl2 in range(2):
                                ml = 2 * mp + sl2
                                run_group(fps[:, sl2, :], grp, n2r, n2i, ml, 0)
                            for sl2 in range(2):
                                ml = 2 * mp + sl2
                                mt = j * 4 + ml
                                nc.scalar.activation(
                                    out=dst[:, mt, :], in_=fps[:, sl2, :],
                                    func=AF.Gelu_apprx_tanh,
                                    bias=bias[:, mt:mt + 1])

                # f2: two passes (re, im), 4 held psums each, packs cycle per pass
                for gi in range(2):
                    bias = b_f2re if gi == 0 else b_f2im
                    x2s = x2r if gi == 0 else x2i
                    pt1 = pair_ps(f"f2a{gi}")
                    pt2 = pair_ps(f"f2b{gi}")
                    psums = [pt1[:, 0, :], pt1[:, 1, :],
                             pt2[:, 0, :], pt2[:, 1, :]]
                    for j in range(4):
                        pk = loadpack(wf2_d[j], f"wf2p_{gi}{j}")
                        f2a, f2b, f2c = msec(pk, 0), msec(pk, 1), msec(pk, 2)
                        pairs = ((f2a, 0), (f2c, 1)) if gi == 0 else \
                                ((f2b, 0), (f2a, 1))
                        for mt in range(DT):
                            i = 0
                            for m_, which in pairs:
                                r_ = g1r if which == 0 else g1i
                                for kl in range(4):
                                    nc.tensor.matmul(
                                        psums[mt], m_[:, kl, mt * P:(mt + 1) * P],
                                        r_[:, j * 4 + kl, :],
                                        start=(j == 0 and i == 0),
                                        stop=(j == 3 and i == 7))
                                    i += 1
                    ov = out_d.ap().rearrange("c (o p) t -> c p o t", p=P)
                    for mt in range(DT):
                        fg = tp.tile([P, TQ], F32, tag="og", name="fg")
                        nc.scalar.activation(out=fg, in_=psums[mt],
                                             func=AF.Identity,
                                             bias=bias[:, mt:mt + 1])
                        oc_ = tp.tile([P, TQ], F32, tag="outc", bufs=2, name="oc")
                        nc.vector.tensor_tensor(out=oc_, in0=fg,
                                                in1=x2s[:, mt, :], op=ALU.add)
                        dma(ov[gi, :, mt, :], oc_)

            for _rep in range(reps):
                emit()

    _split_dma_waits(nc)
    return nc


def _split_dma_waits(nc):
    """Walrus's DIRECT2D DMA encoding takes one sync wait; move extra
    waits onto a preceding sequencer EventSemaphore on the same engine."""
    f = nc.m.functions[0]
    for blk in f.blocks:
        out = []
        for ins in blk.instructions:
            si = getattr(ins, 'sync_info', None)
            tn = type(ins).__name__
            lim = 1
            if si is not None and si.on_wait and len(si.on_wait) > lim:
                waits = list(si.on_wait)
                extra = waits[:-lim]
                si.on_wait = waits[-lim:]
                k = 0
                while extra:
                    ev = mybir.InstEventSemaphore(
                        name=f"{ins.name}_wsplit{k}",
                        engine=ins.engine,
                        ins=[], outs=[],
                        sync_info=mybir.SyncInfo(on_wait=extra[:2],
                                                 on_update=[]),
                    )
                    out.append(ev)
                    extra = extra[2:]
                    k += 1
            out.append(ins)
        blk.instructions = out


_NC_CACHE = None


def _get_nc():
    global _NC_CACHE
    if _NC_CACHE is None:
        _NC_CACHE = build_nc()
    return _NC_CACHE


TRACE = False
LAST_RESULT = None


def kernel(**inputs):
    global LAST_RESULT
    nc = _get_nc()
    in_maps = []
    for c in range(NCORES):
        in_maps.append(_prep_core(inputs, c // 2, c % 2))
    res = run_bass_kernel_spmd(nc, in_maps, list(range(NCORES)),
                               trace=TRACE)
    LAST_RESULT = res
    out = np.empty((2, B, T, D), np.float32)
    for c in range(NCORES):
        b, half = c // 2, c % 2
        y = res.results[c]["outT"]          # [2, D, TQ]
        out[:, b, half * TQ:(half + 1) * TQ, :] = y.transpose(0, 2, 1)
    return out



# revision 9
# speedup vs baseline: 1.1604x; 1.1604x over previous
"""CDiT block kernel for 8 TRN2 NeuronCores.

Sharding: core c handles batch b=c//2, token half h=c%2 (512 of 1024 tokens).
Each core computes the full output for its (b, token-half) slice; K/V are
computed over the full T of the batch (duplicated within the pair), so no
cross-core collectives are needed.

Host folds adaLN modulation (scale/shift) and gates into the projection
weights/biases (per-batch constants), pre-transposes weights to [din, dout]
bf16, and pre-transposes x to feature-major [D, T] with the token axis
rolled so each core's own tokens are [0:512].

Device structure (no DMA transposes anywhere; engines kept decoupled):
- LayerNorm: stats via ones-matmuls into [1,512] psum rows; mean/meansq are
  scaled 1/D during the Act row-evacuation; var/rstd computed on the bf16
  rows (Act Ln/Exp share one table with softmax exp); ONE bf16 ones-matmul
  broadcasts (rstd, mu) to 128 partitions; finals are all-bf16 DVE/GpSimd
  tensor ops split across both engines. LN1 chunk-1 stats/rows run
  interleaved under the Q projection; LN2 under o-proj.
- Attention scores are computed K-MAJOR (stationary = host-stacked
  [Kr;Ki] tile per k-tile, moving = stacked-Q [Qr;-Qi]) in kt-PAIRS into
  [P,2,512] psums; one paired exp() activation per psum (bias -8 recenters
  so ln z fits bf16) writes A tiles [k, kt, q] in the layout AV needs.
- Softmax denominators: z = ones-matmul over A k-tiles -> ln z (Act, bf16
  row) -> broadcast via bf16 ones-matmul -> S = exp(-bcast) = 1/z.
- V is computed DIRECTLY k-major (stationary = LN'd activations as
  [c,t]-tiles, moving = host-repacked V weights) giving VA tiles
  [k, (head: vre|vim)]; the swapped copy [vim|vre] for the A_im half of
  AV is made per-head on GpSimd.
- Heads are software-pipelined: head h-1's AV matmuls are interleaved
  into head h's score stream.
- o-proj runs mt-pair-major with scalar_tensor_tensor epilogues
  ((psum+bias)+residual in one DVE op); LN2 stats overlap it.
- f2 weights are repacked mt-major on host and streamed ONCE (psum held
  over the full K=2048 contraction); epilogues are DVE stt + batched
  2-tile output DMAs.
- Weight packs ride a 4-deep shared SBUF ring, issue order == consumption
  order, split across the sync and gpsimd DMA queues so Q-stack copies
  (sync queue) never delay weight arrival.
- PSUM budget (8 banks): "sc2" [P,2,512] ring 2 + "avt" [P,2,512] ring 2.
"""

import os
import sys
import numpy as np

for _p in ("/opt/trn_rl_repo",):
    if _p not in sys.path:
        sys.path.insert(0, _p)

import ml_dtypes

import concourse.bass as bass
import concourse.mybir as mybir
import concourse.tile as tile
from concourse.bass_utils import run_bass_kernel_spmd

B, T, D, H = 4, 1024, 512, 8
DH = D // H
MLP = 4 * D
EPS = 1e-6
P = 128
DT = D // P          # 4 feature tiles
TQ = T // 2          # own tokens per core
KT = T // P          # 8 k-tiles
NCORES = 8

F32 = mybir.dt.float32
BF16 = mybir.dt.bfloat16
AF = mybir.ActivationFunctionType
ALU = mybir.AluOpType
BF = ml_dtypes.bfloat16


# ----------------------------------------------------------------------------
# Host-side prep
# ----------------------------------------------------------------------------

def _feat_major(w_t):
    """[din, dout] -> [128, din//128 * dout] with din = kt*128 + p."""
    din, dout = w_t.shape
    return np.ascontiguousarray(
        w_t.reshape(din // P, P, dout).transpose(1, 0, 2).reshape(P, -1)
    )


def _col(v):
    """[dout] -> [128, dout//128] per-partition bias layout (d = o*128+p)."""
    return np.ascontiguousarray(v.reshape(-1, P).T)


def _silu(x):
    return x / (1.0 + np.exp(-x))


def _prep_core(inputs, b, half):
    """Build the per-core input map (numpy arrays, host precomputation)."""
    f = np.float32
    g = lambda n: np.asarray(inputs[n], dtype=f)

    # adaLN on host (tiny): complex silu -> complex linear -> 6 chunks
    sr, si = _silu(g('c_re')[b]), _silu(g('c_im')[b])
    aWr, aWi = g('ada_Wr'), g('ada_Wi')
    m_re = aWr @ sr - aWi @ si + (g('ada_br') - g('ada_bi'))
    m_im = aWr @ si + aWi @ sr + (g('ada_br') + g('ada_bi'))
    sh_ar, sc_ar, g_ar, sh_mr, sc_mr, g_mr = np.split(m_re, 6)
    sh_ai, sc_ai, g_ai, sh_mi, sc_mi, g_mi = np.split(m_im, 6)

    def fold_mod(Wr, Wi, br, bi, a, bb, shr, shi):
        """Fold complex modulate diag(a+ib)+shift into complex linear."""
        Mr = Wr * a[None, :] - Wi * bb[None, :]
        Mi = Wi * a[None, :] + Wr * bb[None, :]
        bre = (br - bi) + Wr @ shr - Wi @ shi
        bim = (br + bi) + Wi @ shr + Wr @ shi
        return Mr, Mi, bre, bim

    a1, b1 = 1.0 + sc_ar, sc_ai
    a2, b2 = 1.0 + sc_mr, sc_mi

    qMr, qMi, qbre, qbim = fold_mod(g('q_Wr'), g('q_Wi'), g('q_br'), g('q_bi'),
                                    a1, b1, sh_ar, sh_ai)
    kMr, kMi, kbre, kbim = fold_mod(g('k_Wr'), g('k_Wi'), g('k_br'), g('k_bi'),
                                    a1, b1, sh_ar, sh_ai)
    vMr, vMi, vbre, vbim = fold_mod(g('v_Wr'), g('v_Wi'), g('v_br'), g('v_bi'),
                                    a1, b1, sh_ar, sh_ai)
    scale = 1.0 / np.sqrt(np.float32(DH))
    qMr, qMi, qbre, qbim = qMr * scale, qMi * scale, qbre * scale, qbim * scale

    f1Mr, f1Mi, f1bre, f1bim = fold_mod(g('f1_Wr'), g('f1_Wi'),
                                        g('f1_br'), g('f1_bi'),
                                        a2, b2, sh_mr, sh_mi)

    # o-proj with attention gate folded (row scaling by complex g_a)
    oWr, oWi = g('o_Wr'), g('o_Wi')
    oGr = g_ar[:, None] * oWr - g_ai[:, None] * oWi
    oGi = g_ai[:, None] * oWr + g_ar[:, None] * oWi
    obre, obim = g('o_br') - g('o_bi'), g('o_br') + g('o_bi')
    ogbre = g_ar * obre - g_ai * obim
    ogbim = g_ai * obre + g_ar * obim

    # f2 with MLP gate folded
    fWr, fWi = g('f2_Wr'), g('f2_Wi')
    fGr = g_mr[:, None] * fWr - g_mi[:, None] * fWi
    fGi = g_mi[:, None] * fWr + g_mr[:, None] * fWi
    fbre, fbim = g('f2_br') - g('f2_bi'), g('f2_br') + g('f2_bi')
    fgbre = g_mr * fbre - g_mi * fbim
    fgbim = g_mi * fbre + g_mr * fbim

    # KA stacked weights: out rows = per head [Kr_h(64); Ki_h(64)]
    kA = np.empty((D * 2, D), f)   # rows for nr
    kB = np.empty((D * 2, D), f)   # rows for ni
    ka_b = np.empty(D * 2, f)
    for h in range(H):
        r = slice(h * DH, (h + 1) * DH)
        blk = slice(h * P, h * P + DH)
        blk2 = slice(h * P + DH, (h + 1) * P)
        kA[blk], kA[blk2] = kMr[r], kMi[r]
        kB[blk], kB[blk2] = -kMi[r], kMr[r]
        ka_b[blk], ka_b[blk2] = kbre[r], kbim[r]

    # AV epilogue bias: per head col [vbre-vbim ; vbre+vbim]
    av_b = np.empty(D * 2, f)
    for h in range(H):
        r = slice(h * DH, (h + 1) * DH)
        av_b[h * P: h * P + DH] = vbre[r] - vbim[r]
        av_b[h * P + DH: (h + 1) * P] = vbre[r] + vbim[r]

    bf = lambda w: _feat_major(w).astype(BF)

    wq = np.concatenate([bf(qMr.T), bf(qMi.T), bf(-qMi.T)], axis=1)
    wka0 = np.concatenate([bf(kA.T[:, 0:512]), bf(kB.T[:, 0:512])], axis=1)
    wka1 = np.concatenate([bf(kA.T[:, 512:1024]), bf(kB.T[:, 512:1024])],
                          axis=1)

    # V k-major pack: [ct*128+c, comp, (h, re|im, j)] -> [128, 2*4*1024]
    # comp0 (moving vs h_r stationary): re<-vMr, im<-vMi
    # comp1 (vs h_i): re<- -vMi, im<- vMr
    vMr_h = vMr.reshape(H, DH, D)            # [h, j, din]
    vMi_h = vMi.reshape(H, DH, D)
    wvk_np = np.empty((P, 2, DT, H, 2, DH), f)
    for ct in range(DT):
        cs = slice(ct * P, (ct + 1) * P)
        # [din_c, h, j]
        wvk_np[:, 0, ct, :, 0, :] = vMr_h[:, :, cs].transpose(2, 0, 1)
        wvk_np[:, 0, ct, :, 1, :] = vMi_h[:, :, cs].transpose(2, 0, 1)
        wvk_np[:, 1, ct, :, 0, :] = -vMi_h[:, :, cs].transpose(2, 0, 1)
        wvk_np[:, 1, ct, :, 1, :] = vMr_h[:, :, cs].transpose(2, 0, 1)
    wvk0 = np.ascontiguousarray(wvk_np[:, 0].reshape(P, -1)).astype(BF)
    wvk1 = np.ascontiguousarray(wvk_np[:, 1].reshape(P, -1)).astype(BF)

    # o-proj pack consuming head-stacked attn tiles:
    # wo[c(=head feat: j<64 re, j>=64 im), oc, h, do]
    # oc0 (x2r): j<64 -> oGr[do, h*64+j]; j>=64 -> -oGi[do, h*64+j-64]
    # oc1 (x2i): j<64 -> oGi[...];        j>=64 -> +oGr[...]
    oGr_h = oGr.reshape(D, H, DH)            # [do, h, j]
    oGi_h = oGi.reshape(D, H, DH)
    wo_np = np.empty((P, 2, H, D), f)
    wo_np[0:DH, 0] = oGr_h.transpose(2, 1, 0)      # [j, h, do]
    wo_np[DH:P, 0] = -oGi_h.transpose(2, 1, 0)
    wo_np[0:DH, 1] = oGi_h.transpose(2, 1, 0)
    wo_np[DH:P, 1] = oGr_h.transpose(2, 1, 0)
    wo0 = np.ascontiguousarray(wo_np[:, 0].reshape(P, -1)).astype(BF)
    wo1 = np.ascontiguousarray(wo_np[:, 1].reshape(P, -1)).astype(BF)

    wf1 = [np.concatenate([bf(f1Mr.T[:, j * 512:(j + 1) * 512]),
                           bf(f1Mi.T[:, j * 512:(j + 1) * 512]),
                           bf(-f1Mi.T[:, j * 512:(j + 1) * 512])], axis=1)
           for j in range(4)]
    # f2: mt-major packs, each holds the FULL K=2048 contraction for 128
    # output features: sections (a=Gr, b=Gi, c=-Gi), each [P, 16*128]
    wf2 = [np.concatenate([bf(fGr.T[:, m * P:(m + 1) * P]),
                           bf(fGi.T[:, m * P:(m + 1) * P]),
                           bf(-fGi.T[:, m * P:(m + 1) * P])], axis=1)
           for m in range(4)]

    smalls = np.concatenate([
        _col(qbre), _col(qbim), _col(-qbim),           # 0:4, 4:8, 8:12
        _col(ka_b),                                    # 12:20
        _col(av_b),                                    # 20:28
        _col(ogbre), _col(ogbim),                      # 28:32, 32:36
        _col(f1bre), _col(f1bim),                      # 36:52, 52:68
        _col(fgbre), _col(fgbim),                      # 68:72, 72:76
        np.full((P, 1), EPS, f),                       # 76
    ], axis=1)

    roll = lambda a: np.roll(a, -half * TQ, axis=0)
    xTr = np.ascontiguousarray(roll(g('x_re')[b]).T)
    xTi = np.ascontiguousarray(roll(g('x_im')[b]).T)

    im = {'xTr': xTr, 'xTi': xTi,
          'xbr': xTr.astype(BF), 'xbi': xTi.astype(BF),
          'wq': wq, 'wka0': wka0, 'wka1': wka1,
          'wvk0': wvk0, 'wvk1': wvk1, 'wo0': wo0, 'wo1': wo1,
          'smalls': smalls}
    for j in range(4):
        im[f'wf1_{j}'] = wf1[j]
        im[f'wf2_{j}'] = wf2[j]
    return im


# ----------------------------------------------------------------------------
# Device program
# ----------------------------------------------------------------------------

def build_nc(reps=1):
    nc = bass.Bass()

    xTr_d = nc.declare_dram_parameter("xTr", [D, T], F32, isOutput=False)
    xTi_d = nc.declare_dram_parameter("xTi", [D, T], F32, isOutput=False)
    xbr_d = nc.declare_dram_parameter("xbr", [D, T], BF16, isOutput=False)
    xbi_d = nc.declare_dram_parameter("xbi", [D, T], BF16, isOutput=False)
    wq_d = nc.declare_dram_parameter("wq", [P, 6144], BF16, isOutput=False)
    wka0_d = nc.declare_dram_parameter("wka0", [P, 4096], BF16, isOutput=False)
    wka1_d = nc.declare_dram_parameter("wka1", [P, 4096], BF16, isOutput=False)
    wvk_d = [nc.declare_dram_parameter(f"wvk{j}", [P, 4096], BF16,
                                       isOutput=False) for j in range(2)]
    wo_d = [nc.declare_dram_parameter(f"wo{j}", [P, 4096], BF16,
                                      isOutput=False) for j in range(2)]
    wf1_d = [nc.declare_dram_parameter(f"wf1_{j}", [P, 6144], BF16,
                                       isOutput=False) for j in range(4)]
    wf2_d = [nc.declare_dram_parameter(f"wf2_{j}", [P, 6144], BF16,
                                       isOutput=False) for j in range(4)]
    smalls_d = nc.declare_dram_parameter("smalls", [P, 77], F32, isOutput=False)
    out_d = nc.declare_dram_parameter("outT", [2, D, TQ], F32, isOutput=True)

    with tile.TileContext(nc) as tc:
        with (
            tc.tile_pool(name="persist", bufs=1) as pp,
            tc.tile_pool(name="acts", bufs=1) as ap_,
            tc.tile_pool(name="tmp", bufs=2) as tp,
            tc.tile_pool(name="attn", bufs=2) as atp,
            tc.tile_pool(name="psum", bufs=2, space="PSUM") as psp,
            tc.tile_pool(name="psum2", bufs=1, space="PSUM") as ps2,
        ):
            def emit():
                dma = nc.sync.dma_start
                gdma = nc.gpsimd.dma_start

                # ---------------- x loads first (startup latency) -----------
                xt = {}

                def xload(ch, comp):
                    t = tp.tile([P, DT, 512], BF16, tag="xt", bufs=3,
                                name=f"xt{ch}{comp}")
                    src = (xbr_d if comp == 0 else xbi_d).ap().rearrange(
                        "(o p) t -> p o t", p=P)
                    dma(t[:, 0:2, :], src[:, 0:2, ch * 512:(ch + 1) * 512])
                    dma(t[:, 2:4, :], src[:, 2:4, ch * 512:(ch + 1) * 512])
                    xt[(ch, comp)] = t

                xload(0, 0)
                xload(0, 1)

                smalls = pp.tile([P, 77], F32)
                dma(smalls, smalls_d.ap())
                b_qre, b_qim, b_nqim = smalls[:, 0:4], smalls[:, 4:8], smalls[:, 8:12]
                b_ka = smalls[:, 12:20]
                b_av = smalls[:, 20:28]
                b_ore, b_oim = smalls[:, 28:32], smalls[:, 32:36]
                b_f1re, b_f1im = smalls[:, 36:52], smalls[:, 52:68]
                b_f2re, b_f2im = smalls[:, 68:72], smalls[:, 72:76]
                eps = smalls[:, 76:77]

                ones = pp.tile([P, 1], BF16)
                nc.vector.memset(ones, 1.0)
                onesb = pp.tile([1, P], BF16)
                nc.vector.memset(onesb, 1.0)
                m8 = pp.tile([P, 1], F32)
                nc.vector.memset(m8, -8.0)

                # preload the Exp/Ln act table off the LN critical path
                tpre = tp.tile([1, 1], F32, tag="pre", bufs=1)
                nc.scalar.activation(out=tpre, in_=ones[0:1, 0:1], func=AF.Exp)

                def loadpack(src, n, eng=None):
                    wpk = pp.tile([P, 6144], BF16, tag="wpk", bufs=4, name=n)
                    d_ = eng.dma_start if eng is not None else dma
                    d_(wpk[:, 0:src.shape[1]], src.ap())
                    return wpk

                # weight ring: issue order == consumption order
                wq = loadpack(wq_d, "wq")
                xload(1, 0)
                xload(1, 1)
                wvk0 = loadpack(wvk_d[0], "wvk0", eng=nc.gpsimd)
                wvk1 = loadpack(wvk_d[1], "wvk1", eng=nc.gpsimd)
                wka0 = loadpack(wka0_d, "wka0")

                def pair_ps(name):
                    return psp.tile([P, 2, 512], F32, tag="sc2", bufs=2,
                                    name=name)

                def big_ps(name):
                    return ps2.tile([P, 2, 512], F32, tag="avt", bufs=2,
                                    name=name)

                # ---------------- LayerNorm helpers -------------------------
                def ln_stats(xtile, nm):
                    """squares (DVE) + stats matmuls -> st psum
                    rows: [0]=sum(x), [1]=sum(x^2) (raw; 1/D at evac)."""
                    q = tp.tile([P, DT, 512], BF16, tag="xq", bufs=1,
                                name=f"xq{nm}")
                    for d in range(DT):
                        nc.vector.tensor_tensor(
                            out=q[:, d, :], in0=xtile[:, d, :],
                            in1=xtile[:, d, :], op=ALU.mult)
                    st = pair_ps(f"st{nm}")
                    for d in range(DT):
                        nc.tensor.matmul(st[0:1, 0, :], ones[:, 0:1],
                                         xtile[:, d, :],
                                         start=(d == 0), stop=(d == DT - 1))
                    for d in range(DT):
                        nc.tensor.matmul(st[0:1, 1, :], ones[:, 0:1],
                                         q[:, d, :],
                                         start=(d == 0), stop=(d == DT - 1))
                    return st

                def ln_rows(st, nm):
                    """rows bf16 [1,2,512]: [0]=rstd, [1]=mu."""
                    rows = tp.tile([1, 2, 512], BF16, tag="rows", bufs=2,
                                   name=f"rows{nm}")
                    scr = tp.tile([1, 512], BF16, tag="scr", bufs=1,
                                  name=f"scr{nm}")
                    mu2 = tp.tile([1, 512], BF16, tag="mu2", bufs=1,
                                  name=f"mu2{nm}")
                    nc.scalar.activation(out=rows[0:1, 1, :], in_=st[0:1, 0, :],
                                         func=AF.Copy, scale=1.0 / D)
                    nc.scalar.activation(out=scr, in_=st[0:1, 1, :],
                                         func=AF.Copy, scale=1.0 / D)
                    nc.vector.tensor_tensor(out=mu2, in0=rows[0:1, 1, :],
                                            in1=rows[0:1, 1, :], op=ALU.mult)
                    nc.vector.tensor_tensor(out=scr, in0=scr, in1=mu2,
                                            op=ALU.subtract)
                    nc.scalar.activation(out=scr, in_=scr, func=AF.Ln,
                                         bias=eps[0:1, 0:1])
                    nc.scalar.activation(out=rows[0:1, 0, :], in_=scr,
                                         func=AF.Exp, scale=-0.5)
                    return rows

                def ln_bcast(rows, nm):
                    """broadcast (rstd, mu) to all partitions -> lnb bf16."""
                    bc = big_ps(f"bc{nm}")
                    for s in range(2):
                        nc.tensor.matmul(bc[:, s, :], onesb,
                                         rows[0:1, s, :],
                                         start=True, stop=True)
                    lnb = tp.tile([P, 2, 512], BF16, tag="lnb", bufs=2,
                                  name=f"lnb{nm}")
                    nc.scalar.activation(out=lnb, in_=bc, func=AF.Copy)
                    return lnb

                def ln_finals(lnb, xtile, nout, nm):
                    """nout_d = (x_d - mu)*rstd, split DVE/GpSimd."""
                    for d in range(DT):
                        eng = nc.vector if d < 2 else nc.gpsimd
                        ts = tp.tile([P, 512], BF16, tag=f"ts{d % 2}",
                                     bufs=1, name=f"ts{nm}{d}")
                        eng.tensor_tensor(out=ts, in0=xtile[:, d, :],
                                          in1=lnb[:, 1, :], op=ALU.subtract)
                        eng.tensor_tensor(out=nout[:, d, :], in0=ts,
                                          in1=lnb[:, 0, :], op=ALU.mult)

                nrf = ap_.tile([P, DT, T], BF16, tag="nbig1")
                nif = ap_.tile([P, DT, T], BF16, tag="nbig2")

                # ---------------- LN1 chunk 0 + rows(ch1 prepped) -----------
                stA = ln_stats(xt[(0, 0)], "A")
                stB = ln_stats(xt[(0, 1)], "B")
                rowsA = ln_rows(stA, "A")
                rowsB = ln_rows(stB, "B")
                lnbA = ln_bcast(rowsA, "A")
                lnbB = ln_bcast(rowsB, "B")
                stC = ln_stats(xt[(1, 0)], "C")
                stD = ln_stats(xt[(1, 1)], "D")
                rowsC = ln_rows(stC, "C")
                rowsD = ln_rows(stD, "D")
                ln_finals(lnbA, xt[(0, 0)], nrf[:, :, 0:512], "A")
                ln_finals(lnbB, xt[(0, 1)], nif[:, :, 0:512], "B")

                # ---------------- Q (own half) + stacks, per dtile ----------
                # LN1 chunk-1 broadcasts interleave into the Q matmul stream.
                qa, qb_, qc = (wq[:, i * 2048:(i + 1) * 2048].rearrange(
                    "p (k n) -> p k n", k=DT) for i in range(3))
                QS = []   # (QC_h, QD_h) per head
                lnbC = lnbD = None
                for d in range(DT):
                    qre_t = atp.tile([P, 512], BF16, tag="qp", bufs=6, name="qre")
                    qim_t = atp.tile([P, 512], BF16, tag="qp", bufs=6, name="qim")
                    nqim_t = atp.tile([P, 512], BF16, tag="qp", bufs=6, name="nqim")
                    qps = pair_ps("psq")
                    i = 0
                    for dk in range(DT):
                        nc.tensor.matmul(qps[:, 0, :],
                                         qa[:, dk, d * P:(d + 1) * P],
                                         nrf[:, dk, 0:512],
                                         start=(i == 0), stop=False)
                        nc.tensor.matmul(qps[:, 0, :],
                                         qc[:, dk, d * P:(d + 1) * P],
                                         nif[:, dk, 0:512],
                                         start=False, stop=(dk == DT - 1))
                        i += 1
                    i = 0
                    for dk in range(DT):
                        nc.tensor.matmul(qps[:, 1, :],
                                         qb_[:, dk, d * P:(d + 1) * P],
                                         nrf[:, dk, 0:512],
                                         start=(i == 0), stop=False)
                        nc.tensor.matmul(qps[:, 1, :],
                                         qa[:, dk, d * P:(d + 1) * P],
                                         nif[:, dk, 0:512],
                                         start=False, stop=(dk == DT - 1))
                        i += 1
                    if d == 0:
                        lnbC = ln_bcast(rowsC, "C")
                    elif d == 1:
                        lnbD = ln_bcast(rowsD, "D")
                    nc.scalar.activation(out=qre_t, in_=qps[:, 0, :],
                                         func=AF.Identity,
                                         bias=b_qre[:, d:d + 1])
                    nc.scalar.activation(out=qim_t, in_=qps[:, 1, :],
                                         func=AF.Identity,
                                         bias=b_qim[:, d:d + 1])
                    nc.scalar.activation(out=nqim_t, in_=qps[:, 1, :],
                                         func=AF.Identity,
                                         scale=-1.0, bias=b_nqim[:, d:d + 1])
                    for hh in range(2):
                        h = 2 * d + hh
                        qc_h = atp.tile([P, 512], BF16, tag="qs", bufs=8,
                                        name=f"qc{h}")
                        qd_h = atp.tile([P, 512], BF16, tag="qs", bufs=8,
                                        name=f"qd{h}")
                        sl = slice(hh * DH, hh * DH + DH)
                        gdma(qc_h[0:DH, :], qre_t[sl, :])
                        gdma(qc_h[DH:P, :], nqim_t[sl, :])
                        gdma(qd_h[0:DH, :], qim_t[sl, :])
                        gdma(qd_h[DH:P, :], qre_t[sl, :])
                        QS.append((qc_h, qd_h))

                ln_finals(lnbC, xt[(1, 0)], nrf[:, :, 512:1024], "C")
                ln_finals(lnbD, xt[(1, 1)], nif[:, :, 512:1024], "D")

                # ---------------- V direct k-major ---------------------------
                # VA[k, kt, (h: vre|vim)]
                wka1 = loadpack(wka1_d, "wka1")
                wvv = [w[:, 0:4096].rearrange("p (k n) -> p k n", k=DT)
                       for w in (wvk0, wvk1)]
                VA = ap_.tile([P, KT, 2 * D], BF16, tag="VAx")
                for kt in range(KT):
                    vps = pair_ps("psv")
                    for half in range(2):
                        i = 0
                        for comp, stat in ((0, nrf), (1, nif)):
                            for ct in range(DT):
                                nc.tensor.matmul(
                                    vps[:, half, :],
                                    stat[:, ct, kt * P:(kt + 1) * P],
                                    wvv[comp][:, ct,
                                              half * 512:(half + 1) * 512],
                                    start=(i == 0), stop=(i == 7))
                                i += 1
                    nc.scalar.activation(out=VA[:, kt, :], in_=vps,
                                         func=AF.Copy)
                VAv = VA.rearrange("p k (h s j) -> p k h s j", h=H, s=2)

                # ---------------- KA per head (full T) -----------------------
                wo0 = loadpack(wo_d[0], "wo0", eng=nc.gpsimd)
                wo1 = loadpack(wo_d[1], "wo1", eng=nc.gpsimd)
                KAh = [atp.tile([P, T], BF16, tag="kah", bufs=8,
                                name=f"ka{h}") for h in range(H)]
                for ch in range(T // 512):
                    for hp in range(H // 2):
                        kps = pair_ps("psk")
                        for sl2 in range(2):
                            h = 2 * hp + sl2
                            pk = wka0 if h < 4 else wka1
                            hl = h % 4
                            kaA = pk[:, 0:2048].rearrange(
                                "p (k n) -> p k n", k=DT)
                            kaB = pk[:, 2048:4096].rearrange(
                                "p (k n) -> p k n", k=DT)
                            i = 0
                            for m_, r_ in ((kaA, nrf), (kaB, nif)):
                                for d in range(DT):
                                    nc.tensor.matmul(
                                        kps[:, sl2, :],
                                        m_[:, d, hl * P:(hl + 1) * P],
                                        r_[:, d, ch * 512:(ch + 1) * 512],
                                        start=(i == 0), stop=(i == 7))
                                    i += 1
                        for sl2 in range(2):
                            h = 2 * hp + sl2
                            nc.scalar.activation(
                                out=KAh[h][:, ch * 512:(ch + 1) * 512],
                                in_=kps[:, sl2, :], func=AF.Identity,
                                bias=b_ka[:, h:h + 1])

                # ---------------- attention (software-pipelined heads) ------
                OT = [None] * H

                def emit_z(st):
                    """z row sums; S = exp(-ln z) broadcast, all on Act/PE."""
                    zp = big_ps(f"zp{st['h']}")
                    st['zp'] = zp
                    st['lnz'] = []
                    for cn in range(2):
                        At = st['A'][cn]
                        for kt in range(KT):
                            nc.tensor.matmul(zp[0:1, cn, :], ones[:, 0:1],
                                             At[:, kt, :],
                                             start=(kt == 0),
                                             stop=(kt == KT - 1))
                        lnz = tp.tile([1, 512], BF16, tag="rz", bufs=2,
                                      name="lnz")
                        nc.scalar.activation(out=lnz, in_=zp[0:1, cn, :],
                                             func=AF.Ln)
                        st['lnz'].append(lnz)
                    for cn in range(2):
                        nc.tensor.matmul(zp[:, cn, :], onesb, st['lnz'][cn],
                                         start=True, stop=True)

                def emit_epilogue(st):
                    """normalize + bias + combine into OT[h] (DVE)."""
                    h, avt = st['h'], st['avt']
                    SS = []
                    for cn in range(2):
                        S = atp.tile([P, 512], F32, tag="S", bufs=2,
                                     name=f"S{cn}")
                        nc.scalar.activation(out=S, in_=st['zp'][:, cn, :],
                                             func=AF.Exp, scale=-1.0)
                        SS.append(S)
                    t1 = tp.tile([P, 512], F32, tag="avt", bufs=2, name="t1")
                    t2 = tp.tile([P, 512], F32, tag="avt", bufs=2, name="t2")
                    nc.vector.tensor_tensor(out=t1, in0=avt[:, 0, :],
                                            in1=SS[0], op=ALU.mult)
                    nc.vector.tensor_tensor(out=t2, in0=avt[:, 1, :],
                                            in1=SS[1], op=ALU.mult)
                    otmp = atp.tile([P, 512], BF16, tag=f"ot{h}", bufs=1,
                                    name=f"ot{h}")
                    nc.vector.scalar_tensor_tensor(
                        out=otmp[0:DH, :], in0=t1[0:DH, :],
                        scalar=b_av[0:DH, h:h + 1], in1=t2[0:DH, :],
                        op0=ALU.add, op1=ALU.subtract)
                    nc.vector.scalar_tensor_tensor(
                        out=otmp[DH:P, :], in0=t1[DH:P, :],
                        scalar=b_av[DH:P, h:h + 1], in1=t2[DH:P, :],
                        op0=ALU.add, op1=ALU.add)
                    OT[h] = otmp

                prev = None
                for h in range(H):
                    qc_h, qd_h = QS[h]
                    ka_h = KAh[h]
                    # swapped V copy [vim|vre] for this head (GpSimd, idle)
                    vasw = atp.tile([P, KT, P], BF16, tag="vasw", bufs=2,
                                    name=f"vasw{h}")
                    vswv = vasw.rearrange("p k (s j) -> p k s j", s=2)
                    nc.gpsimd.tensor_copy(out=vswv[:, :, 0, :],
                                          in_=VAv[:, :, h, 1, :])
                    nc.gpsimd.tensor_copy(out=vswv[:, :, 1, :],
                                          in_=VAv[:, :, h, 0, :])

                    if prev is not None:
                        prev['avt'] = big_ps(f"av{prev['h']}")

                    A_re = atp.tile([P, KT, 512], BF16, tag="Are", bufs=1,
                                    name="Are")
                    A_im = atp.tile([P, KT, 512], BF16, tag="Aim", bufs=1,
                                    name="Aim")
                    for ci, (Qm, Atile) in enumerate(((qc_h, A_re),
                                                     (qd_h, A_im))):
                        for i in range(KT // 2):
                            if prev is not None:
                                # interleave prev head's AV matmuls
                                pav, ph = prev['avt'], prev['h']
                                pstat = (VAv[:, :, ph, :, :] if ci == 0
                                         else prev['vasw'])
                                for j in (2 * i, 2 * i + 1):
                                    lhs = (pstat[:, j, :, :] if ci == 0
                                           else pstat[:, j, :])
                                    nc.tensor.matmul(
                                        pav[:, ci, :], lhs,
                                        prev['A'][ci][:, j, :],
                                        start=(j == 0), stop=(j == KT - 1))
                            sp = pair_ps("pss")
                            for j2 in range(2):
                                kt = 2 * i + j2
                                nc.tensor.matmul(
                                    sp[:, j2, :],
                                    ka_h[:, kt * P:(kt + 1) * P], Qm,
                                    start=True, stop=True)
                            nc.scalar.activation(
                                out=Atile[:, 2 * i:2 * i + 2, :], in_=sp,
                                func=AF.Exp, bias=m8[:, 0:1])
                            if prev is not None and ci == 0 and i == 1:
                                # prev's z reduction fills the act-paced
                                # stalls mid-scores (its exps are drained)
                                emit_z(prev)
                    if prev is not None:
                        emit_epilogue(prev)
                    prev = {'h': h, 'A': (A_re, A_im), 'vasw': vasw}

                # drain the pipeline for the last head
                emit_z(prev)
                prev['avt'] = big_ps(f"av{prev['h']}")
                ph = prev['h']
                for ci in range(2):
                    pstat = VAv[:, :, ph, :, :] if ci == 0 else prev['vasw']
                    for j in range(KT):
                        lhs = pstat[:, j, :, :] if ci == 0 else pstat[:, j, :]
                        nc.tensor.matmul(prev['avt'][:, ci, :], lhs,
                                         prev['A'][ci][:, j, :],
                                         start=(j == 0), stop=(j == KT - 1))
                emit_epilogue(prev)

                # ---------------- o-proj (gated) + residual + LN2 -----------
                wov = [w[:, 0:4096].rearrange("p (h n) -> p h n", h=H)
                       for w in (wo0, wo1)]
                x2 = ap_.tile([P, 2, DT, TQ], F32, tag="VAx")
                x2r, x2i = x2[:, 0], x2[:, 1]
                n2r = ap_.tile([P, DT, TQ], BF16, tag="n2r")
                n2i = ap_.tile([P, DT, TQ], BF16, tag="n2i")
                xv_c = [src_d.ap().rearrange("(o p) t -> p o t", p=P)
                        for src_d in (xTr_d, xTi_d)]
                # residual preloads + LN2 x tiles
                xt2 = [tp.tile([P, DT, 512], BF16, tag="xt", bufs=3,
                               name=f"xt2{c}") for c in range(2)]
                xq2 = [tp.tile([P, DT, 512], BF16, tag="xq", bufs=1,
                               name=f"xq2{c}") for c in range(2)]
                xres = {}
                for mp in range(2):
                    for comp in range(2):
                        for sl2 in range(2):
                            mt = 2 * mp + sl2
                            xr_ = tp.tile([P, 512], F32, tag="xch", bufs=3,
                                          name=f"xres{comp}{mt}")
                            gdma(xr_, xv_c[comp][:, mt, 0:TQ])
                            xres[(comp, mt)] = xr_

                for mp in range(2):
                    psR = pair_ps(f"pso{mp}")
                    psI = big_ps(f"psoi{mp}")
                    for ps_, oc in ((psR, 0), (psI, 1)):
                        for sl2 in range(2):
                            mt = 2 * mp + sl2
                            for h in range(H):
                                nc.tensor.matmul(
                                    ps_[:, sl2, :],
                                    wov[oc][:, h, mt * P:(mt + 1) * P], OT[h],
                                    start=(h == 0), stop=(h == H - 1))
                    for sl2 in range(2):
                        mt = 2 * mp + sl2
                        nc.vector.scalar_tensor_tensor(
                            out=x2r[:, mt, :], in0=psR[:, sl2, :],
                            scalar=b_ore[:, mt:mt + 1], in1=xres[(0, mt)],
                            op0=ALU.add, op1=ALU.add)
                        nc.vector.scalar_tensor_tensor(
                            out=x2i[:, mt, :], in0=psI[:, sl2, :],
                            scalar=b_oim[:, mt:mt + 1], in1=xres[(1, mt)],
                            op0=ALU.add, op1=ALU.add)
                        # LN2 prep for this mt (copy to bf16 + squares)
                        for comp, src in ((0, x2r), (1, x2i)):
                            eng = nc.gpsimd if comp == 0 else nc.vector
                            eng.tensor_copy(out=xt2[comp][:, mt, :],
                                            in_=src[:, mt, :])
                            eng.tensor_tensor(out=xq2[comp][:, mt, :],
                                              in0=xt2[comp][:, mt, :],
                                              in1=xt2[comp][:, mt, :],
                                              op=ALU.mult)

                # LN2 stats (squares already done above)
                def ln_stats2(xtile, qtile, nm):
                    st = pair_ps(f"st{nm}")
                    for d in range(DT):
                        nc.tensor.matmul(st[0:1, 0, :], ones[:, 0:1],
                                         xtile[:, d, :],
                                         start=(d == 0), stop=(d == DT - 1))
                    for d in range(DT):
                        nc.tensor.matmul(st[0:1, 1, :], ones[:, 0:1],
                                         qtile[:, d, :],
                                         start=(d == 0), stop=(d == DT - 1))
                    return st

                stE = ln_stats2(xt2[0], xq2[0], "E")
                stF = ln_stats2(xt2[1], xq2[1], "F")
                rowsE = ln_rows(stE, "E")
                rowsF = ln_rows(stF, "F")
                lnbE = ln_bcast(rowsE, "E")
                lnbF = ln_bcast(rowsF, "F")
                ln_finals(lnbE, xt2[0], n2r, "E")
                ln_finals(lnbF, xt2[1], n2i, "F")

                # ---------------- MLP f1 ------------------------------------
                g1r = ap_.tile([P, MLP // P, TQ], BF16, tag="nbig1")
                g1i = ap_.tile([P, MLP // P, TQ], BF16, tag="nbig2")
                for j in range(4):
                    pk = loadpack(wf1_d[j], f"wf1_{j}")
                    f1a, f1b, f1c = (pk[:, i * 2048:(i + 1) * 2048].rearrange(
                        "p (k n) -> p k n", k=DT) for i in range(3))
                    for gi, pairs in enumerate(
                            ((((f1a, 0), (f1c, 1))), (((f1b, 0), (f1a, 1))))):
                        bias = b_f1re if gi == 0 else b_f1im
                        dst = g1r if gi == 0 else g1i
                        for mp in range(2):
                            fps = pair_ps("psf1")
                            for sl2 in range(2):
                                ml = 2 * mp + sl2
                                i = 0
                                for m_, which in pairs:
                                    r_ = n2r if which == 0 else n2i
                                    for dk in range(DT):
                                        nc.tensor.matmul(
                                            fps[:, sl2, :],
                                            m_[:, dk, ml * P:(ml + 1) * P],
                                            r_[:, dk, :],
                                            start=(i == 0), stop=(i == 7))
                                        i += 1
                            for sl2 in range(2):
                                ml = 2 * mp + sl2
                                mt = j * 4 + ml
                                nc.scalar.activation(
                                    out=dst[:, mt, :], in_=fps[:, sl2, :],
                                    func=AF.Gelu_apprx_tanh,
                                    bias=bias[:, mt:mt + 1])

                # ---------------- MLP f2 (mt-major, single weight pass) -----
                ov2 = out_d.ap().rearrange("c (o p) t -> p c o t", p=P)
                octiles = [atp.tile([P, 2, 2, 512], F32, tag=tg, bufs=1,
                                    name=f"oc{tg}")
                           for tg in ("Are", "Aim")]
                for m in range(4):
                    pk = loadpack(wf2_d[m], f"wf2_{m}", eng=nc.gpsimd)
                    f2a, f2b, f2c = (pk[:, i * 2048:(i + 1) * 2048].rearrange(
                        "p (k n) -> p k n", k=16) for i in range(3))
                    fps = pair_ps(f"psf2{m}")
                    for gi, pairs in enumerate(
                            ((((f2a, 0), (f2c, 1))), (((f2b, 0), (f2a, 1))))):
                        i = 0
                        for m_, which in pairs:
                            r_ = g1r if which == 0 else g1i
                            for kl in range(16):
                                nc.tensor.matmul(
                                    fps[:, gi, :], m_[:, kl, :],
                                    r_[:, kl, :],
                                    start=(i == 0), stop=(i == 31))
                                i += 1
                    oct = octiles[m // 2]
                    mi = m % 2
                    nc.vector.scalar_tensor_tensor(
                        out=oct[:, 0, mi, :], in0=fps[:, 0, :],
                        scalar=b_f2re[:, m:m + 1], in1=x2r[:, m, :],
                        op0=ALU.add, op1=ALU.add)
                    nc.vector.scalar_tensor_tensor(
                        out=oct[:, 1, mi, :], in0=fps[:, 1, :],
                        scalar=b_f2im[:, m:m + 1], in1=x2i[:, m, :],
                        op0=ALU.add, op1=ALU.add)
                    if mi == 1:
                        mp = m // 2
                        for c_ in range(2):
                            dma(ov2[:, c_, 2 * mp:2 * mp + 2, :],
                                oct[:, c_])

            for _rep in range(reps):
                emit()

    _split_dma_waits(nc)
    return nc


def _split_dma_waits(nc):
    """Walrus's DIRECT2D DMA encoding takes one sync wait; move extra
    waits onto a preceding sequencer EventSemaphore on the same engine."""
    f = nc.m.functions[0]
    for blk in f.blocks:
        out = []
        for ins in blk.instructions:
            si = getattr(ins, 'sync_info', None)
            tn = type(ins).__name__
            lim = 1
            if si is not None and si.on_wait and len(si.on_wait) > lim:
                waits = list(si.on_wait)
                extra = waits[:-lim]
                si.on_wait = waits[-lim:]
                k = 0
                while extra:
                    ev = mybir.InstEventSemaphore(
                        name=f"{ins.name}_wsplit{k}",
                        engine=ins.engine,
                        ins=[], outs=[],
                        sync_info=mybir.SyncInfo(on_wait=extra[:2],
                                                 on_update=[]),
                    )
                    out.append(ev)
                    extra = extra[2:]
                    k += 1
            out.append(ins)
        blk.instructions = out


_NC_CACHE = None


def _get_nc():
    global _NC_CACHE
    if _NC_CACHE is None:
        _NC_CACHE = build_nc()
    return _NC_CACHE


TRACE = False
LAST_RESULT = None


def kernel(**inputs):
    global LAST_RESULT
    nc = _get_nc()
    in_maps = []
    for c in range(NCORES):
        in_maps.append(_prep_core(inputs, c // 2, c % 2))
    res = run_bass_kernel_spmd(nc, in_maps, list(range(NCORES)),
                               trace=TRACE)
    LAST_RESULT = res
    out = np.empty((2, B, T, D), np.float32)
    for c in range(NCORES):
        b, half = c // 2, c % 2
        y = res.results[c]["outT"]          # [2, D, TQ]
        out[:, b, half * TQ:(half + 1) * TQ, :] = y.transpose(0, 2, 1)
    return out


# revision 11
# speedup vs baseline: 1.2139x; 1.0461x over previous
"""CDiT block kernel for 8 TRN2 NeuronCores.

Sharding: core c handles batch b=c//2, token half h=c%2 (512 of 1024 tokens).
Each core computes the full output for its (b, token-half) slice; K/V are
computed over the full T of the batch (duplicated within the pair), so no
cross-core collectives are needed.

Host folds adaLN modulation (scale/shift) and gates into the projection
weights/biases (per-batch constants), pre-transposes weights to [din, dout]
bf16, and pre-transposes x to feature-major [D, T] with the token axis
rolled so each core's own tokens are [0:512].

Device structure (no DMA transposes anywhere; engines kept decoupled):
- LayerNorm: stats via ones-matmuls into [1,512] psum rows; mean/meansq are
  scaled 1/D during the Act row-evacuation; var/rstd computed on the bf16
  rows (Act Ln/Exp share one table with softmax exp); ONE bf16 ones-matmul
  broadcasts (rstd, mu) to 128 partitions; finals are all-bf16 DVE/GpSimd
  tensor ops split across both engines. LN1 chunk-1 stats/rows run
  interleaved under the Q projection; LN2 under o-proj.
- Attention scores are computed K-MAJOR (stationary = host-stacked
  [Kr;Ki] tile per k-tile, moving = stacked-Q [Qr;-Qi]) in kt-PAIRS into
  [P,2,512] psums; one paired exp() activation per psum (bias -8 recenters
  so ln z fits bf16) writes A tiles [k, kt, q] in the layout AV needs.
- Softmax denominators: z = ones-matmul over A k-tiles -> ln z (Act, bf16
  row) -> broadcast via bf16 ones-matmul -> S = exp(-bcast) = 1/z.
- V is computed DIRECTLY k-major (stationary = LN'd activations as
  [c,t]-tiles, moving = host-repacked V weights) giving VA tiles
  [k, (head: vre|vim)]; the swapped copy [vim|vre] for the A_im half of
  AV is made per-head on GpSimd.
- Heads are software-pipelined: head h-1's AV matmuls are interleaved
  into head h's score stream.
- o-proj runs mt-pair-major with scalar_tensor_tensor epilogues
  ((psum+bias)+residual in one DVE op); LN2 stats overlap it.
- f2 weights are repacked mt-major on host and streamed ONCE (psum held
  over the full K=2048 contraction); epilogues are DVE stt + batched
  2-tile output DMAs.
- Weight packs ride a 4-deep shared SBUF ring, issue order == consumption
  order, split across the sync and gpsimd DMA queues so Q-stack copies
  (sync queue) never delay weight arrival.
- PSUM budget (8 banks): "sc2" [P,2,512] ring 2 + "avt" [P,2,512] ring 2.
"""

import os
import sys
import numpy as np

for _p in ("/opt/trn_rl_repo",):
    if _p not in sys.path:
        sys.path.insert(0, _p)

import ml_dtypes

import concourse.bass as bass
import concourse.mybir as mybir
import concourse.tile as tile
from concourse.bass_utils import run_bass_kernel_spmd

B, T, D, H = 4, 1024, 512, 8
DH = D // H
MLP = 4 * D
EPS = 1e-6
P = 128
DT = D // P          # 4 feature tiles
TQ = T // 2          # own tokens per core
KT = T // P          # 8 k-tiles
NCORES = 8

F32 = mybir.dt.float32
BF16 = mybir.dt.bfloat16
AF = mybir.ActivationFunctionType
ALU = mybir.AluOpType
BF = ml_dtypes.bfloat16


# ----------------------------------------------------------------------------
# Host-side prep
# ----------------------------------------------------------------------------

def _feat_major(w_t):
    """[din, dout] -> [128, din//128 * dout] with din = kt*128 + p."""
    din, dout = w_t.shape
    return np.ascontiguousarray(
        w_t.reshape(din // P, P, dout).transpose(1, 0, 2).reshape(P, -1)
    )


def _col(v):
    """[dout] -> [128, dout//128] per-partition bias layout (d = o*128+p)."""
    return np.ascontiguousarray(v.reshape(-1, P).T)


def _silu(x):
    return x / (1.0 + np.exp(-x))


def _prep_core(inputs, b, half):
    """Build the per-core input map (numpy arrays, host precomputation)."""
    f = np.float32
    g = lambda n: np.asarray(inputs[n], dtype=f)

    # adaLN on host (tiny): complex silu -> complex linear -> 6 chunks
    sr, si = _silu(g('c_re')[b]), _silu(g('c_im')[b])
    aWr, aWi = g('ada_Wr'), g('ada_Wi')
    m_re = aWr @ sr - aWi @ si + (g('ada_br') - g('ada_bi'))
    m_im = aWr @ si + aWi @ sr + (g('ada_br') + g('ada_bi'))
    sh_ar, sc_ar, g_ar, sh_mr, sc_mr, g_mr = np.split(m_re, 6)
    sh_ai, sc_ai, g_ai, sh_mi, sc_mi, g_mi = np.split(m_im, 6)

    def fold_mod(Wr, Wi, br, bi, a, bb, shr, shi):
        """Fold complex modulate diag(a+ib)+shift into complex linear."""
        Mr = Wr * a[None, :] - Wi * bb[None, :]
        Mi = Wi * a[None, :] + Wr * bb[None, :]
        bre = (br - bi) + Wr @ shr - Wi @ shi
        bim = (br + bi) + Wi @ shr + Wr @ shi
        return Mr, Mi, bre, bim

    a1, b1 = 1.0 + sc_ar, sc_ai
    a2, b2 = 1.0 + sc_mr, sc_mi

    qMr, qMi, qbre, qbim = fold_mod(g('q_Wr'), g('q_Wi'), g('q_br'), g('q_bi'),
                                    a1, b1, sh_ar, sh_ai)
    kMr, kMi, kbre, kbim = fold_mod(g('k_Wr'), g('k_Wi'), g('k_br'), g('k_bi'),
                                    a1, b1, sh_ar, sh_ai)
    vMr, vMi, vbre, vbim = fold_mod(g('v_Wr'), g('v_Wi'), g('v_br'), g('v_bi'),
                                    a1, b1, sh_ar, sh_ai)
    scale = 1.0 / np.sqrt(np.float32(DH))
    qMr, qMi, qbre, qbim = qMr * scale, qMi * scale, qbre * scale, qbim * scale

    f1Mr, f1Mi, f1bre, f1bim = fold_mod(g('f1_Wr'), g('f1_Wi'),
                                        g('f1_br'), g('f1_bi'),
                                        a2, b2, sh_mr, sh_mi)

    # o-proj with attention gate folded (row scaling by complex g_a)
    oWr, oWi = g('o_Wr'), g('o_Wi')
    oGr = g_ar[:, None] * oWr - g_ai[:, None] * oWi
    oGi = g_ai[:, None] * oWr + g_ar[:, None] * oWi
    obre, obim = g('o_br') - g('o_bi'), g('o_br') + g('o_bi')
    ogbre = g_ar * obre - g_ai * obim
    ogbim = g_ai * obre + g_ar * obim

    # f2 with MLP gate folded
    fWr, fWi = g('f2_Wr'), g('f2_Wi')
    fGr = g_mr[:, None] * fWr - g_mi[:, None] * fWi
    fGi = g_mi[:, None] * fWr + g_mr[:, None] * fWi
    fbre, fbim = g('f2_br') - g('f2_bi'), g('f2_br') + g('f2_bi')
    fgbre = g_mr * fbre - g_mi * fbim
    fgbim = g_mi * fbre + g_mr * fbim

    # KA stacked weights: out rows = per head [Kr_h(64); Ki_h(64)]
    kA = np.empty((D * 2, D), f)   # rows for nr
    kB = np.empty((D * 2, D), f)   # rows for ni
    ka_b = np.empty(D * 2, f)
    for h in range(H):
        r = slice(h * DH, (h + 1) * DH)
        blk = slice(h * P, h * P + DH)
        blk2 = slice(h * P + DH, (h + 1) * P)
        kA[blk], kA[blk2] = kMr[r], kMi[r]
        kB[blk], kB[blk2] = -kMi[r], kMr[r]
        ka_b[blk], ka_b[blk2] = kbre[r], kbim[r]

    # AV epilogue bias: per head col [vbre-vbim ; vbre+vbim]
    av_b = np.empty(D * 2, f)
    for h in range(H):
        r = slice(h * DH, (h + 1) * DH)
        av_b[h * P: h * P + DH] = vbre[r] - vbim[r]
        av_b[h * P + DH: (h + 1) * P] = vbre[r] + vbim[r]

    bf = lambda w: _feat_major(w).astype(BF)

    wq = np.concatenate([bf(qMr.T), bf(qMi.T), bf(-qMi.T)], axis=1)
    wka0 = np.concatenate([bf(kA.T[:, 0:512]), bf(kB.T[:, 0:512])], axis=1)
    wka1 = np.concatenate([bf(kA.T[:, 512:1024]), bf(kB.T[:, 512:1024])],
                          axis=1)

    # V k-major pack: [ct*128+c, comp, (h, re|im, j)] -> [128, 2*4*1024]
    # comp0 (moving vs h_r stationary): re<-vMr, im<-vMi
    # comp1 (vs h_i): re<- -vMi, im<- vMr
    vMr_h = vMr.reshape(H, DH, D)            # [h, j, din]
    vMi_h = vMi.reshape(H, DH, D)
    wvk_np = np.empty((P, 2, DT, H, 2, DH), f)
    for ct in range(DT):
        cs = slice(ct * P, (ct + 1) * P)
        # [din_c, h, j]
        wvk_np[:, 0, ct, :, 0, :] = vMr_h[:, :, cs].transpose(2, 0, 1)
        wvk_np[:, 0, ct, :, 1, :] = vMi_h[:, :, cs].transpose(2, 0, 1)
        wvk_np[:, 1, ct, :, 0, :] = -vMi_h[:, :, cs].transpose(2, 0, 1)
        wvk_np[:, 1, ct, :, 1, :] = vMr_h[:, :, cs].transpose(2, 0, 1)
    wvk0 = np.ascontiguousarray(wvk_np[:, 0].reshape(P, -1)).astype(BF)
    wvk1 = np.ascontiguousarray(wvk_np[:, 1].reshape(P, -1)).astype(BF)

    # o-proj pack consuming head-stacked attn tiles:
    # wo[c(=head feat: j<64 re, j>=64 im), oc, h, do]
    # oc0 (x2r): j<64 -> oGr[do, h*64+j]; j>=64 -> -oGi[do, h*64+j-64]
    # oc1 (x2i): j<64 -> oGi[...];        j>=64 -> +oGr[...]
    oGr_h = oGr.reshape(D, H, DH)            # [do, h, j]
    oGi_h = oGi.reshape(D, H, DH)
    wo_np = np.empty((P, 2, H, D), f)
    wo_np[0:DH, 0] = oGr_h.transpose(2, 1, 0)      # [j, h, do]
    wo_np[DH:P, 0] = -oGi_h.transpose(2, 1, 0)
    wo_np[0:DH, 1] = oGi_h.transpose(2, 1, 0)
    wo_np[DH:P, 1] = oGr_h.transpose(2, 1, 0)
    wo0 = np.ascontiguousarray(wo_np[:, 0].reshape(P, -1)).astype(BF)
    wo1 = np.ascontiguousarray(wo_np[:, 1].reshape(P, -1)).astype(BF)

    f1Ms = f1Mr + f1Mi
    wf1 = [np.concatenate([bf(f1Mr.T[:, j * 512:(j + 1) * 512]),
                           bf(f1Mi.T[:, j * 512:(j + 1) * 512]),
                           bf(f1Ms.T[:, j * 512:(j + 1) * 512])], axis=1)
           for j in range(4)]
    # f2: mt-major packs, each holds the FULL K=2048 contraction for 128
    # output features: sections (a=Gr, b=Gi, c=-Gi), each [P, 16*128]
    fGs = fGr + fGi
    wf2 = [np.concatenate([bf(fGr.T[:, m * P:(m + 1) * P]),
                           bf(fGi.T[:, m * P:(m + 1) * P]),
                           bf(fGs.T[:, m * P:(m + 1) * P])], axis=1)
           for m in range(4)]

    smalls = np.concatenate([
        _col(qbre), _col(qbim), _col(-qbim),           # 0:4, 4:8, 8:12
        _col(ka_b),                                    # 12:20
        _col(av_b),                                    # 20:28
        _col(ogbre), _col(ogbim),                      # 28:32, 32:36
        _col(f1bre), _col(f1bim),                      # 36:52, 52:68
        _col(fgbre), _col(fgbim),                      # 68:72, 72:76
        np.full((P, 1), EPS, f),                       # 76
    ], axis=1)

    roll = lambda a: np.roll(a, -half * TQ, axis=0)
    xTr = np.ascontiguousarray(roll(g('x_re')[b]).T)
    xTi = np.ascontiguousarray(roll(g('x_im')[b]).T)

    im = {'xTr': xTr, 'xTi': xTi,
          'xbr': xTr.astype(BF), 'xbi': xTi.astype(BF),
          'wq': wq, 'wka0': wka0, 'wka1': wka1,
          'wvk0': wvk0, 'wvk1': wvk1, 'wo0': wo0, 'wo1': wo1,
          'smalls': smalls}
    for j in range(4):
        im[f'wf1_{j}'] = wf1[j]
        im[f'wf2_{j}'] = wf2[j]
    return im


# ----------------------------------------------------------------------------
# Device program
# ----------------------------------------------------------------------------

def build_nc(reps=1):
    nc = bass.Bass()

    xTr_d = nc.declare_dram_parameter("xTr", [D, T], F32, isOutput=False)
    xTi_d = nc.declare_dram_parameter("xTi", [D, T], F32, isOutput=False)
    xbr_d = nc.declare_dram_parameter("xbr", [D, T], BF16, isOutput=False)
    xbi_d = nc.declare_dram_parameter("xbi", [D, T], BF16, isOutput=False)
    wq_d = nc.declare_dram_parameter("wq", [P, 6144], BF16, isOutput=False)
    wka0_d = nc.declare_dram_parameter("wka0", [P, 4096], BF16, isOutput=False)
    wka1_d = nc.declare_dram_parameter("wka1", [P, 4096], BF16, isOutput=False)
    wvk_d = [nc.declare_dram_parameter(f"wvk{j}", [P, 4096], BF16,
                                       isOutput=False) for j in range(2)]
    wo_d = [nc.declare_dram_parameter(f"wo{j}", [P, 4096], BF16,
                                      isOutput=False) for j in range(2)]
    wf1_d = [nc.declare_dram_parameter(f"wf1_{j}", [P, 6144], BF16,
                                       isOutput=False) for j in range(4)]
    wf2_d = [nc.declare_dram_parameter(f"wf2_{j}", [P, 6144], BF16,
                                       isOutput=False) for j in range(4)]
    smalls_d = nc.declare_dram_parameter("smalls", [P, 77], F32, isOutput=False)
    out_d = nc.declare_dram_parameter("outT", [2, D, TQ], F32, isOutput=True)

    with tile.TileContext(nc) as tc:
        with (
            tc.tile_pool(name="persist", bufs=1) as pp,
            tc.tile_pool(name="acts", bufs=1) as ap_,
            tc.tile_pool(name="tmp", bufs=2) as tp,
            tc.tile_pool(name="attn", bufs=2) as atp,
            tc.tile_pool(name="psum", bufs=2, space="PSUM") as psp,
            tc.tile_pool(name="psum2", bufs=1, space="PSUM") as ps2,
        ):
            def emit():
                dma = nc.sync.dma_start
                gdma = nc.gpsimd.dma_start

                # ---------------- x loads first (startup latency) -----------
                xt = {}

                def xload(ch, comp):
                    t = tp.tile([P, DT, 512], BF16, tag="xt", bufs=3,
                                name=f"xt{ch}{comp}")
                    src = (xbr_d if comp == 0 else xbi_d).ap().rearrange(
                        "(o p) t -> p o t", p=P)
                    dma(t[:, 0:2, :], src[:, 0:2, ch * 512:(ch + 1) * 512])
                    dma(t[:, 2:4, :], src[:, 2:4, ch * 512:(ch + 1) * 512])
                    xt[(ch, comp)] = t

                xload(0, 0)
                xload(0, 1)

                smalls = pp.tile([P, 77], F32)
                dma(smalls, smalls_d.ap())
                b_qre, b_qim, b_nqim = smalls[:, 0:4], smalls[:, 4:8], smalls[:, 8:12]
                b_ka = smalls[:, 12:20]
                b_av = smalls[:, 20:28]
                b_ore, b_oim = smalls[:, 28:32], smalls[:, 32:36]
                b_f1re, b_f1im = smalls[:, 36:52], smalls[:, 52:68]
                b_f2re, b_f2im = smalls[:, 68:72], smalls[:, 72:76]
                eps = smalls[:, 76:77]

                ones = pp.tile([P, 1], BF16)
                nc.vector.memset(ones, 1.0)
                onesb = pp.tile([1, P], BF16)
                nc.vector.memset(onesb, 1.0)
                m8 = pp.tile([P, 1], F32)
                nc.vector.memset(m8, -8.0)

                # preload the Exp/Ln act table off the LN critical path
                tpre = tp.tile([1, 1], F32, tag="pre", bufs=1)
                nc.scalar.activation(out=tpre, in_=ones[0:1, 0:1], func=AF.Exp)

                def loadpack(src, n, eng=None):
                    wpk = pp.tile([P, 6144], BF16, tag="wpk", bufs=4, name=n)
                    d_ = eng.dma_start if eng is not None else dma
                    d_(wpk[:, 0:src.shape[1]], src.ap())
                    return wpk

                # weight ring: issue order == consumption order
                wq = loadpack(wq_d, "wq")
                xload(1, 0)
                xload(1, 1)
                wvk0 = loadpack(wvk_d[0], "wvk0", eng=nc.gpsimd)
                wvk1 = loadpack(wvk_d[1], "wvk1", eng=nc.gpsimd)
                wka0 = loadpack(wka0_d, "wka0")

                def pair_ps(name):
                    return psp.tile([P, 2, 512], F32, tag="sc2", bufs=2,
                                    name=name)

                def big_ps(name):
                    return ps2.tile([P, 2, 512], F32, tag="avt", bufs=2,
                                    name=name)

                # ---------------- LayerNorm helpers -------------------------
                def ln_stats(xtile, nm):
                    """squares (DVE) + stats matmuls -> st psum
                    rows: [0]=sum(x), [1]=sum(x^2) (raw; 1/D at evac)."""
                    q = tp.tile([P, DT, 512], BF16, tag="xq", bufs=1,
                                name=f"xq{nm}")
                    for d in range(DT):
                        nc.vector.tensor_tensor(
                            out=q[:, d, :], in0=xtile[:, d, :],
                            in1=xtile[:, d, :], op=ALU.mult)
                    st = pair_ps(f"st{nm}")
                    for d in range(DT):
                        nc.tensor.matmul(st[0:1, 0, :], ones[:, 0:1],
                                         xtile[:, d, :],
                                         start=(d == 0), stop=(d == DT - 1))
                    for d in range(DT):
                        nc.tensor.matmul(st[0:1, 1, :], ones[:, 0:1],
                                         q[:, d, :],
                                         start=(d == 0), stop=(d == DT - 1))
                    return st

                def ln_rows(st, nm):
                    """rows bf16 [1,2,512]: [0]=rstd, [1]=mu."""
                    rows = tp.tile([1, 2, 512], BF16, tag="rows", bufs=2,
                                   name=f"rows{nm}")
                    scr = tp.tile([1, 512], BF16, tag="scr", bufs=1,
                                  name=f"scr{nm}")
                    mu2 = tp.tile([1, 512], BF16, tag="mu2", bufs=1,
                                  name=f"mu2{nm}")
                    nc.scalar.activation(out=rows[0:1, 1, :], in_=st[0:1, 0, :],
                                         func=AF.Copy, scale=1.0 / D)
                    nc.scalar.activation(out=scr, in_=st[0:1, 1, :],
                                         func=AF.Copy, scale=1.0 / D)
                    nc.vector.tensor_tensor(out=mu2, in0=rows[0:1, 1, :],
                                            in1=rows[0:1, 1, :], op=ALU.mult)
                    nc.vector.tensor_tensor(out=scr, in0=scr, in1=mu2,
                                            op=ALU.subtract)
                    nc.scalar.activation(out=scr, in_=scr, func=AF.Ln,
                                         bias=eps[0:1, 0:1])
                    nc.scalar.activation(out=rows[0:1, 0, :], in_=scr,
                                         func=AF.Exp, scale=-0.5)
                    return rows

                def ln_bcast(rows, nm):
                    """broadcast (rstd, mu) to all partitions -> lnb bf16."""
                    bc = big_ps(f"bc{nm}")
                    for s in range(2):
                        nc.tensor.matmul(bc[:, s, :], onesb,
                                         rows[0:1, s, :],
                                         start=True, stop=True)
                    lnb = tp.tile([P, 2, 512], BF16, tag="lnb", bufs=2,
                                  name=f"lnb{nm}")
                    nc.scalar.activation(out=lnb, in_=bc, func=AF.Copy)
                    return lnb

                def ln_finals(lnb, xtile, nout, nm):
                    """nout_d = (x_d - mu)*rstd, split DVE/GpSimd."""
                    for d in range(DT):
                        eng = nc.vector if d < 2 else nc.gpsimd
                        ts = tp.tile([P, 512], BF16, tag=f"ts{d % 2}",
                                     bufs=1, name=f"ts{nm}{d}")
                        eng.tensor_tensor(out=ts, in0=xtile[:, d, :],
                                          in1=lnb[:, 1, :], op=ALU.subtract)
                        eng.tensor_tensor(out=nout[:, d, :], in0=ts,
                                          in1=lnb[:, 0, :], op=ALU.mult)

                nrf = ap_.tile([P, DT, T], BF16, tag="nbig1")
                nif = ap_.tile([P, DT, T], BF16, tag="nbig2")

                # ---------------- LN1 chunk 0 + rows(ch1 prepped) -----------
                stA = ln_stats(xt[(0, 0)], "A")
                stB = ln_stats(xt[(0, 1)], "B")
                rowsA = ln_rows(stA, "A")
                rowsB = ln_rows(stB, "B")
                lnbA = ln_bcast(rowsA, "A")
                lnbB = ln_bcast(rowsB, "B")
                stC = ln_stats(xt[(1, 0)], "C")
                stD = ln_stats(xt[(1, 1)], "D")
                rowsC = ln_rows(stC, "C")
                rowsD = ln_rows(stD, "D")
                ln_finals(lnbA, xt[(0, 0)], nrf[:, :, 0:512], "A")
                ln_finals(lnbB, xt[(0, 1)], nif[:, :, 0:512], "B")

                # ---------------- Q (own half) + stacks, per dtile ----------
                # LN1 chunk-1 broadcasts interleave into the Q matmul stream.
                qa, qb_, qc = (wq[:, i * 2048:(i + 1) * 2048].rearrange(
                    "p (k n) -> p k n", k=DT) for i in range(3))
                QS = []   # (QC_h, QD_h) per head
                lnbC = lnbD = None
                for d in range(DT):
                    qre_t = atp.tile([P, 512], BF16, tag="qp", bufs=6, name="qre")
                    qim_t = atp.tile([P, 512], BF16, tag="qp", bufs=6, name="qim")
                    nqim_t = atp.tile([P, 512], BF16, tag="qp", bufs=6, name="nqim")
                    qps = pair_ps("psq")
                    i = 0
                    for dk in range(DT):
                        nc.tensor.matmul(qps[:, 0, :],
                                         qa[:, dk, d * P:(d + 1) * P],
                                         nrf[:, dk, 0:512],
                                         start=(i == 0), stop=False)
                        nc.tensor.matmul(qps[:, 0, :],
                                         qc[:, dk, d * P:(d + 1) * P],
                                         nif[:, dk, 0:512],
                                         start=False, stop=(dk == DT - 1))
                        i += 1
                    i = 0
                    for dk in range(DT):
                        nc.tensor.matmul(qps[:, 1, :],
                                         qb_[:, dk, d * P:(d + 1) * P],
                                         nrf[:, dk, 0:512],
                                         start=(i == 0), stop=False)
                        nc.tensor.matmul(qps[:, 1, :],
                                         qa[:, dk, d * P:(d + 1) * P],
                                         nif[:, dk, 0:512],
                                         start=False, stop=(dk == DT - 1))
                        i += 1
                    if d == 0:
                        lnbC = ln_bcast(rowsC, "C")
                    elif d == 1:
                        lnbD = ln_bcast(rowsD, "D")
                    nc.scalar.activation(out=qre_t, in_=qps[:, 0, :],
                                         func=AF.Identity,
                                         bias=b_qre[:, d:d + 1])
                    nc.scalar.activation(out=qim_t, in_=qps[:, 1, :],
                                         func=AF.Identity,
                                         bias=b_qim[:, d:d + 1])
                    nc.scalar.activation(out=nqim_t, in_=qps[:, 1, :],
                                         func=AF.Identity,
                                         scale=-1.0, bias=b_nqim[:, d:d + 1])
                    for hh in range(2):
                        h = 2 * d + hh
                        qc_h = atp.tile([P, 512], BF16, tag="qs", bufs=8,
                                        name=f"qc{h}")
                        qd_h = atp.tile([P, 512], BF16, tag="qs", bufs=8,
                                        name=f"qd{h}")
                        sl = slice(hh * DH, hh * DH + DH)
                        gdma(qc_h[0:DH, :], qre_t[sl, :])
                        gdma(qc_h[DH:P, :], nqim_t[sl, :])
                        gdma(qd_h[0:DH, :], qim_t[sl, :])
                        gdma(qd_h[DH:P, :], qre_t[sl, :])
                        QS.append((qc_h, qd_h))

                ln_finals(lnbC, xt[(1, 0)], nrf[:, :, 512:1024], "C")
                ln_finals(lnbD, xt[(1, 1)], nif[:, :, 512:1024], "D")

                # ---------------- V direct k-major ---------------------------
                # VA[k, kt, (h: vre|vim)]
                wka1 = loadpack(wka1_d, "wka1")
                wvv = [w[:, 0:4096].rearrange("p (k n) -> p k n", k=DT)
                       for w in (wvk0, wvk1)]
                VA = ap_.tile([P, KT, 2 * D], BF16, tag="VAx")
                for kt in range(KT):
                    vps = pair_ps("psv")
                    for half in range(2):
                        i = 0
                        for comp, stat in ((0, nrf), (1, nif)):
                            for ct in range(DT):
                                nc.tensor.matmul(
                                    vps[:, half, :],
                                    stat[:, ct, kt * P:(kt + 1) * P],
                                    wvv[comp][:, ct,
                                              half * 512:(half + 1) * 512],
                                    start=(i == 0), stop=(i == 7))
                                i += 1
                    nc.scalar.activation(out=VA[:, kt, :], in_=vps,
                                         func=AF.Copy)
                VAv = VA.rearrange("p k (h s j) -> p k h s j", h=H, s=2)

                # ---------------- KA per head (full T) -----------------------
                wo0 = loadpack(wo_d[0], "wo0", eng=nc.gpsimd)
                wo1 = loadpack(wo_d[1], "wo1", eng=nc.gpsimd)
                KAh = [atp.tile([P, T], BF16, tag="kah", bufs=8,
                                name=f"ka{h}") for h in range(H)]
                for ch in range(T // 512):
                    for hp in range(H // 2):
                        kps = pair_ps("psk")
                        for sl2 in range(2):
                            h = 2 * hp + sl2
                            pk = wka0 if h < 4 else wka1
                            hl = h % 4
                            kaA = pk[:, 0:2048].rearrange(
                                "p (k n) -> p k n", k=DT)
                            kaB = pk[:, 2048:4096].rearrange(
                                "p (k n) -> p k n", k=DT)
                            i = 0
                            for m_, r_ in ((kaA, nrf), (kaB, nif)):
                                for d in range(DT):
                                    nc.tensor.matmul(
                                        kps[:, sl2, :],
                                        m_[:, d, hl * P:(hl + 1) * P],
                                        r_[:, d, ch * 512:(ch + 1) * 512],
                                        start=(i == 0), stop=(i == 7))
                                    i += 1
                        for sl2 in range(2):
                            h = 2 * hp + sl2
                            nc.scalar.activation(
                                out=KAh[h][:, ch * 512:(ch + 1) * 512],
                                in_=kps[:, sl2, :], func=AF.Identity,
                                bias=b_ka[:, h:h + 1])

                # ---------------- attention (software-pipelined heads) ------
                OT = [None] * H

                def emit_z(st):
                    """z row sums; S = exp(-ln z) broadcast, all on Act/PE."""
                    zp = big_ps(f"zp{st['h']}")
                    st['zp'] = zp
                    st['lnz'] = []
                    for cn in range(2):
                        At = st['A'][cn]
                        for kt in range(KT):
                            nc.tensor.matmul(zp[0:1, cn, :], ones[:, 0:1],
                                             At[:, kt, :],
                                             start=(kt == 0),
                                             stop=(kt == KT - 1))
                        lnz = tp.tile([1, 512], BF16, tag="rz", bufs=2,
                                      name="lnz")
                        nc.scalar.activation(out=lnz, in_=zp[0:1, cn, :],
                                             func=AF.Ln)
                        st['lnz'].append(lnz)
                    for cn in range(2):
                        nc.tensor.matmul(zp[:, cn, :], onesb, st['lnz'][cn],
                                         start=True, stop=True)

                def emit_epilogue(st):
                    """normalize + bias + combine into OT[h] (DVE)."""
                    h, avt = st['h'], st['avt']
                    SS = []
                    for cn in range(2):
                        S = atp.tile([P, 512], F32, tag="S", bufs=2,
                                     name=f"S{cn}")
                        nc.scalar.activation(out=S, in_=st['zp'][:, cn, :],
                                             func=AF.Exp, scale=-1.0)
                        SS.append(S)
                    t1 = tp.tile([P, 512], F32, tag="avt", bufs=2, name="t1")
                    t2 = tp.tile([P, 512], F32, tag="avt", bufs=2, name="t2")
                    nc.vector.tensor_tensor(out=t1, in0=avt[:, 0, :],
                                            in1=SS[0], op=ALU.mult)
                    nc.vector.tensor_tensor(out=t2, in0=avt[:, 1, :],
                                            in1=SS[1], op=ALU.mult)
                    otmp = atp.tile([P, 512], BF16, tag=f"ot{h}", bufs=1,
                                    name=f"ot{h}")
                    nc.vector.scalar_tensor_tensor(
                        out=otmp[0:DH, :], in0=t1[0:DH, :],
                        scalar=b_av[0:DH, h:h + 1], in1=t2[0:DH, :],
                        op0=ALU.add, op1=ALU.subtract)
                    nc.vector.scalar_tensor_tensor(
                        out=otmp[DH:P, :], in0=t1[DH:P, :],
                        scalar=b_av[DH:P, h:h + 1], in1=t2[DH:P, :],
                        op0=ALU.add, op1=ALU.add)
                    OT[h] = otmp

                prev = None
                for h in range(H):
                    qc_h, qd_h = QS[h]
                    ka_h = KAh[h]
                    # swapped V copy [vim|vre] for this head (GpSimd, idle)
                    vasw = atp.tile([P, KT, P], BF16, tag="vasw", bufs=2,
                                    name=f"vasw{h}")
                    vswv = vasw.rearrange("p k (s j) -> p k s j", s=2)
                    nc.gpsimd.tensor_copy(out=vswv[:, :, 0, :],
                                          in_=VAv[:, :, h, 1, :])
                    nc.gpsimd.tensor_copy(out=vswv[:, :, 1, :],
                                          in_=VAv[:, :, h, 0, :])

                    if prev is not None:
                        prev['avt'] = big_ps(f"av{prev['h']}")

                    A_re = atp.tile([P, KT, 512], BF16, tag="Are", bufs=1,
                                    name="Are")
                    A_im = atp.tile([P, KT, 512], BF16, tag="Aim", bufs=1,
                                    name="Aim")
                    for ci, (Qm, Atile) in enumerate(((qc_h, A_re),
                                                     (qd_h, A_im))):
                        for i in range(KT // 2):
                            if prev is not None:
                                # interleave prev head's AV matmuls
                                pav, ph = prev['avt'], prev['h']
                                pstat = (VAv[:, :, ph, :, :] if ci == 0
                                         else prev['vasw'])
                                for j in (2 * i, 2 * i + 1):
                                    lhs = (pstat[:, j, :, :] if ci == 0
                                           else pstat[:, j, :])
                                    nc.tensor.matmul(
                                        pav[:, ci, :], lhs,
                                        prev['A'][ci][:, j, :],
                                        start=(j == 0), stop=(j == KT - 1))
                            sp = pair_ps("pss")
                            for j2 in range(2):
                                kt = 2 * i + j2
                                nc.tensor.matmul(
                                    sp[:, j2, :],
                                    ka_h[:, kt * P:(kt + 1) * P], Qm,
                                    start=True, stop=True)
                            nc.scalar.activation(
                                out=Atile[:, 2 * i:2 * i + 2, :], in_=sp,
                                func=AF.Exp, bias=m8[:, 0:1])
                            if prev is not None and ci == 0 and i == 1:
                                # prev's z reduction fills the act-paced
                                # stalls mid-scores (its exps are drained)
                                emit_z(prev)
                    if prev is not None:
                        emit_epilogue(prev)
                    prev = {'h': h, 'A': (A_re, A_im), 'vasw': vasw}

                # drain the pipeline for the last head
                emit_z(prev)
                prev['avt'] = big_ps(f"av{prev['h']}")
                ph = prev['h']
                for ci in range(2):
                    pstat = VAv[:, :, ph, :, :] if ci == 0 else prev['vasw']
                    for j in range(KT):
                        lhs = pstat[:, j, :, :] if ci == 0 else pstat[:, j, :]
                        nc.tensor.matmul(prev['avt'][:, ci, :], lhs,
                                         prev['A'][ci][:, j, :],
                                         start=(j == 0), stop=(j == KT - 1))
                emit_epilogue(prev)

                # ---------------- o-proj (gated) + residual + LN2 -----------
                wov = [w[:, 0:4096].rearrange("p (h n) -> p h n", h=H)
                       for w in (wo0, wo1)]
                x2 = ap_.tile([P, 2, DT, TQ], F32, tag="VAx")
                x2r, x2i = x2[:, 0], x2[:, 1]
                n2r = ap_.tile([P, DT, TQ], BF16, tag="n2r")
                n2i = ap_.tile([P, DT, TQ], BF16, tag="n2i")
                xv_c = [src_d.ap().rearrange("(o p) t -> p o t", p=P)
                        for src_d in (xTr_d, xTi_d)]
                # residual preloads + LN2 x tiles
                xt2 = [tp.tile([P, DT, 512], BF16, tag="xt", bufs=3,
                               name=f"xt2{c}") for c in range(2)]
                xq2 = [tp.tile([P, DT, 512], BF16, tag="xq", bufs=1,
                               name=f"xq2{c}") for c in range(2)]
                xres = {}
                for mp in range(2):
                    for comp in range(2):
                        for sl2 in range(2):
                            mt = 2 * mp + sl2
                            xr_ = tp.tile([P, 512], F32, tag="xch", bufs=3,
                                          name=f"xres{comp}{mt}")
                            gdma(xr_, xv_c[comp][:, mt, 0:TQ])
                            xres[(comp, mt)] = xr_

                for mp in range(2):
                    psR = pair_ps(f"pso{mp}")
                    psI = big_ps(f"psoi{mp}")
                    for ps_, oc in ((psR, 0), (psI, 1)):
                        for sl2 in range(2):
                            mt = 2 * mp + sl2
                            for h in range(H):
                                nc.tensor.matmul(
                                    ps_[:, sl2, :],
                                    wov[oc][:, h, mt * P:(mt + 1) * P], OT[h],
                                    start=(h == 0), stop=(h == H - 1))
                    for sl2 in range(2):
                        mt = 2 * mp + sl2
                        nc.vector.scalar_tensor_tensor(
                            out=x2r[:, mt, :], in0=psR[:, sl2, :],
                            scalar=b_ore[:, mt:mt + 1], in1=xres[(0, mt)],
                            op0=ALU.add, op1=ALU.add)
                        nc.vector.scalar_tensor_tensor(
                            out=x2i[:, mt, :], in0=psI[:, sl2, :],
                            scalar=b_oim[:, mt:mt + 1], in1=xres[(1, mt)],
                            op0=ALU.add, op1=ALU.add)
                        # LN2 prep for this mt (copy to bf16 + squares)
                        for comp, src in ((0, x2r), (1, x2i)):
                            eng = nc.gpsimd if comp == 0 else nc.vector
                            eng.tensor_copy(out=xt2[comp][:, mt, :],
                                            in_=src[:, mt, :])
                            eng.tensor_tensor(out=xq2[comp][:, mt, :],
                                              in0=xt2[comp][:, mt, :],
                                              in1=xt2[comp][:, mt, :],
                                              op=ALU.mult)

                # LN2 stats (squares already done above)
                def ln_stats2(xtile, qtile, nm):
                    st = pair_ps(f"st{nm}")
                    for d in range(DT):
                        nc.tensor.matmul(st[0:1, 0, :], ones[:, 0:1],
                                         xtile[:, d, :],
                                         start=(d == 0), stop=(d == DT - 1))
                    for d in range(DT):
                        nc.tensor.matmul(st[0:1, 1, :], ones[:, 0:1],
                                         qtile[:, d, :],
                                         start=(d == 0), stop=(d == DT - 1))
                    return st

                stE = ln_stats2(xt2[0], xq2[0], "E")
                stF = ln_stats2(xt2[1], xq2[1], "F")
                rowsE = ln_rows(stE, "E")
                rowsF = ln_rows(stF, "F")
                lnbE = ln_bcast(rowsE, "E")
                lnbF = ln_bcast(rowsF, "F")
                ln_finals(lnbE, xt2[0], n2r, "E")
                ln_finals(lnbF, xt2[1], n2i, "F")

                # ---------------- MLP f1 (Karatsuba: 12 matmuls / mt) -------
                n2s = tp.tile([P, DT, 512], BF16, tag="xq", bufs=1, name="n2s")
                for d in range(DT):
                    eng = nc.vector if d % 2 == 0 else nc.gpsimd
                    eng.tensor_tensor(out=n2s[:, d, :], in0=n2r[:, d, :],
                                      in1=n2i[:, d, :], op=ALU.add)
                g1r = ap_.tile([P, MLP // P, TQ], BF16, tag="nbig1")
                g1i = ap_.tile([P, MLP // P, TQ], BF16, tag="nbig2")
                for j in range(4):
                    pk = loadpack(wf1_d[j], f"wf1_{j}")
                    f1a, f1b, f1s = (pk[:, i * 2048:(i + 1) * 2048].rearrange(
                        "p (k n) -> p k n", k=DT) for i in range(3))
                    for ml in range(4):
                        mt = j * 4 + ml
                        p12 = pair_ps("psf1")
                        p3 = big_ps("psf1b")
                        for sec, r_, ps_ in ((f1a, n2r, p12[:, 0, :]),
                                             (f1b, n2i, p12[:, 1, :]),
                                             (f1s, n2s, p3[:, 0, :])):
                            for dk in range(DT):
                                nc.tensor.matmul(
                                    ps_, sec[:, dk, ml * P:(ml + 1) * P],
                                    r_[:, dk, :],
                                    start=(dk == 0), stop=(dk == DT - 1))
                        t1s = tp.tile([P, 512], F32, tag="xch", bufs=3,
                                      name="t1s")
                        rpre = tp.tile([P, 512], F32, tag="xch", bufs=3,
                                       name="rpre")
                        ipre = tp.tile([P, 512], F32, tag="xch", bufs=3,
                                       name="ipre")
                        nc.scalar.activation(out=t1s, in_=p12[:, 0, :],
                                             func=AF.Copy)
                        nc.vector.tensor_tensor(
                            out=rpre, in0=t1s, in1=p12[:, 1, :],
                            op=ALU.subtract)
                        nc.vector.scalar_tensor_tensor(
                            out=ipre, in0=p3[:, 0, :],
                            scalar=b_f1im[:, mt:mt + 1], in1=t1s,
                            op0=ALU.add, op1=ALU.subtract)
                        nc.vector.tensor_tensor(
                            out=ipre, in0=ipre, in1=p12[:, 1, :],
                            op=ALU.subtract)
                        nc.scalar.activation(out=g1r[:, mt, :], in_=rpre,
                                             func=AF.Gelu_apprx_tanh,
                                             bias=b_f1re[:, mt:mt + 1])
                        nc.scalar.activation(out=g1i[:, mt, :], in_=ipre,
                                             func=AF.Gelu_apprx_tanh)

                # ---------------- MLP f2 (Karatsuba, single weight pass) ----
                # g1s = g1r + g1i, fragmented across now-dead tags
                g1s_parts = [ap_.tile([P, DT, 512], BF16, tag="n2r",
                                      name="g1sa"),
                             ap_.tile([P, DT, 512], BF16, tag="n2i",
                                      name="g1sb"),
                             tp.tile([P, DT, 512], BF16, tag="xq", bufs=1,
                                     name="g1sc"),
                             tp.tile([P, DT, 512], BF16, tag="xt", bufs=3,
                                     name="g1sd")]

                def g1s(kl):
                    return g1s_parts[kl // 4][:, kl % 4, :]

                for kl in range(16):
                    nc.gpsimd.tensor_tensor(out=g1s(kl), in0=g1r[:, kl, :],
                                            in1=g1i[:, kl, :], op=ALU.add)
                ov2 = out_d.ap().rearrange("c (o p) t -> p c o t", p=P)
                octiles = [atp.tile([P, 2, 2, 512], F32, tag=tg, bufs=1,
                                    name=f"oc{tg}")
                           for tg in ("Are", "Aim")]
                for m in range(4):
                    pk = loadpack(wf2_d[m], f"wf2_{m}", eng=nc.gpsimd)
                    f2a, f2b, f2s = (pk[:, i * 2048:(i + 1) * 2048].rearrange(
                        "p (k n) -> p k n", k=16) for i in range(3))
                    fps = pair_ps(f"psf2{m}")
                    p3 = big_ps(f"psf2b{m}")
                    for gi, (sec, r_) in enumerate(((f2a, g1r), (f2b, g1i))):
                        for kl in range(16):
                            nc.tensor.matmul(
                                fps[:, gi, :], sec[:, kl, :], r_[:, kl, :],
                                start=(kl == 0), stop=(kl == 15))
                    for kl in range(16):
                        nc.tensor.matmul(
                            p3[:, 0, :], f2s[:, kl, :], g1s(kl),
                            start=(kl == 0), stop=(kl == 15))
                    oct = octiles[m // 2]
                    mi = m % 2
                    t1s = tp.tile([P, 512], F32, tag="xch", bufs=3,
                                  name="t1s2")
                    tre = tp.tile([P, 512], F32, tag="xch", bufs=3,
                                  name="tre")
                    tim = tp.tile([P, 512], F32, tag="xch", bufs=3,
                                  name="tim")
                    nc.scalar.activation(out=t1s, in_=fps[:, 0, :],
                                         func=AF.Copy)
                    nc.vector.tensor_tensor(
                        out=tre, in0=t1s, in1=fps[:, 1, :],
                        op=ALU.subtract)
                    nc.vector.scalar_tensor_tensor(
                        out=oct[:, 0, mi, :], in0=tre,
                        scalar=b_f2re[:, m:m + 1], in1=x2r[:, m, :],
                        op0=ALU.add, op1=ALU.add)
                    nc.vector.scalar_tensor_tensor(
                        out=tim, in0=p3[:, 0, :],
                        scalar=b_f2im[:, m:m + 1], in1=t1s,
                        op0=ALU.add, op1=ALU.subtract)
                    nc.vector.tensor_tensor(
                        out=tim, in0=tim, in1=fps[:, 1, :],
                        op=ALU.subtract)
                    nc.vector.tensor_tensor(
                        out=oct[:, 1, mi, :], in0=tim, in1=x2i[:, m, :],
                        op=ALU.add)
                    if mi == 1:
                        mp = m // 2
                        for c_ in range(2):
                            dma(ov2[:, c_, 2 * mp:2 * mp + 2, :],
                                oct[:, c_])

            for _rep in range(reps):
                emit()

    _split_dma_waits(nc)
    return nc


def _split_dma_waits(nc):
    """Walrus's DIRECT2D DMA encoding takes one sync wait; move extra
    waits onto a preceding sequencer EventSemaphore on the same engine."""
    f = nc.m.functions[0]
    for blk in f.blocks:
        out = []
        for ins in blk.instructions:
            si = getattr(ins, 'sync_info', None)
            tn = type(ins).__name__
            lim = 1
            if si is not None and si.on_wait and len(si.on_wait) > lim:
                waits = list(si.on_wait)
                extra = waits[:-lim]
                si.on_wait = waits[-lim:]
                k = 0
                while extra:
                    ev = mybir.InstEventSemaphore(
                        name=f"{ins.name}_wsplit{k}",
                        engine=ins.engine,
                        ins=[], outs=[],
                        sync_info=mybir.SyncInfo(on_wait=extra[:2],
                                                 on_update=[]),
                    )
                    out.append(ev)
                    extra = extra[2:]
                    k += 1
            out.append(ins)
        blk.instructions = out


_NC_CACHE = None


def _get_nc():
    global _NC_CACHE
    if _NC_CACHE is None:
        _NC_CACHE = build_nc()
    return _NC_CACHE


TRACE = False
LAST_RESULT = None


def kernel(**inputs):
    global LAST_RESULT
    nc = _get_nc()
    in_maps = []
    for c in range(NCORES):
        in_maps.append(_prep_core(inputs, c // 2, c % 2))
    res = run_bass_kernel_spmd(nc, in_maps, list(range(NCORES)),
                               trace=TRACE)
    LAST_RESULT = res
    out = np.empty((2, B, T, D), np.float32)
    for c in range(NCORES):
        b, half = c // 2, c % 2
        y = res.results[c]["outT"]          # [2, D, TQ]
        out[:, b, half * TQ:(half + 1) * TQ, :] = y.transpose(0, 2, 1)
    return out


# revision 12
# speedup vs baseline: 1.2444x; 1.0252x over previous
"""CDiT block kernel for 8 TRN2 NeuronCores.

Sharding: core c handles batch b=c//2, token half h=c%2 (512 of 1024 tokens).
Each core computes the full output for its (b, token-half) slice; K/V are
computed over the full T of the batch (duplicated within the pair), so no
cross-core collectives are needed.

Host folds adaLN modulation (scale/shift) and gates into the projection
weights/biases (per-batch constants), pre-transposes weights to [din, dout]
bf16, and pre-transposes x to feature-major [D, T] with the token axis
rolled so each core's own tokens are [0:512].

Device structure (no DMA transposes anywhere; engines kept decoupled):
- LayerNorm: stats via ones-matmuls into [1,512] psum rows; mean/meansq are
  scaled 1/D during the Act row-evacuation; var/rstd computed on the bf16
  rows (Act Ln/Exp share one table with softmax exp); ONE bf16 ones-matmul
  broadcasts (rstd, mu) to 128 partitions; finals are all-bf16 DVE/GpSimd
  tensor ops split across both engines. LN1 chunk-1 stats/rows run
  interleaved under the Q projection; LN2 under o-proj.
- Attention scores are computed K-MAJOR (stationary = host-stacked
  [Kr;Ki] tile per k-tile, moving = stacked-Q [Qr;-Qi]) in kt-PAIRS into
  [P,2,512] psums; one paired exp() activation per psum (bias -8 recenters
  so ln z fits bf16) writes A tiles [k, kt, q] in the layout AV needs.
- Softmax denominators: z = ones-matmul over A k-tiles -> ln z (Act, bf16
  row) -> broadcast via bf16 ones-matmul -> S = exp(-bcast) = 1/z.
- V is computed DIRECTLY k-major (stationary = LN'd activations as
  [c,t]-tiles, moving = host-repacked V weights) giving VA tiles
  [k, (head: vre|vim)]; the swapped copy [vim|vre] for the A_im half of
  AV is made per-head on GpSimd.
- Heads are software-pipelined: head h-1's AV matmuls are interleaved
  into head h's score stream.
- o-proj runs mt-pair-major with scalar_tensor_tensor epilogues
  ((psum+bias)+residual in one DVE op); LN2 stats overlap it.
- f2 weights are repacked mt-major on host and streamed ONCE (psum held
  over the full K=2048 contraction); epilogues are DVE stt + batched
  2-tile output DMAs.
- Weight packs ride a 4-deep shared SBUF ring, issue order == consumption
  order, split across the sync and gpsimd DMA queues so Q-stack copies
  (sync queue) never delay weight arrival.
- PSUM budget (8 banks): "sc2" [P,2,512] ring 2 + "avt" [P,2,512] ring 2.
"""

import os
import sys
import numpy as np

for _p in ("/opt/trn_rl_repo",):
    if _p not in sys.path:
        sys.path.insert(0, _p)

import ml_dtypes

import concourse.bass as bass
import concourse.mybir as mybir
import concourse.tile as tile
from concourse.bass_utils import run_bass_kernel_spmd

B, T, D, H = 4, 1024, 512, 8
DH = D // H
MLP = 4 * D
EPS = 1e-6
P = 128
DT = D // P          # 4 feature tiles
TQ = T // 2          # own tokens per core
KT = T // P          # 8 k-tiles
NCORES = 8

F32 = mybir.dt.float32
BF16 = mybir.dt.bfloat16
AF = mybir.ActivationFunctionType
ALU = mybir.AluOpType
BF = ml_dtypes.bfloat16


# ----------------------------------------------------------------------------
# Host-side prep
# ----------------------------------------------------------------------------

def _feat_major(w_t):
    """[din, dout] -> [128, din//128 * dout] with din = kt*128 + p."""
    din, dout = w_t.shape
    return np.ascontiguousarray(
        w_t.reshape(din // P, P, dout).transpose(1, 0, 2).reshape(P, -1)
    )


def _col(v):
    """[dout] -> [128, dout//128] per-partition bias layout (d = o*128+p)."""
    return np.ascontiguousarray(v.reshape(-1, P).T)


def _silu(x):
    return x / (1.0 + np.exp(-x))


def _prep_core(inputs, b, half):
    """Build the per-core input map (numpy arrays, host precomputation)."""
    f = np.float32
    g = lambda n: np.asarray(inputs[n], dtype=f)

    # adaLN on host (tiny): complex silu -> complex linear -> 6 chunks
    sr, si = _silu(g('c_re')[b]), _silu(g('c_im')[b])
    aWr, aWi = g('ada_Wr'), g('ada_Wi')
    m_re = aWr @ sr - aWi @ si + (g('ada_br') - g('ada_bi'))
    m_im = aWr @ si + aWi @ sr + (g('ada_br') + g('ada_bi'))
    sh_ar, sc_ar, g_ar, sh_mr, sc_mr, g_mr = np.split(m_re, 6)
    sh_ai, sc_ai, g_ai, sh_mi, sc_mi, g_mi = np.split(m_im, 6)

    def fold_mod(Wr, Wi, br, bi, a, bb, shr, shi):
        """Fold complex modulate diag(a+ib)+shift into complex linear."""
        Mr = Wr * a[None, :] - Wi * bb[None, :]
        Mi = Wi * a[None, :] + Wr * bb[None, :]
        bre = (br - bi) + Wr @ shr - Wi @ shi
        bim = (br + bi) + Wi @ shr + Wr @ shi
        return Mr, Mi, bre, bim

    a1, b1 = 1.0 + sc_ar, sc_ai
    a2, b2 = 1.0 + sc_mr, sc_mi

    qMr, qMi, qbre, qbim = fold_mod(g('q_Wr'), g('q_Wi'), g('q_br'), g('q_bi'),
                                    a1, b1, sh_ar, sh_ai)
    kMr, kMi, kbre, kbim = fold_mod(g('k_Wr'), g('k_Wi'), g('k_br'), g('k_bi'),
                                    a1, b1, sh_ar, sh_ai)
    vMr, vMi, vbre, vbim = fold_mod(g('v_Wr'), g('v_Wi'), g('v_br'), g('v_bi'),
                                    a1, b1, sh_ar, sh_ai)
    scale = 1.0 / np.sqrt(np.float32(DH))
    qMr, qMi, qbre, qbim = qMr * scale, qMi * scale, qbre * scale, qbim * scale

    f1Mr, f1Mi, f1bre, f1bim = fold_mod(g('f1_Wr'), g('f1_Wi'),
                                        g('f1_br'), g('f1_bi'),
                                        a2, b2, sh_mr, sh_mi)

    # o-proj with attention gate folded (row scaling by complex g_a)
    oWr, oWi = g('o_Wr'), g('o_Wi')
    oGr = g_ar[:, None] * oWr - g_ai[:, None] * oWi
    oGi = g_ai[:, None] * oWr + g_ar[:, None] * oWi
    obre, obim = g('o_br') - g('o_bi'), g('o_br') + g('o_bi')
    ogbre = g_ar * obre - g_ai * obim
    ogbim = g_ai * obre + g_ar * obim

    # f2 with MLP gate folded
    fWr, fWi = g('f2_Wr'), g('f2_Wi')
    fGr = g_mr[:, None] * fWr - g_mi[:, None] * fWi
    fGi = g_mi[:, None] * fWr + g_mr[:, None] * fWi
    fbre, fbim = g('f2_br') - g('f2_bi'), g('f2_br') + g('f2_bi')
    fgbre = g_mr * fbre - g_mi * fbim
    fgbim = g_mi * fbre + g_mr * fbim

    # KA stacked weights: out rows = per head [Kr_h(64); Ki_h(64)]
    kA = np.empty((D * 2, D), f)   # rows for nr
    kB = np.empty((D * 2, D), f)   # rows for ni
    ka_b = np.empty(D * 2, f)
    for h in range(H):
        r = slice(h * DH, (h + 1) * DH)
        blk = slice(h * P, h * P + DH)
        blk2 = slice(h * P + DH, (h + 1) * P)
        kA[blk], kA[blk2] = kMr[r], kMi[r]
        kB[blk], kB[blk2] = -kMi[r], kMr[r]
        ka_b[blk], ka_b[blk2] = kbre[r], kbim[r]

    # AV epilogue bias: per head col [vbre-vbim ; vbre+vbim]
    av_b = np.empty(D * 2, f)
    for h in range(H):
        r = slice(h * DH, (h + 1) * DH)
        av_b[h * P: h * P + DH] = vbre[r] - vbim[r]
        av_b[h * P + DH: (h + 1) * P] = vbre[r] + vbim[r]

    bf = lambda w: _feat_major(w).astype(BF)

    wq = np.concatenate([bf(qMr.T), bf(qMi.T), bf(-qMi.T)], axis=1)
    wka0 = np.concatenate([bf(kA.T[:, 0:512]), bf(kB.T[:, 0:512])], axis=1)
    wka1 = np.concatenate([bf(kA.T[:, 512:1024]), bf(kB.T[:, 512:1024])],
                          axis=1)

    # V k-major pack: [ct*128+c, comp, (h, re|im, j)] -> [128, 2*4*1024]
    # comp0 (moving vs h_r stationary): re<-vMr, im<-vMi
    # comp1 (vs h_i): re<- -vMi, im<- vMr
    vMr_h = vMr.reshape(H, DH, D)            # [h, j, din]
    vMi_h = vMi.reshape(H, DH, D)
    wvk_np = np.empty((P, 2, DT, H, 2, DH), f)
    for ct in range(DT):
        cs = slice(ct * P, (ct + 1) * P)
        # [din_c, h, j]
        wvk_np[:, 0, ct, :, 0, :] = vMr_h[:, :, cs].transpose(2, 0, 1)
        wvk_np[:, 0, ct, :, 1, :] = vMi_h[:, :, cs].transpose(2, 0, 1)
        wvk_np[:, 1, ct, :, 0, :] = -vMi_h[:, :, cs].transpose(2, 0, 1)
        wvk_np[:, 1, ct, :, 1, :] = vMr_h[:, :, cs].transpose(2, 0, 1)
    wvk0 = np.ascontiguousarray(wvk_np[:, 0].reshape(P, -1)).astype(BF)
    wvk1 = np.ascontiguousarray(wvk_np[:, 1].reshape(P, -1)).astype(BF)

    # o-proj pack consuming head-stacked attn tiles:
    # wo[c(=head feat: j<64 re, j>=64 im), oc, h, do]
    # oc0 (x2r): j<64 -> oGr[do, h*64+j]; j>=64 -> -oGi[do, h*64+j-64]
    # oc1 (x2i): j<64 -> oGi[...];        j>=64 -> +oGr[...]
    oGr_h = oGr.reshape(D, H, DH)            # [do, h, j]
    oGi_h = oGi.reshape(D, H, DH)
    wo_np = np.empty((P, 2, H, D), f)
    wo_np[0:DH, 0] = oGr_h.transpose(2, 1, 0)      # [j, h, do]
    wo_np[DH:P, 0] = -oGi_h.transpose(2, 1, 0)
    wo_np[0:DH, 1] = oGi_h.transpose(2, 1, 0)
    wo_np[DH:P, 1] = oGr_h.transpose(2, 1, 0)
    wo0 = np.ascontiguousarray(wo_np[:, 0].reshape(P, -1)).astype(BF)
    wo1 = np.ascontiguousarray(wo_np[:, 1].reshape(P, -1)).astype(BF)

    f1Ms = f1Mr + f1Mi
    wf1 = [np.concatenate([bf(f1Mr.T[:, j * 512:(j + 1) * 512]),
                           bf(f1Mi.T[:, j * 512:(j + 1) * 512]),
                           bf(f1Ms.T[:, j * 512:(j + 1) * 512])], axis=1)
           for j in range(4)]
    # f2: mt-major packs, each holds the FULL K=2048 contraction for 128
    # output features: sections (a=Gr, b=Gi, c=-Gi), each [P, 16*128]
    fGs = fGr + fGi
    wf2 = [np.concatenate([bf(fGr.T[:, m * P:(m + 1) * P]),
                           bf(fGi.T[:, m * P:(m + 1) * P]),
                           bf(fGs.T[:, m * P:(m + 1) * P])], axis=1)
           for m in range(4)]

    smalls = np.concatenate([
        _col(qbre), _col(qbim), _col(-qbim),           # 0:4, 4:8, 8:12
        _col(ka_b),                                    # 12:20
        _col(av_b),                                    # 20:28
        _col(ogbre), _col(ogbim),                      # 28:32, 32:36
        _col(f1bre), _col(f1bim),                      # 36:52, 52:68
        _col(fgbre), _col(fgbim),                      # 68:72, 72:76
        np.full((P, 1), EPS, f),                       # 76
    ], axis=1)

    roll = lambda a: np.roll(a, -half * TQ, axis=0)
    xTr = np.ascontiguousarray(roll(g('x_re')[b]).T)
    xTi = np.ascontiguousarray(roll(g('x_im')[b]).T)

    im = {'xTr': xTr, 'xTi': xTi,
          'xbr': xTr.astype(BF), 'xbi': xTi.astype(BF),
          'wq': wq, 'wka0': wka0, 'wka1': wka1,
          'wvk0': wvk0, 'wvk1': wvk1, 'wo0': wo0, 'wo1': wo1,
          'smalls': smalls}
    for j in range(4):
        im[f'wf1_{j}'] = wf1[j]
        im[f'wf2_{j}'] = wf2[j]
    return im


# ----------------------------------------------------------------------------
# Device program
# ----------------------------------------------------------------------------

def build_nc(reps=1):
    nc = bass.Bass()

    xTr_d = nc.declare_dram_parameter("xTr", [D, T], F32, isOutput=False)
    xTi_d = nc.declare_dram_parameter("xTi", [D, T], F32, isOutput=False)
    xbr_d = nc.declare_dram_parameter("xbr", [D, T], BF16, isOutput=False)
    xbi_d = nc.declare_dram_parameter("xbi", [D, T], BF16, isOutput=False)
    wq_d = nc.declare_dram_parameter("wq", [P, 6144], BF16, isOutput=False)
    wka0_d = nc.declare_dram_parameter("wka0", [P, 4096], BF16, isOutput=False)
    wka1_d = nc.declare_dram_parameter("wka1", [P, 4096], BF16, isOutput=False)
    wvk_d = [nc.declare_dram_parameter(f"wvk{j}", [P, 4096], BF16,
                                       isOutput=False) for j in range(2)]
    wo_d = [nc.declare_dram_parameter(f"wo{j}", [P, 4096], BF16,
                                      isOutput=False) for j in range(2)]
    wf1_d = [nc.declare_dram_parameter(f"wf1_{j}", [P, 6144], BF16,
                                       isOutput=False) for j in range(4)]
    wf2_d = [nc.declare_dram_parameter(f"wf2_{j}", [P, 6144], BF16,
                                       isOutput=False) for j in range(4)]
    smalls_d = nc.declare_dram_parameter("smalls", [P, 77], F32, isOutput=False)
    out_d = nc.declare_dram_parameter("outT", [2, D, TQ], F32, isOutput=True)

    with tile.TileContext(nc) as tc:
        with (
            tc.tile_pool(name="persist", bufs=1) as pp,
            tc.tile_pool(name="acts", bufs=1) as ap_,
            tc.tile_pool(name="tmp", bufs=2) as tp,
            tc.tile_pool(name="attn", bufs=2) as atp,
            tc.tile_pool(name="psum", bufs=2, space="PSUM") as psp,
            tc.tile_pool(name="psum2", bufs=1, space="PSUM") as ps2,
        ):
            def emit():
                dma = nc.sync.dma_start
                gdma = nc.gpsimd.dma_start

                # ---------------- x loads first (startup latency) -----------
                xt = {}

                def xload(ch, comp):
                    t = tp.tile([P, DT, 512], BF16, tag="xt", bufs=3,
                                name=f"xt{ch}{comp}")
                    src = (xbr_d if comp == 0 else xbi_d).ap().rearrange(
                        "(o p) t -> p o t", p=P)
                    dma(t[:, 0:2, :], src[:, 0:2, ch * 512:(ch + 1) * 512])
                    dma(t[:, 2:4, :], src[:, 2:4, ch * 512:(ch + 1) * 512])
                    xt[(ch, comp)] = t

                xload(0, 0)
                xload(0, 1)

                smalls = pp.tile([P, 77], F32)
                dma(smalls, smalls_d.ap())
                b_qre, b_qim, b_nqim = smalls[:, 0:4], smalls[:, 4:8], smalls[:, 8:12]
                b_ka = smalls[:, 12:20]
                b_av = smalls[:, 20:28]
                b_ore, b_oim = smalls[:, 28:32], smalls[:, 32:36]
                b_f1re, b_f1im = smalls[:, 36:52], smalls[:, 52:68]
                b_f2re, b_f2im = smalls[:, 68:72], smalls[:, 72:76]
                eps = smalls[:, 76:77]

                ones = pp.tile([P, 1], BF16)
                nc.vector.memset(ones, 1.0)
                onesb = pp.tile([1, P], BF16)
                nc.vector.memset(onesb, 1.0)
                m8 = pp.tile([P, 1], F32)
                nc.vector.memset(m8, -8.0)

                # preload the Exp/Ln act table off the LN critical path
                tpre = tp.tile([1, 1], F32, tag="pre", bufs=1)
                nc.scalar.activation(out=tpre, in_=ones[0:1, 0:1], func=AF.Exp)

                def loadpack(src, n, eng=None):
                    wpk = pp.tile([P, 6144], BF16, tag="wpk", bufs=4, name=n)
                    d_ = eng.dma_start if eng is not None else dma
                    d_(wpk[:, 0:src.shape[1]], src.ap())
                    return wpk

                # weight ring: issue order == consumption order
                wq = loadpack(wq_d, "wq")
                xload(1, 0)
                xload(1, 1)
                wvk0 = loadpack(wvk_d[0], "wvk0")
                wvk1 = loadpack(wvk_d[1], "wvk1")
                wka0 = loadpack(wka0_d, "wka0")

                def pair_ps(name):
                    return psp.tile([P, 2, 512], F32, tag="sc2", bufs=2,
                                    name=name)

                def big_ps(name):
                    return ps2.tile([P, 2, 512], F32, tag="avt", bufs=2,
                                    name=name)

                # ---------------- LayerNorm helpers -------------------------
                def ln_stats(xtile, nm):
                    """squares (DVE) + stats matmuls -> st psum
                    rows: [0]=sum(x), [1]=sum(x^2) (raw; 1/D at evac)."""
                    q = tp.tile([P, DT, 512], BF16, tag="xq", bufs=1,
                                name=f"xq{nm}")
                    for d in range(DT):
                        nc.vector.tensor_tensor(
                            out=q[:, d, :], in0=xtile[:, d, :],
                            in1=xtile[:, d, :], op=ALU.mult)
                    st = pair_ps(f"st{nm}")
                    for d in range(DT):
                        nc.tensor.matmul(st[0:1, 0, :], ones[:, 0:1],
                                         xtile[:, d, :],
                                         start=(d == 0), stop=(d == DT - 1))
                    for d in range(DT):
                        nc.tensor.matmul(st[0:1, 1, :], ones[:, 0:1],
                                         q[:, d, :],
                                         start=(d == 0), stop=(d == DT - 1))
                    return st

                def ln_rows(st, nm):
                    """rows bf16 [1,2,512]: [0]=rstd, [1]=mu."""
                    rows = tp.tile([1, 2, 512], BF16, tag="rows", bufs=2,
                                   name=f"rows{nm}")
                    scr = tp.tile([1, 512], BF16, tag="scr", bufs=1,
                                  name=f"scr{nm}")
                    mu2 = tp.tile([1, 512], BF16, tag="mu2", bufs=1,
                                  name=f"mu2{nm}")
                    nc.scalar.activation(out=rows[0:1, 1, :], in_=st[0:1, 0, :],
                                         func=AF.Copy, scale=1.0 / D)
                    nc.scalar.activation(out=scr, in_=st[0:1, 1, :],
                                         func=AF.Copy, scale=1.0 / D)
                    nc.vector.tensor_tensor(out=mu2, in0=rows[0:1, 1, :],
                                            in1=rows[0:1, 1, :], op=ALU.mult)
                    nc.vector.tensor_tensor(out=scr, in0=scr, in1=mu2,
                                            op=ALU.subtract)
                    nc.scalar.activation(out=scr, in_=scr, func=AF.Ln,
                                         bias=eps[0:1, 0:1])
                    nc.scalar.activation(out=rows[0:1, 0, :], in_=scr,
                                         func=AF.Exp, scale=-0.5)
                    return rows

                def ln_bcast(rows, nm):
                    """broadcast (rstd, mu) to all partitions -> lnb bf16."""
                    bc = big_ps(f"bc{nm}")
                    for s in range(2):
                        nc.tensor.matmul(bc[:, s, :], onesb,
                                         rows[0:1, s, :],
                                         start=True, stop=True)
                    lnb = tp.tile([P, 2, 512], BF16, tag="lnb", bufs=2,
                                  name=f"lnb{nm}")
                    nc.scalar.activation(out=lnb, in_=bc, func=AF.Copy)
                    return lnb

                def ln_finals(lnb, xtile, nout, nm):
                    """nout_d = (x_d - mu)*rstd, split DVE/GpSimd."""
                    for d in range(DT):
                        eng = nc.vector if d < 2 else nc.gpsimd
                        ts = tp.tile([P, 512], BF16, tag=f"ts{d % 2}",
                                     bufs=1, name=f"ts{nm}{d}")
                        eng.tensor_tensor(out=ts, in0=xtile[:, d, :],
                                          in1=lnb[:, 1, :], op=ALU.subtract)
                        eng.tensor_tensor(out=nout[:, d, :], in0=ts,
                                          in1=lnb[:, 0, :], op=ALU.mult)

                nrf = ap_.tile([P, DT, T], BF16, tag="nbig1")
                nif = ap_.tile([P, DT, T], BF16, tag="nbig2")

                # ---------------- LN1 chunk 0 + rows(ch1 prepped) -----------
                stA = ln_stats(xt[(0, 0)], "A")
                stB = ln_stats(xt[(0, 1)], "B")
                rowsA = ln_rows(stA, "A")
                rowsB = ln_rows(stB, "B")
                lnbA = ln_bcast(rowsA, "A")
                lnbB = ln_bcast(rowsB, "B")
                stC = ln_stats(xt[(1, 0)], "C")
                stD = ln_stats(xt[(1, 1)], "D")
                rowsC = ln_rows(stC, "C")
                rowsD = ln_rows(stD, "D")
                ln_finals(lnbA, xt[(0, 0)], nrf[:, :, 0:512], "A")
                ln_finals(lnbB, xt[(0, 1)], nif[:, :, 0:512], "B")

                # ---------------- Q (own half) + stacks, per dtile ----------
                # LN1 chunk-1 broadcasts interleave into the Q matmul stream.
                qa, qb_, qc = (wq[:, i * 2048:(i + 1) * 2048].rearrange(
                    "p (k n) -> p k n", k=DT) for i in range(3))
                QS = []   # (QC_h, QD_h) per head
                lnbC = lnbD = None
                for d in range(DT):
                    qre_t = atp.tile([P, 512], BF16, tag="qp", bufs=6, name="qre")
                    qim_t = atp.tile([P, 512], BF16, tag="qp", bufs=6, name="qim")
                    nqim_t = atp.tile([P, 512], BF16, tag="qp", bufs=6, name="nqim")
                    qps = pair_ps("psq")
                    i = 0
                    for dk in range(DT):
                        nc.tensor.matmul(qps[:, 0, :],
                                         qa[:, dk, d * P:(d + 1) * P],
                                         nrf[:, dk, 0:512],
                                         start=(i == 0), stop=False)
                        nc.tensor.matmul(qps[:, 0, :],
                                         qc[:, dk, d * P:(d + 1) * P],
                                         nif[:, dk, 0:512],
                                         start=False, stop=(dk == DT - 1))
                        i += 1
                    i = 0
                    for dk in range(DT):
                        nc.tensor.matmul(qps[:, 1, :],
                                         qb_[:, dk, d * P:(d + 1) * P],
                                         nrf[:, dk, 0:512],
                                         start=(i == 0), stop=False)
                        nc.tensor.matmul(qps[:, 1, :],
                                         qa[:, dk, d * P:(d + 1) * P],
                                         nif[:, dk, 0:512],
                                         start=False, stop=(dk == DT - 1))
                        i += 1
                    if d == 0:
                        lnbC = ln_bcast(rowsC, "C")
                    elif d == 1:
                        lnbD = ln_bcast(rowsD, "D")
                    nc.scalar.activation(out=qre_t, in_=qps[:, 0, :],
                                         func=AF.Identity,
                                         bias=b_qre[:, d:d + 1])
                    nc.scalar.activation(out=qim_t, in_=qps[:, 1, :],
                                         func=AF.Identity,
                                         bias=b_qim[:, d:d + 1])
                    nc.scalar.activation(out=nqim_t, in_=qps[:, 1, :],
                                         func=AF.Identity,
                                         scale=-1.0, bias=b_nqim[:, d:d + 1])
                    for hh in range(2):
                        h = 2 * d + hh
                        qc_h = atp.tile([P, 512], BF16, tag="qs", bufs=8,
                                        name=f"qc{h}")
                        qd_h = atp.tile([P, 512], BF16, tag="qs", bufs=8,
                                        name=f"qd{h}")
                        sl = slice(hh * DH, hh * DH + DH)
                        gdma(qc_h[0:DH, :], qre_t[sl, :])
                        gdma(qc_h[DH:P, :], nqim_t[sl, :])
                        gdma(qd_h[0:DH, :], qim_t[sl, :])
                        gdma(qd_h[DH:P, :], qre_t[sl, :])
                        QS.append((qc_h, qd_h))

                ln_finals(lnbC, xt[(1, 0)], nrf[:, :, 512:1024], "C")
                ln_finals(lnbD, xt[(1, 1)], nif[:, :, 512:1024], "D")

                # ---------------- V direct k-major ---------------------------
                # VA[k, kt, (h: vre|vim)]
                wka1 = loadpack(wka1_d, "wka1")
                wvv = [w[:, 0:4096].rearrange("p (k n) -> p k n", k=DT)
                       for w in (wvk0, wvk1)]
                VA = ap_.tile([P, KT, 2 * D], BF16, tag="VAx")
                for kt in range(KT):
                    vps = pair_ps("psv")
                    for half in range(2):
                        i = 0
                        for comp, stat in ((0, nrf), (1, nif)):
                            for ct in range(DT):
                                nc.tensor.matmul(
                                    vps[:, half, :],
                                    stat[:, ct, kt * P:(kt + 1) * P],
                                    wvv[comp][:, ct,
                                              half * 512:(half + 1) * 512],
                                    start=(i == 0), stop=(i == 7))
                                i += 1
                    nc.scalar.activation(out=VA[:, kt, :], in_=vps,
                                         func=AF.Copy)
                VAv = VA.rearrange("p k (h s j) -> p k h s j", h=H, s=2)

                # ---------------- KA per head (full T) -----------------------
                wo0 = loadpack(wo_d[0], "wo0", eng=nc.gpsimd)
                wo1 = loadpack(wo_d[1], "wo1", eng=nc.gpsimd)
                KAh = [atp.tile([P, T], BF16, tag="kah", bufs=8,
                                name=f"ka{h}") for h in range(H)]
                for ch in range(T // 512):
                    for hp in range(H // 2):
                        kps = pair_ps("psk")
                        for sl2 in range(2):
                            h = 2 * hp + sl2
                            pk = wka0 if h < 4 else wka1
                            hl = h % 4
                            kaA = pk[:, 0:2048].rearrange(
                                "p (k n) -> p k n", k=DT)
                            kaB = pk[:, 2048:4096].rearrange(
                                "p (k n) -> p k n", k=DT)
                            i = 0
                            for m_, r_ in ((kaA, nrf), (kaB, nif)):
                                for d in range(DT):
                                    nc.tensor.matmul(
                                        kps[:, sl2, :],
                                        m_[:, d, hl * P:(hl + 1) * P],
                                        r_[:, d, ch * 512:(ch + 1) * 512],
                                        start=(i == 0), stop=(i == 7))
                                    i += 1
                        for sl2 in range(2):
                            h = 2 * hp + sl2
                            nc.scalar.activation(
                                out=KAh[h][:, ch * 512:(ch + 1) * 512],
                                in_=kps[:, sl2, :], func=AF.Identity,
                                bias=b_ka[:, h:h + 1])

                # ---------------- attention (software-pipelined heads) ------
                OT = [None] * H

                def emit_z(st):
                    """z row sums; S = exp(-ln z) broadcast, all on Act/PE."""
                    zp = big_ps(f"zp{st['h']}")
                    st['zp'] = zp
                    st['lnz'] = []
                    for cn in range(2):
                        At = st['A'][cn]
                        for kt in range(KT):
                            nc.tensor.matmul(zp[0:1, cn, :], ones[:, 0:1],
                                             At[:, kt, :],
                                             start=(kt == 0),
                                             stop=(kt == KT - 1))
                        lnz = tp.tile([1, 512], BF16, tag="rz", bufs=2,
                                      name="lnz")
                        nc.scalar.activation(out=lnz, in_=zp[0:1, cn, :],
                                             func=AF.Ln)
                        st['lnz'].append(lnz)
                    for cn in range(2):
                        nc.tensor.matmul(zp[:, cn, :], onesb, st['lnz'][cn],
                                         start=True, stop=True)

                def emit_epilogue(st):
                    """normalize + bias + combine into OT[h] (DVE)."""
                    h, avt = st['h'], st['avt']
                    SS = []
                    for cn in range(2):
                        S = atp.tile([P, 512], F32, tag="S", bufs=2,
                                     name=f"S{cn}")
                        nc.scalar.activation(out=S, in_=st['zp'][:, cn, :],
                                             func=AF.Exp, scale=-1.0)
                        SS.append(S)
                    t1 = tp.tile([P, 512], F32, tag="avt", bufs=2, name="t1")
                    t2 = tp.tile([P, 512], F32, tag="avt", bufs=2, name="t2")
                    nc.vector.tensor_tensor(out=t1, in0=avt[:, 0, :],
                                            in1=SS[0], op=ALU.mult)
                    nc.vector.tensor_tensor(out=t2, in0=avt[:, 1, :],
                                            in1=SS[1], op=ALU.mult)
                    otmp = atp.tile([P, 512], BF16, tag=f"ot{h}", bufs=1,
                                    name=f"ot{h}")
                    nc.vector.scalar_tensor_tensor(
                        out=otmp[0:DH, :], in0=t1[0:DH, :],
                        scalar=b_av[0:DH, h:h + 1], in1=t2[0:DH, :],
                        op0=ALU.add, op1=ALU.subtract)
                    nc.vector.scalar_tensor_tensor(
                        out=otmp[DH:P, :], in0=t1[DH:P, :],
                        scalar=b_av[DH:P, h:h + 1], in1=t2[DH:P, :],
                        op0=ALU.add, op1=ALU.add)
                    OT[h] = otmp

                prev = None
                for h in range(H):
                    qc_h, qd_h = QS[h]
                    ka_h = KAh[h]
                    # swapped V copy [vim|vre] for this head (GpSimd, idle)
                    vasw = atp.tile([P, KT, P], BF16, tag="vasw", bufs=2,
                                    name=f"vasw{h}")
                    vswv = vasw.rearrange("p k (s j) -> p k s j", s=2)
                    nc.gpsimd.tensor_copy(out=vswv[:, :, 0, :],
                                          in_=VAv[:, :, h, 1, :])
                    nc.gpsimd.tensor_copy(out=vswv[:, :, 1, :],
                                          in_=VAv[:, :, h, 0, :])

                    if prev is not None:
                        prev['avt'] = big_ps(f"av{prev['h']}")

                    A_re = atp.tile([P, KT, 512], BF16, tag="Are", bufs=1,
                                    name="Are")
                    A_im = atp.tile([P, KT, 512], BF16, tag="Aim", bufs=1,
                                    name="Aim")
                    for ci, (Qm, Atile) in enumerate(((qc_h, A_re),
                                                     (qd_h, A_im))):
                        for i in range(KT // 2):
                            if prev is not None:
                                # interleave prev head's AV matmuls
                                pav, ph = prev['avt'], prev['h']
                                pstat = (VAv[:, :, ph, :, :] if ci == 0
                                         else prev['vasw'])
                                for j in (2 * i, 2 * i + 1):
                                    lhs = (pstat[:, j, :, :] if ci == 0
                                           else pstat[:, j, :])
                                    nc.tensor.matmul(
                                        pav[:, ci, :], lhs,
                                        prev['A'][ci][:, j, :],
                                        start=(j == 0), stop=(j == KT - 1))
                            sp = pair_ps("pss")
                            for j2 in range(2):
                                kt = 2 * i + j2
                                nc.tensor.matmul(
                                    sp[:, j2, :],
                                    ka_h[:, kt * P:(kt + 1) * P], Qm,
                                    start=True, stop=True)
                            nc.scalar.activation(
                                out=Atile[:, 2 * i:2 * i + 2, :], in_=sp,
                                func=AF.Exp, bias=m8[:, 0:1])
                            if prev is not None and ci == 0 and i == 1:
                                # prev's z reduction fills the act-paced
                                # stalls mid-scores (its exps are drained)
                                emit_z(prev)
                    if prev is not None:
                        emit_epilogue(prev)
                    prev = {'h': h, 'A': (A_re, A_im), 'vasw': vasw}

                # drain the pipeline for the last head
                emit_z(prev)
                prev['avt'] = big_ps(f"av{prev['h']}")
                ph = prev['h']
                for ci in range(2):
                    pstat = VAv[:, :, ph, :, :] if ci == 0 else prev['vasw']
                    for j in range(KT):
                        lhs = pstat[:, j, :, :] if ci == 0 else pstat[:, j, :]
                        nc.tensor.matmul(prev['avt'][:, ci, :], lhs,
                                         prev['A'][ci][:, j, :],
                                         start=(j == 0), stop=(j == KT - 1))
                emit_epilogue(prev)

                # ---------------- o-proj (gated) + residual + LN2 -----------
                wov = [w[:, 0:4096].rearrange("p (h n) -> p h n", h=H)
                       for w in (wo0, wo1)]
                x2 = ap_.tile([P, 2, DT, TQ], F32, tag="VAx")
                x2r, x2i = x2[:, 0], x2[:, 1]
                n2r = ap_.tile([P, DT, TQ], BF16, tag="n2r")
                n2i = ap_.tile([P, DT, TQ], BF16, tag="n2i")
                xv_c = [src_d.ap().rearrange("(o p) t -> p o t", p=P)
                        for src_d in (xTr_d, xTi_d)]
                # residual preloads + LN2 x tiles
                xt2 = [tp.tile([P, DT, 512], BF16, tag="xt", bufs=3,
                               name=f"xt2{c}") for c in range(2)]
                xq2 = [tp.tile([P, DT, 512], BF16, tag="xq", bufs=1,
                               name=f"xq2{c}") for c in range(2)]
                xres = {}
                for mp in range(2):
                    for comp in range(2):
                        for sl2 in range(2):
                            mt = 2 * mp + sl2
                            xr_ = tp.tile([P, 512], F32, tag="xch", bufs=3,
                                          name=f"xres{comp}{mt}")
                            gdma(xr_, xv_c[comp][:, mt, 0:TQ])
                            xres[(comp, mt)] = xr_

                for mp in range(2):
                    psR = pair_ps(f"pso{mp}")
                    psI = big_ps(f"psoi{mp}")
                    for ps_, oc in ((psR, 0), (psI, 1)):
                        for sl2 in range(2):
                            mt = 2 * mp + sl2
                            for h in range(H):
                                nc.tensor.matmul(
                                    ps_[:, sl2, :],
                                    wov[oc][:, h, mt * P:(mt + 1) * P], OT[h],
                                    start=(h == 0), stop=(h == H - 1))
                    for sl2 in range(2):
                        mt = 2 * mp + sl2
                        nc.vector.scalar_tensor_tensor(
                            out=x2r[:, mt, :], in0=psR[:, sl2, :],
                            scalar=b_ore[:, mt:mt + 1], in1=xres[(0, mt)],
                            op0=ALU.add, op1=ALU.add)
                        nc.vector.scalar_tensor_tensor(
                            out=x2i[:, mt, :], in0=psI[:, sl2, :],
                            scalar=b_oim[:, mt:mt + 1], in1=xres[(1, mt)],
                            op0=ALU.add, op1=ALU.add)
                        # LN2 prep for this mt (copy to bf16 + squares)
                        for comp, src in ((0, x2r), (1, x2i)):
                            eng = nc.gpsimd if comp == 0 else nc.vector
                            eng.tensor_copy(out=xt2[comp][:, mt, :],
                                            in_=src[:, mt, :])
                            eng.tensor_tensor(out=xq2[comp][:, mt, :],
                                              in0=xt2[comp][:, mt, :],
                                              in1=xt2[comp][:, mt, :],
                                              op=ALU.mult)

                # LN2 stats (squares already done above)
                def ln_stats2(xtile, qtile, nm):
                    st = pair_ps(f"st{nm}")
                    for d in range(DT):
                        nc.tensor.matmul(st[0:1, 0, :], ones[:, 0:1],
                                         xtile[:, d, :],
                                         start=(d == 0), stop=(d == DT - 1))
                    for d in range(DT):
                        nc.tensor.matmul(st[0:1, 1, :], ones[:, 0:1],
                                         qtile[:, d, :],
                                         start=(d == 0), stop=(d == DT - 1))
                    return st

                stE = ln_stats2(xt2[0], xq2[0], "E")
                stF = ln_stats2(xt2[1], xq2[1], "F")
                rowsE = ln_rows(stE, "E")
                rowsF = ln_rows(stF, "F")
                lnbE = ln_bcast(rowsE, "E")
                lnbF = ln_bcast(rowsF, "F")
                ln_finals(lnbE, xt2[0], n2r, "E")
                ln_finals(lnbF, xt2[1], n2i, "F")

                # ---------------- MLP f1 (Karatsuba: 12 matmuls / mt) -------
                n2s = tp.tile([P, DT, 512], BF16, tag="xq", bufs=1, name="n2s")
                for d in range(DT):
                    eng = nc.vector if d % 2 == 0 else nc.gpsimd
                    eng.tensor_tensor(out=n2s[:, d, :], in0=n2r[:, d, :],
                                      in1=n2i[:, d, :], op=ALU.add)
                g1r = ap_.tile([P, MLP // P, TQ], BF16, tag="nbig1")
                g1i = ap_.tile([P, MLP // P, TQ], BF16, tag="nbig2")
                for j in range(4):
                    pk = loadpack(wf1_d[j], f"wf1_{j}")
                    f1a, f1b, f1s = (pk[:, i * 2048:(i + 1) * 2048].rearrange(
                        "p (k n) -> p k n", k=DT) for i in range(3))
                    for ml in range(4):
                        mt = j * 4 + ml
                        p12 = pair_ps("psf1")
                        p3 = big_ps("psf1b")
                        for sec, r_, ps_ in ((f1a, n2r, p12[:, 0, :]),
                                             (f1b, n2i, p12[:, 1, :]),
                                             (f1s, n2s, p3[:, 0, :])):
                            for dk in range(DT):
                                nc.tensor.matmul(
                                    ps_, sec[:, dk, ml * P:(ml + 1) * P],
                                    r_[:, dk, :],
                                    start=(dk == 0), stop=(dk == DT - 1))
                        t1s = tp.tile([P, 512], F32, tag="xch", bufs=3,
                                      name="t1s")
                        rpre = tp.tile([P, 512], F32, tag="xch", bufs=3,
                                       name="rpre")
                        ipre = tp.tile([P, 512], F32, tag="xch", bufs=3,
                                       name="ipre")
                        nc.scalar.activation(out=t1s, in_=p12[:, 0, :],
                                             func=AF.Copy)
                        nc.vector.tensor_tensor(
                            out=rpre, in0=t1s, in1=p12[:, 1, :],
                            op=ALU.subtract)
                        nc.vector.scalar_tensor_tensor(
                            out=ipre, in0=p3[:, 0, :],
                            scalar=b_f1im[:, mt:mt + 1], in1=t1s,
                            op0=ALU.add, op1=ALU.subtract)
                        nc.vector.tensor_tensor(
                            out=ipre, in0=ipre, in1=p12[:, 1, :],
                            op=ALU.subtract)
                        nc.scalar.activation(out=g1r[:, mt, :], in_=rpre,
                                             func=AF.Gelu_apprx_tanh,
                                             bias=b_f1re[:, mt:mt + 1])
                        nc.scalar.activation(out=g1i[:, mt, :], in_=ipre,
                                             func=AF.Gelu_apprx_tanh)

                # ---------------- MLP f2 (Karatsuba, single weight pass) ----
                # g1s = g1r + g1i, fragmented across now-dead tags
                g1s_parts = [ap_.tile([P, DT, 512], BF16, tag="n2r",
                                      name="g1sa"),
                             ap_.tile([P, DT, 512], BF16, tag="n2i",
                                      name="g1sb"),
                             tp.tile([P, DT, 512], BF16, tag="xq", bufs=1,
                                     name="g1sc"),
                             tp.tile([P, DT, 512], BF16, tag="xt", bufs=3,
                                     name="g1sd")]

                def g1s(kl):
                    return g1s_parts[kl // 4][:, kl % 4, :]

                wf2p = [loadpack(wf2_d[m], f"wf2_{m}", eng=nc.gpsimd)
                        for m in range(4)]
                for kl in range(16):
                    nc.gpsimd.tensor_tensor(out=g1s(kl), in0=g1r[:, kl, :],
                                            in1=g1i[:, kl, :], op=ALU.add)
                ov2 = out_d.ap().rearrange("c (o p) t -> p c o t", p=P)
                octiles = [atp.tile([P, 2, 2, 512], F32, tag=tg, bufs=1,
                                    name=f"oc{tg}")
                           for tg in ("Are", "Aim")]
                for m in range(4):
                    pk = wf2p[m]
                    f2a, f2b, f2s = (pk[:, i * 2048:(i + 1) * 2048].rearrange(
                        "p (k n) -> p k n", k=16) for i in range(3))
                    fps = pair_ps(f"psf2{m}")
                    p3 = big_ps(f"psf2b{m}")
                    for gi, (sec, r_) in enumerate(((f2a, g1r), (f2b, g1i))):
                        for kl in range(16):
                            nc.tensor.matmul(
                                fps[:, gi, :], sec[:, kl, :], r_[:, kl, :],
                                start=(kl == 0), stop=(kl == 15))
                    for kl in range(16):
                        nc.tensor.matmul(
                            p3[:, 0, :], f2s[:, kl, :], g1s(kl),
                            start=(kl == 0), stop=(kl == 15))
                    oct = octiles[m // 2]
                    mi = m % 2
                    t1s = tp.tile([P, 512], F32, tag="xch", bufs=3,
                                  name="t1s2")
                    tre = tp.tile([P, 512], F32, tag="xch", bufs=3,
                                  name="tre")
                    tim = tp.tile([P, 512], F32, tag="xch", bufs=3,
                                  name="tim")
                    nc.scalar.activation(out=t1s, in_=fps[:, 0, :],
                                         func=AF.Copy)
                    nc.vector.tensor_tensor(
                        out=tre, in0=t1s, in1=fps[:, 1, :],
                        op=ALU.subtract)
                    nc.vector.scalar_tensor_tensor(
                        out=oct[:, 0, mi, :], in0=tre,
                        scalar=b_f2re[:, m:m + 1], in1=x2r[:, m, :],
                        op0=ALU.add, op1=ALU.add)
                    nc.vector.scalar_tensor_tensor(
                        out=tim, in0=p3[:, 0, :],
                        scalar=b_f2im[:, m:m + 1], in1=t1s,
                        op0=ALU.add, op1=ALU.subtract)
                    nc.vector.tensor_tensor(
                        out=tim, in0=tim, in1=fps[:, 1, :],
                        op=ALU.subtract)
                    nc.vector.tensor_tensor(
                        out=oct[:, 1, mi, :], in0=tim, in1=x2i[:, m, :],
                        op=ALU.add)
                    if mi == 1:
                        mp = m // 2
                        for c_ in range(2):
                            dma(ov2[:, c_, 2 * mp:2 * mp + 2, :],
                                oct[:, c_])

            for _rep in range(reps):
                emit()

    _split_dma_waits(nc)
    return nc


def _split_dma_waits(nc):
    """Walrus's DIRECT2D DMA encoding takes one sync wait; move extra
    waits onto a preceding sequencer EventSemaphore on the same engine."""
    f = nc.m.functions[0]
    for blk in f.blocks:
        out = []
        for ins in blk.instructions:
            si = getattr(ins, 'sync_info', None)
            tn = type(ins).__name__
            lim = 1
            if si is not None and si.on_wait and len(si.on_wait) > lim:
                waits = list(si.on_wait)
                extra = waits[:-lim]
                si.on_wait = waits[-lim:]
                k = 0
                while extra:
                    ev = mybir.InstEventSemaphore(
                        name=f"{ins.name}_wsplit{k}",
                        engine=ins.engine,
                        ins=[], outs=[],
                        sync_info=mybir.SyncInfo(on_wait=extra[:2],
                                                 on_update=[]),
                    )
                    out.append(ev)
                    extra = extra[2:]
                    k += 1
            out.append(ins)
        blk.instructions = out


_NC_CACHE = None


def _get_nc():
    global _NC_CACHE
    if _NC_CACHE is None:
        _NC_CACHE = build_nc()
    return _NC_CACHE


TRACE = False
LAST_RESULT = None


def kernel(**inputs):
    global LAST_RESULT
    nc = _get_nc()
    in_maps = []
    for c in range(NCORES):
        in_maps.append(_prep_core(inputs, c // 2, c % 2))
    res = run_bass_kernel_spmd(nc, in_maps, list(range(NCORES)),
                               trace=TRACE)
    LAST_RESULT = res
    out = np.empty((2, B, T, D), np.float32)
    for c in range(NCORES):
        b, half = c // 2, c % 2
        y = res.results[c]["outT"]          # [2, D, TQ]
        out[:, b, half * TQ:(half + 1) * TQ, :] = y.transpose(0, 2, 1)
    return out


# revision 14
# speedup vs baseline: 1.2593x; 1.0120x over previous
"""CDiT block kernel for 8 TRN2 NeuronCores.

Sharding: core c handles batch b=c//2, token half h=c%2 (512 of 1024 tokens).
Each core computes the full output for its (b, token-half) slice; K/V are
computed over the full T of the batch (duplicated within the pair), so no
cross-core collectives are needed.

Host folds adaLN modulation (scale/shift) and gates into the projection
weights/biases (per-batch constants), pre-transposes weights to [din, dout]
bf16, and pre-transposes x to feature-major [D, T] with the token axis
rolled so each core's own tokens are [0:512].

Device structure (no DMA transposes anywhere; engines kept decoupled):
- LayerNorm: stats via ones-matmuls into [1,512] psum rows; mean/meansq are
  scaled 1/D during the Act row-evacuation; var/rstd computed on the bf16
  rows (Act Ln/Exp share one table with softmax exp); ONE bf16 ones-matmul
  broadcasts (rstd, mu) to 128 partitions; finals are all-bf16 DVE/GpSimd
  tensor ops split across both engines. LN1 chunk-1 stats/rows run
  interleaved under the Q projection; LN2 under o-proj.
- Attention scores are computed K-MAJOR (stationary = host-stacked
  [Kr;Ki] tile per k-tile, moving = stacked-Q [Qr;-Qi]) in kt-PAIRS into
  [P,2,512] psums; one paired exp() activation per psum (bias -8 recenters
  so ln z fits bf16) writes A tiles [k, kt, q] in the layout AV needs.
- Softmax denominators: z = ones-matmul over A k-tiles -> ln z (Act, bf16
  row) -> broadcast via bf16 ones-matmul -> S = exp(-bcast) = 1/z.
- V is computed DIRECTLY k-major (stationary = LN'd activations as
  [c,t]-tiles, moving = host-repacked V weights) giving VA tiles
  [k, (head: vre|vim)]; the swapped copy [vim|vre] for the A_im half of
  AV is made per-head on GpSimd.
- Heads are software-pipelined: head h-1's AV matmuls are interleaved
  into head h's score stream.
- o-proj runs mt-pair-major with scalar_tensor_tensor epilogues
  ((psum+bias)+residual in one DVE op); LN2 stats overlap it.
- f2 weights are repacked mt-major on host and streamed ONCE (psum held
  over the full K=2048 contraction); epilogues are DVE stt + batched
  2-tile output DMAs.
- Weight packs ride a 4-deep shared SBUF ring, issue order == consumption
  order, split across the sync and gpsimd DMA queues so Q-stack copies
  (sync queue) never delay weight arrival.
- PSUM budget (8 banks): "sc2" [P,2,512] ring 2 + "avt" [P,2,512] ring 2.
"""

import os
import sys
import numpy as np

for _p in ("/opt/trn_rl_repo",):
    if _p not in sys.path:
        sys.path.insert(0, _p)

import ml_dtypes

import concourse.bass as bass
import concourse.mybir as mybir
import concourse.tile as tile
from concourse.bass_utils import run_bass_kernel_spmd

B, T, D, H = 4, 1024, 512, 8
DH = D // H
MLP = 4 * D
EPS = 1e-6
P = 128
DT = D // P          # 4 feature tiles
TQ = T // 2          # own tokens per core
KT = T // P          # 8 k-tiles
NCORES = 8

F32 = mybir.dt.float32
BF16 = mybir.dt.bfloat16
AF = mybir.ActivationFunctionType
ALU = mybir.AluOpType
BF = ml_dtypes.bfloat16


# ----------------------------------------------------------------------------
# Host-side prep
# ----------------------------------------------------------------------------

def _feat_major(w_t):
    """[din, dout] -> [128, din//128 * dout] with din = kt*128 + p."""
    din, dout = w_t.shape
    return np.ascontiguousarray(
        w_t.reshape(din // P, P, dout).transpose(1, 0, 2).reshape(P, -1)
    )


def _col(v):
    """[dout] -> [128, dout//128] per-partition bias layout (d = o*128+p)."""
    return np.ascontiguousarray(v.reshape(-1, P).T)


def _silu(x):
    return x / (1.0 + np.exp(-x))


def _prep_core(inputs, b, half):
    """Build the per-core input map (numpy arrays, host precomputation)."""
    f = np.float32
    g = lambda n: np.asarray(inputs[n], dtype=f)

    # adaLN on host (tiny): complex silu -> complex linear -> 6 chunks
    sr, si = _silu(g('c_re')[b]), _silu(g('c_im')[b])
    aWr, aWi = g('ada_Wr'), g('ada_Wi')
    m_re = aWr @ sr - aWi @ si + (g('ada_br') - g('ada_bi'))
    m_im = aWr @ si + aWi @ sr + (g('ada_br') + g('ada_bi'))
    sh_ar, sc_ar, g_ar, sh_mr, sc_mr, g_mr = np.split(m_re, 6)
    sh_ai, sc_ai, g_ai, sh_mi, sc_mi, g_mi = np.split(m_im, 6)

    def fold_mod(Wr, Wi, br, bi, a, bb, shr, shi):
        """Fold complex modulate diag(a+ib)+shift into complex linear."""
        Mr = Wr * a[None, :] - Wi * bb[None, :]
        Mi = Wi * a[None, :] + Wr * bb[None, :]
        bre = (br - bi) + Wr @ shr - Wi @ shi
        bim = (br + bi) + Wi @ shr + Wr @ shi
        return Mr, Mi, bre, bim

    a1, b1 = 1.0 + sc_ar, sc_ai
    a2, b2 = 1.0 + sc_mr, sc_mi

    qMr, qMi, qbre, qbim = fold_mod(g('q_Wr'), g('q_Wi'), g('q_br'), g('q_bi'),
                                    a1, b1, sh_ar, sh_ai)
    kMr, kMi, kbre, kbim = fold_mod(g('k_Wr'), g('k_Wi'), g('k_br'), g('k_bi'),
                                    a1, b1, sh_ar, sh_ai)
    vMr, vMi, vbre, vbim = fold_mod(g('v_Wr'), g('v_Wi'), g('v_br'), g('v_bi'),
                                    a1, b1, sh_ar, sh_ai)
    scale = 1.0 / np.sqrt(np.float32(DH))
    qMr, qMi, qbre, qbim = qMr * scale, qMi * scale, qbre * scale, qbim * scale

    f1Mr, f1Mi, f1bre, f1bim = fold_mod(g('f1_Wr'), g('f1_Wi'),
                                        g('f1_br'), g('f1_bi'),
                                        a2, b2, sh_mr, sh_mi)

    # o-proj with attention gate folded (row scaling by complex g_a)
    oWr, oWi = g('o_Wr'), g('o_Wi')
    oGr = g_ar[:, None] * oWr - g_ai[:, None] * oWi
    oGi = g_ai[:, None] * oWr + g_ar[:, None] * oWi
    obre, obim = g('o_br') - g('o_bi'), g('o_br') + g('o_bi')
    ogbre = g_ar * obre - g_ai * obim
    ogbim = g_ai * obre + g_ar * obim

    # f2 with MLP gate folded
    fWr, fWi = g('f2_Wr'), g('f2_Wi')
    fGr = g_mr[:, None] * fWr - g_mi[:, None] * fWi
    fGi = g_mi[:, None] * fWr + g_mr[:, None] * fWi
    fbre, fbim = g('f2_br') - g('f2_bi'), g('f2_br') + g('f2_bi')
    fgbre = g_mr * fbre - g_mi * fbim
    fgbim = g_mi * fbre + g_mr * fbim

    # KA stacked weights: out rows = per head [Kr_h(64); Ki_h(64)]
    kA = np.empty((D * 2, D), f)   # rows for nr
    kB = np.empty((D * 2, D), f)   # rows for ni
    ka_b = np.empty(D * 2, f)
    for h in range(H):
        r = slice(h * DH, (h + 1) * DH)
        blk = slice(h * P, h * P + DH)
        blk2 = slice(h * P + DH, (h + 1) * P)
        kA[blk], kA[blk2] = kMr[r], kMi[r]
        kB[blk], kB[blk2] = -kMi[r], kMr[r]
        ka_b[blk], ka_b[blk2] = kbre[r], kbim[r]

    # AV epilogue bias: per head col [vbre-vbim ; vbre+vbim]
    av_b = np.empty(D * 2, f)
    for h in range(H):
        r = slice(h * DH, (h + 1) * DH)
        av_b[h * P: h * P + DH] = vbre[r] - vbim[r]
        av_b[h * P + DH: (h + 1) * P] = vbre[r] + vbim[r]

    bf = lambda w: _feat_major(w).astype(BF)

    wq = np.concatenate([bf(qMr.T), bf(qMi.T), bf(-qMi.T)], axis=1)
    wka0 = np.concatenate([bf(kA.T[:, 0:512]), bf(kB.T[:, 0:512])], axis=1)
    wka1 = np.concatenate([bf(kA.T[:, 512:1024]), bf(kB.T[:, 512:1024])],
                          axis=1)

    # V k-major pack: [ct*128+c, comp, (h, re|im, j)] -> [128, 2*4*1024]
    # comp0 (moving vs h_r stationary): re<-vMr, im<-vMi
    # comp1 (vs h_i): re<- -vMi, im<- vMr
    vMr_h = vMr.reshape(H, DH, D)            # [h, j, din]
    vMi_h = vMi.reshape(H, DH, D)
    wvk_np = np.empty((P, 2, DT, H, 2, DH), f)
    for ct in range(DT):
        cs = slice(ct * P, (ct + 1) * P)
        # [din_c, h, j]
        wvk_np[:, 0, ct, :, 0, :] = vMr_h[:, :, cs].transpose(2, 0, 1)
        wvk_np[:, 0, ct, :, 1, :] = vMi_h[:, :, cs].transpose(2, 0, 1)
        wvk_np[:, 1, ct, :, 0, :] = -vMi_h[:, :, cs].transpose(2, 0, 1)
        wvk_np[:, 1, ct, :, 1, :] = vMr_h[:, :, cs].transpose(2, 0, 1)
    wvk0 = np.ascontiguousarray(wvk_np[:, 0].reshape(P, -1)).astype(BF)
    wvk1 = np.ascontiguousarray(wvk_np[:, 1].reshape(P, -1)).astype(BF)

    # o-proj pack consuming head-stacked attn tiles:
    # wo[c(=head feat: j<64 re, j>=64 im), oc, h, do]
    # oc0 (x2r): j<64 -> oGr[do, h*64+j]; j>=64 -> -oGi[do, h*64+j-64]
    # oc1 (x2i): j<64 -> oGi[...];        j>=64 -> +oGr[...]
    oGr_h = oGr.reshape(D, H, DH)            # [do, h, j]
    oGi_h = oGi.reshape(D, H, DH)
    wo_np = np.empty((P, 2, H, D), f)
    wo_np[0:DH, 0] = oGr_h.transpose(2, 1, 0)      # [j, h, do]
    wo_np[DH:P, 0] = -oGi_h.transpose(2, 1, 0)
    wo_np[0:DH, 1] = oGi_h.transpose(2, 1, 0)
    wo_np[DH:P, 1] = oGr_h.transpose(2, 1, 0)
    wo0 = np.ascontiguousarray(wo_np[:, 0].reshape(P, -1)).astype(BF)
    wo1 = np.ascontiguousarray(wo_np[:, 1].reshape(P, -1)).astype(BF)

    f1Ms = f1Mr + f1Mi
    wf1 = [np.concatenate([bf(f1Mr.T[:, j * 512:(j + 1) * 512]),
                           bf(f1Mi.T[:, j * 512:(j + 1) * 512]),
                           bf(f1Ms.T[:, j * 512:(j + 1) * 512])], axis=1)
           for j in range(4)]
    # f2: mt-major packs, each holds the FULL K=2048 contraction for 128
    # output features: sections (a=Gr, b=Gi, c=-Gi), each [P, 16*128]
    fGs = fGr + fGi
    wf2 = [np.concatenate([bf(fGr.T[:, m * P:(m + 1) * P]),
                           bf(fGi.T[:, m * P:(m + 1) * P]),
                           bf(fGs.T[:, m * P:(m + 1) * P])], axis=1)
           for m in range(4)]

    smalls = np.concatenate([
        _col(qbre), _col(qbim), _col(-qbim),           # 0:4, 4:8, 8:12
        _col(ka_b),                                    # 12:20
        _col(av_b),                                    # 20:28
        _col(ogbre), _col(ogbim),                      # 28:32, 32:36
        _col(f1bre), _col(f1bim),                      # 36:52, 52:68
        _col(fgbre), _col(fgbim),                      # 68:72, 72:76
        np.full((P, 1), EPS, f),                       # 76
    ], axis=1)

    roll = lambda a: np.roll(a, -half * TQ, axis=0)
    xTr = np.ascontiguousarray(roll(g('x_re')[b]).T)
    xTi = np.ascontiguousarray(roll(g('x_im')[b]).T)

    im = {'xTr': xTr, 'xTi': xTi,
          'xbr': xTr.astype(BF), 'xbi': xTi.astype(BF),
          'wq': wq, 'wka0': wka0, 'wka1': wka1,
          'wvk0': wvk0, 'wvk1': wvk1, 'wo0': wo0, 'wo1': wo1,
          'smalls': smalls}
    for j in range(4):
        im[f'wf1_{j}'] = wf1[j]
        im[f'wf2_{j}'] = wf2[j]
    return im


# ----------------------------------------------------------------------------
# Device program
# ----------------------------------------------------------------------------

def build_nc(reps=1):
    nc = bass.Bass()

    xTr_d = nc.declare_dram_parameter("xTr", [D, T], F32, isOutput=False)
    xTi_d = nc.declare_dram_parameter("xTi", [D, T], F32, isOutput=False)
    xbr_d = nc.declare_dram_parameter("xbr", [D, T], BF16, isOutput=False)
    xbi_d = nc.declare_dram_parameter("xbi", [D, T], BF16, isOutput=False)
    wq_d = nc.declare_dram_parameter("wq", [P, 6144], BF16, isOutput=False)
    wka0_d = nc.declare_dram_parameter("wka0", [P, 4096], BF16, isOutput=False)
    wka1_d = nc.declare_dram_parameter("wka1", [P, 4096], BF16, isOutput=False)
    wvk_d = [nc.declare_dram_parameter(f"wvk{j}", [P, 4096], BF16,
                                       isOutput=False) for j in range(2)]
    wo_d = [nc.declare_dram_parameter(f"wo{j}", [P, 4096], BF16,
                                      isOutput=False) for j in range(2)]
    wf1_d = [nc.declare_dram_parameter(f"wf1_{j}", [P, 6144], BF16,
                                       isOutput=False) for j in range(4)]
    wf2_d = [nc.declare_dram_parameter(f"wf2_{j}", [P, 6144], BF16,
                                       isOutput=False) for j in range(4)]
    smalls_d = nc.declare_dram_parameter("smalls", [P, 77], F32, isOutput=False)
    out_d = nc.declare_dram_parameter("outT", [2, D, TQ], F32, isOutput=True)

    with tile.TileContext(nc) as tc:
        with (
            tc.tile_pool(name="persist", bufs=1) as pp,
            tc.tile_pool(name="acts", bufs=1) as ap_,
            tc.tile_pool(name="tmp", bufs=2) as tp,
            tc.tile_pool(name="attn", bufs=2) as atp,
            tc.tile_pool(name="psum", bufs=2, space="PSUM") as psp,
            tc.tile_pool(name="psum2", bufs=1, space="PSUM") as ps2,
        ):
            def emit():
                dma = nc.sync.dma_start
                gdma = nc.gpsimd.dma_start

                # ---------------- x loads first (startup latency) -----------
                xt = {}

                def xload(ch, comp):
                    t = tp.tile([P, DT, 512], BF16, tag="xt", bufs=3,
                                name=f"xt{ch}{comp}")
                    src = (xbr_d if comp == 0 else xbi_d).ap().rearrange(
                        "(o p) t -> p o t", p=P)
                    dma(t[:, 0:2, :], src[:, 0:2, ch * 512:(ch + 1) * 512])
                    dma(t[:, 2:4, :], src[:, 2:4, ch * 512:(ch + 1) * 512])
                    xt[(ch, comp)] = t

                xload(0, 0)
                xload(0, 1)

                smalls = pp.tile([P, 77], F32)
                dma(smalls, smalls_d.ap())
                b_qre, b_qim, b_nqim = smalls[:, 0:4], smalls[:, 4:8], smalls[:, 8:12]
                b_ka = smalls[:, 12:20]
                b_av = smalls[:, 20:28]
                b_ore, b_oim = smalls[:, 28:32], smalls[:, 32:36]
                b_f1re, b_f1im = smalls[:, 36:52], smalls[:, 52:68]
                b_f2re, b_f2im = smalls[:, 68:72], smalls[:, 72:76]
                eps = smalls[:, 76:77]

                ones = pp.tile([P, 1], BF16)
                nc.vector.memset(ones, 1.0)
                onesb = pp.tile([1, P], BF16)
                nc.vector.memset(onesb, 1.0)
                m8 = pp.tile([P, 1], F32)
                nc.vector.memset(m8, -8.0)

                # preload the Exp/Ln act table off the LN critical path
                tpre = tp.tile([1, 1], F32, tag="pre", bufs=1)
                nc.scalar.activation(out=tpre, in_=ones[0:1, 0:1], func=AF.Exp)

                def loadpack(src, n, eng=None):
                    wpk = pp.tile([P, 6144], BF16, tag="wpk", bufs=4, name=n)
                    d_ = eng.dma_start if eng is not None else dma
                    d_(wpk[:, 0:src.shape[1]], src.ap())
                    return wpk

                # weight ring: issue order == consumption order
                wq = loadpack(wq_d, "wq")
                xload(1, 0)
                xload(1, 1)
                wvk0 = loadpack(wvk_d[0], "wvk0")
                wvk1 = loadpack(wvk_d[1], "wvk1")
                wka0 = loadpack(wka0_d, "wka0")

                def pair_ps(name):
                    return psp.tile([P, 2, 512], F32, tag="sc2", bufs=2,
                                    name=name)

                def big_ps(name):
                    return ps2.tile([P, 2, 512], F32, tag="avt", bufs=2,
                                    name=name)

                # ---------------- LayerNorm helpers -------------------------
                def ln_stats(xtile, nm):
                    """squares (DVE) + stats matmuls -> st psum
                    rows: [0]=sum(x), [1]=sum(x^2) (raw; 1/D at evac)."""
                    q = tp.tile([P, DT, 512], BF16, tag="xq", bufs=1,
                                name=f"xq{nm}")
                    for d in range(DT):
                        nc.vector.tensor_tensor(
                            out=q[:, d, :], in0=xtile[:, d, :],
                            in1=xtile[:, d, :], op=ALU.mult)
                    st = pair_ps(f"st{nm}")
                    for d in range(DT):
                        nc.tensor.matmul(st[0:1, 0, :], ones[:, 0:1],
                                         xtile[:, d, :],
                                         start=(d == 0), stop=(d == DT - 1))
                    for d in range(DT):
                        nc.tensor.matmul(st[0:1, 1, :], ones[:, 0:1],
                                         q[:, d, :],
                                         start=(d == 0), stop=(d == DT - 1))
                    return st

                def ln_rows(st, nm):
                    """rows bf16 [1,2,512]: [0]=rstd, [1]=mu."""
                    rows = tp.tile([1, 2, 512], BF16, tag="rows", bufs=2,
                                   name=f"rows{nm}")
                    scr = tp.tile([1, 512], BF16, tag="scr", bufs=1,
                                  name=f"scr{nm}")
                    mu2 = tp.tile([1, 512], BF16, tag="mu2", bufs=1,
                                  name=f"mu2{nm}")
                    nc.scalar.activation(out=rows[0:1, 1, :], in_=st[0:1, 0, :],
                                         func=AF.Copy, scale=1.0 / D)
                    nc.scalar.activation(out=scr, in_=st[0:1, 1, :],
                                         func=AF.Copy, scale=1.0 / D)
                    nc.vector.tensor_tensor(out=mu2, in0=rows[0:1, 1, :],
                                            in1=rows[0:1, 1, :], op=ALU.mult)
                    nc.vector.tensor_tensor(out=scr, in0=scr, in1=mu2,
                                            op=ALU.subtract)
                    nc.scalar.activation(out=scr, in_=scr, func=AF.Ln,
                                         bias=eps[0:1, 0:1])
                    nc.scalar.activation(out=rows[0:1, 0, :], in_=scr,
                                         func=AF.Exp, scale=-0.5)
                    return rows

                def ln_bcast(rows, nm):
                    """broadcast (rstd, mu) to all partitions -> lnb bf16."""
                    bc = big_ps(f"bc{nm}")
                    for s in range(2):
                        nc.tensor.matmul(bc[:, s, :], onesb,
                                         rows[0:1, s, :],
                                         start=True, stop=True)
                    lnb = tp.tile([P, 2, 512], BF16, tag="lnb", bufs=2,
                                  name=f"lnb{nm}")
                    nc.scalar.activation(out=lnb, in_=bc, func=AF.Copy)
                    return lnb

                def ln_finals(lnb, xtile, nout, nm):
                    """nout_d = (x_d - mu)*rstd, split DVE/GpSimd."""
                    for d in range(DT):
                        eng = nc.vector if d < 2 else nc.gpsimd
                        ts = tp.tile([P, 512], BF16, tag=f"ts{d % 2}",
                                     bufs=1, name=f"ts{nm}{d}")
                        eng.tensor_tensor(out=ts, in0=xtile[:, d, :],
                                          in1=lnb[:, 1, :], op=ALU.subtract)
                        eng.tensor_tensor(out=nout[:, d, :], in0=ts,
                                          in1=lnb[:, 0, :], op=ALU.mult)

                nrf = ap_.tile([P, DT, T], BF16, tag="nbig1")
                nif = ap_.tile([P, DT, T], BF16, tag="nbig2")

                # ---------------- LN1 chunk 0 + rows(ch1 prepped) -----------
                stA = ln_stats(xt[(0, 0)], "A")
                stB = ln_stats(xt[(0, 1)], "B")
                rowsA = ln_rows(stA, "A")
                rowsB = ln_rows(stB, "B")
                lnbA = ln_bcast(rowsA, "A")
                lnbB = ln_bcast(rowsB, "B")
                stC = ln_stats(xt[(1, 0)], "C")
                stD = ln_stats(xt[(1, 1)], "D")
                rowsC = ln_rows(stC, "C")
                rowsD = ln_rows(stD, "D")
                ln_finals(lnbA, xt[(0, 0)], nrf[:, :, 0:512], "A")
                ln_finals(lnbB, xt[(0, 1)], nif[:, :, 0:512], "B")

                # ---------------- Q (own half) + stacks, per dtile ----------
                # LN1 chunk-1 broadcasts interleave into the Q matmul stream.
                qa, qb_, qc = (wq[:, i * 2048:(i + 1) * 2048].rearrange(
                    "p (k n) -> p k n", k=DT) for i in range(3))
                QS = []   # (QC_h, QD_h) per head
                lnbC = lnbD = None
                for d in range(DT):
                    qre_t = atp.tile([P, 512], BF16, tag="qp", bufs=6, name="qre")
                    qim_t = atp.tile([P, 512], BF16, tag="qp", bufs=6, name="qim")
                    nqim_t = atp.tile([P, 512], BF16, tag="qp", bufs=6, name="nqim")
                    qps = pair_ps("psq")
                    i = 0
                    for dk in range(DT):
                        nc.tensor.matmul(qps[:, 0, :],
                                         qa[:, dk, d * P:(d + 1) * P],
                                         nrf[:, dk, 0:512],
                                         start=(i == 0), stop=False)
                        nc.tensor.matmul(qps[:, 0, :],
                                         qc[:, dk, d * P:(d + 1) * P],
                                         nif[:, dk, 0:512],
                                         start=False, stop=(dk == DT - 1))
                        i += 1
                    i = 0
                    for dk in range(DT):
                        nc.tensor.matmul(qps[:, 1, :],
                                         qb_[:, dk, d * P:(d + 1) * P],
                                         nrf[:, dk, 0:512],
                                         start=(i == 0), stop=False)
                        nc.tensor.matmul(qps[:, 1, :],
                                         qa[:, dk, d * P:(d + 1) * P],
                                         nif[:, dk, 0:512],
                                         start=False, stop=(dk == DT - 1))
                        i += 1
                    if d == 0:
                        lnbC = ln_bcast(rowsC, "C")
                    elif d == 1:
                        lnbD = ln_bcast(rowsD, "D")
                    nc.scalar.activation(out=qre_t, in_=qps[:, 0, :],
                                         func=AF.Identity,
                                         bias=b_qre[:, d:d + 1])
                    nc.scalar.activation(out=qim_t, in_=qps[:, 1, :],
                                         func=AF.Identity,
                                         bias=b_qim[:, d:d + 1])
                    nc.scalar.activation(out=nqim_t, in_=qps[:, 1, :],
                                         func=AF.Identity,
                                         scale=-1.0, bias=b_nqim[:, d:d + 1])
                    for hh in range(2):
                        h = 2 * d + hh
                        qc_h = atp.tile([P, 512], BF16, tag="qs", bufs=8,
                                        name=f"qc{h}")
                        qd_h = atp.tile([P, 512], BF16, tag="qs", bufs=8,
                                        name=f"qd{h}")
                        sl = slice(hh * DH, hh * DH + DH)
                        gdma(qc_h[0:DH, :], qre_t[sl, :])
                        gdma(qc_h[DH:P, :], nqim_t[sl, :])
                        gdma(qd_h[0:DH, :], qim_t[sl, :])
                        gdma(qd_h[DH:P, :], qre_t[sl, :])
                        QS.append((qc_h, qd_h))

                ln_finals(lnbC, xt[(1, 0)], nrf[:, :, 512:1024], "C")
                ln_finals(lnbD, xt[(1, 1)], nif[:, :, 512:1024], "D")

                # ---------------- V direct k-major ---------------------------
                # VA[k, kt, (h: vre|vim)]
                wka1 = loadpack(wka1_d, "wka1")
                wvv = [w[:, 0:4096].rearrange("p (k n) -> p k n", k=DT)
                       for w in (wvk0, wvk1)]
                VA = ap_.tile([P, KT, 2 * D], BF16, tag="VAx")
                for kt in range(KT):
                    vps = pair_ps("psv")
                    for half in range(2):
                        i = 0
                        for comp, stat in ((0, nrf), (1, nif)):
                            for ct in range(DT):
                                nc.tensor.matmul(
                                    vps[:, half, :],
                                    stat[:, ct, kt * P:(kt + 1) * P],
                                    wvv[comp][:, ct,
                                              half * 512:(half + 1) * 512],
                                    start=(i == 0), stop=(i == 7))
                                i += 1
                    nc.scalar.activation(out=VA[:, kt, :], in_=vps,
                                         func=AF.Copy)
                VAv = VA.rearrange("p k (h s j) -> p k h s j", h=H, s=2)

                # ---------------- KA per head (full T) -----------------------
                wo0 = loadpack(wo_d[0], "wo0", eng=nc.gpsimd)
                wo1 = loadpack(wo_d[1], "wo1", eng=nc.gpsimd)
                KAh = [atp.tile([P, T], BF16, tag="kah", bufs=8,
                                name=f"ka{h}") for h in range(H)]
                for ch in range(T // 512):
                    for hp in range(H // 2):
                        kps = pair_ps("psk")
                        for sl2 in range(2):
                            h = 2 * hp + sl2
                            pk = wka0 if h < 4 else wka1
                            hl = h % 4
                            kaA = pk[:, 0:2048].rearrange(
                                "p (k n) -> p k n", k=DT)
                            kaB = pk[:, 2048:4096].rearrange(
                                "p (k n) -> p k n", k=DT)
                            i = 0
                            for m_, r_ in ((kaA, nrf), (kaB, nif)):
                                for d in range(DT):
                                    nc.tensor.matmul(
                                        kps[:, sl2, :],
                                        m_[:, d, hl * P:(hl + 1) * P],
                                        r_[:, d, ch * 512:(ch + 1) * 512],
                                        start=(i == 0), stop=(i == 7))
                                    i += 1
                        for sl2 in range(2):
                            h = 2 * hp + sl2
                            nc.scalar.activation(
                                out=KAh[h][:, ch * 512:(ch + 1) * 512],
                                in_=kps[:, sl2, :], func=AF.Identity,
                                bias=b_ka[:, h:h + 1])

                # ---------------- attention (software-pipelined heads) ------
                OT = [None] * H

                def emit_z(st):
                    """z row sums; S = exp(-ln z) broadcast, all on Act/PE."""
                    zp = big_ps(f"zp{st['h']}")
                    st['zp'] = zp
                    for cn in range(2):
                        At = st['A'][cn]
                        for kt in range(KT):
                            nc.tensor.matmul(zp[0:1, cn, :], ones[:, 0:1],
                                             At[:, kt, :],
                                             start=(kt == 0),
                                             stop=(kt == KT - 1))
                    lnz = tp.tile([1, 2, 512], BF16, tag="rz", bufs=1,
                                  name="lnz")
                    nc.scalar.activation(out=lnz, in_=zp[0:1, :, :],
                                         func=AF.Ln)
                    st['lnz'] = lnz
                    for cn in range(2):
                        nc.tensor.matmul(zp[:, cn, :], onesb,
                                         lnz[0:1, cn, :],
                                         start=True, stop=True)

                def emit_epilogue(st):
                    """normalize + bias + combine into OT[h] (DVE)."""
                    h, avt = st['h'], st['avt']
                    S = atp.tile([P, 2, 512], F32, tag="S", bufs=1,
                                 name="S")
                    nc.scalar.activation(out=S, in_=st['zp'],
                                         func=AF.Exp, scale=-1.0)
                    t1 = tp.tile([P, 512], F32, tag="avt", bufs=2, name="t1")
                    t2 = tp.tile([P, 512], F32, tag="avt", bufs=2, name="t2")
                    nc.vector.tensor_tensor(out=t1, in0=avt[:, 0, :],
                                            in1=S[:, 0, :], op=ALU.mult)
                    nc.vector.tensor_tensor(out=t2, in0=avt[:, 1, :],
                                            in1=S[:, 1, :], op=ALU.mult)
                    otmp = atp.tile([P, 512], BF16, tag=f"ot{h}", bufs=1,
                                    name=f"ot{h}")
                    nc.vector.scalar_tensor_tensor(
                        out=otmp[0:DH, :], in0=t1[0:DH, :],
                        scalar=b_av[0:DH, h:h + 1], in1=t2[0:DH, :],
                        op0=ALU.add, op1=ALU.subtract)
                    nc.vector.scalar_tensor_tensor(
                        out=otmp[DH:P, :], in0=t1[DH:P, :],
                        scalar=b_av[DH:P, h:h + 1], in1=t2[DH:P, :],
                        op0=ALU.add, op1=ALU.add)
                    OT[h] = otmp

                prev = None
                for h in range(H):
                    qc_h, qd_h = QS[h]
                    ka_h = KAh[h]
                    # swapped V copy [vim|vre] for this head (GpSimd, idle)
                    vasw = atp.tile([P, KT, P], BF16, tag="vasw", bufs=2,
                                    name=f"vasw{h}")
                    vswv = vasw.rearrange("p k (s j) -> p k s j", s=2)
                    nc.gpsimd.tensor_copy(out=vswv[:, :, 0, :],
                                          in_=VAv[:, :, h, 1, :])
                    nc.gpsimd.tensor_copy(out=vswv[:, :, 1, :],
                                          in_=VAv[:, :, h, 0, :])

                    if prev is not None:
                        prev['avt'] = big_ps(f"av{prev['h']}")

                    A_re = atp.tile([P, KT, 512], BF16, tag="Are", bufs=1,
                                    name="Are")
                    A_im = atp.tile([P, KT, 512], BF16, tag="Aim", bufs=1,
                                    name="Aim")
                    for ci, (Qm, Atile) in enumerate(((qc_h, A_re),
                                                     (qd_h, A_im))):
                        for i in range(KT // 2):
                            if prev is not None:
                                # interleave prev head's AV matmuls
                                pav, ph = prev['avt'], prev['h']
                                pstat = (VAv[:, :, ph, :, :] if ci == 0
                                         else prev['vasw'])
                                for j in (2 * i, 2 * i + 1):
                                    lhs = (pstat[:, j, :, :] if ci == 0
                                           else pstat[:, j, :])
                                    nc.tensor.matmul(
                                        pav[:, ci, :], lhs,
                                        prev['A'][ci][:, j, :],
                                        start=(j == 0), stop=(j == KT - 1))
                            sp = pair_ps("pss")
                            for j2 in range(2):
                                kt = 2 * i + j2
                                nc.tensor.matmul(
                                    sp[:, j2, :],
                                    ka_h[:, kt * P:(kt + 1) * P], Qm,
                                    start=True, stop=True)
                            nc.scalar.activation(
                                out=Atile[:, 2 * i:2 * i + 2, :], in_=sp,
                                func=AF.Exp, bias=m8[:, 0:1])
                            if prev is not None and ci == 0 and i == 1:
                                # prev's z reduction fills the act-paced
                                # stalls mid-scores (its exps are drained)
                                emit_z(prev)
                    if prev is not None:
                        emit_epilogue(prev)
                    prev = {'h': h, 'A': (A_re, A_im), 'vasw': vasw}

                # drain the pipeline for the last head
                emit_z(prev)
                prev['avt'] = big_ps(f"av{prev['h']}")
                ph = prev['h']
                for ci in range(2):
                    pstat = VAv[:, :, ph, :, :] if ci == 0 else prev['vasw']
                    for j in range(KT):
                        lhs = pstat[:, j, :, :] if ci == 0 else pstat[:, j, :]
                        nc.tensor.matmul(prev['avt'][:, ci, :], lhs,
                                         prev['A'][ci][:, j, :],
                                         start=(j == 0), stop=(j == KT - 1))
                emit_epilogue(prev)

                # ---------------- o-proj (gated) + residual + LN2 -----------
                wov = [w[:, 0:4096].rearrange("p (h n) -> p h n", h=H)
                       for w in (wo0, wo1)]
                x2 = ap_.tile([P, 2, DT, TQ], F32, tag="VAx")
                x2r, x2i = x2[:, 0], x2[:, 1]
                n2r = ap_.tile([P, DT, TQ], BF16, tag="n2r")
                n2i = ap_.tile([P, DT, TQ], BF16, tag="n2i")
                xv_c = [src_d.ap().rearrange("(o p) t -> p o t", p=P)
                        for src_d in (xTr_d, xTi_d)]
                # residual preloads + LN2 x tiles
                xt2 = [tp.tile([P, DT, 512], BF16, tag="xt", bufs=3,
                               name=f"xt2{c}") for c in range(2)]
                xq2 = [tp.tile([P, DT, 512], BF16, tag="xq", bufs=1,
                               name=f"xq2{c}") for c in range(2)]
                xres = {}
                for mp in range(2):
                    for comp in range(2):
                        for sl2 in range(2):
                            mt = 2 * mp + sl2
                            xr_ = tp.tile([P, 512], F32, tag="xch", bufs=3,
                                          name=f"xres{comp}{mt}")
                            gdma(xr_, xv_c[comp][:, mt, 0:TQ])
                            xres[(comp, mt)] = xr_

                for mp in range(2):
                    psR = pair_ps(f"pso{mp}")
                    psI = big_ps(f"psoi{mp}")
                    for ps_, oc in ((psR, 0), (psI, 1)):
                        for sl2 in range(2):
                            mt = 2 * mp + sl2
                            for h in range(H):
                                nc.tensor.matmul(
                                    ps_[:, sl2, :],
                                    wov[oc][:, h, mt * P:(mt + 1) * P], OT[h],
                                    start=(h == 0), stop=(h == H - 1))
                    for sl2 in range(2):
                        mt = 2 * mp + sl2
                        nc.vector.scalar_tensor_tensor(
                            out=x2r[:, mt, :], in0=psR[:, sl2, :],
                            scalar=b_ore[:, mt:mt + 1], in1=xres[(0, mt)],
                            op0=ALU.add, op1=ALU.add)
                        nc.vector.scalar_tensor_tensor(
                            out=x2i[:, mt, :], in0=psI[:, sl2, :],
                            scalar=b_oim[:, mt:mt + 1], in1=xres[(1, mt)],
                            op0=ALU.add, op1=ALU.add)
                        # LN2 prep for this mt (copy to bf16 + squares)
                        for comp, src in ((0, x2r), (1, x2i)):
                            eng = nc.gpsimd if comp == 0 else nc.vector
                            eng.tensor_copy(out=xt2[comp][:, mt, :],
                                            in_=src[:, mt, :])
                            eng.tensor_tensor(out=xq2[comp][:, mt, :],
                                              in0=xt2[comp][:, mt, :],
                                              in1=xt2[comp][:, mt, :],
                                              op=ALU.mult)

                # LN2 stats (squares already done above)
                def ln_stats2(xtile, qtile, nm):
                    st = pair_ps(f"st{nm}")
                    for d in range(DT):
                        nc.tensor.matmul(st[0:1, 0, :], ones[:, 0:1],
                                         xtile[:, d, :],
                                         start=(d == 0), stop=(d == DT - 1))
                    for d in range(DT):
                        nc.tensor.matmul(st[0:1, 1, :], ones[:, 0:1],
                                         qtile[:, d, :],
                                         start=(d == 0), stop=(d == DT - 1))
                    return st

                stE = ln_stats2(xt2[0], xq2[0], "E")
                stF = ln_stats2(xt2[1], xq2[1], "F")
                rowsE = ln_rows(stE, "E")
                rowsF = ln_rows(stF, "F")
                lnbE = ln_bcast(rowsE, "E")
                lnbF = ln_bcast(rowsF, "F")
                ln_finals(lnbE, xt2[0], n2r, "E")
                ln_finals(lnbF, xt2[1], n2i, "F")

                # ---------------- MLP f1 (Karatsuba: 12 matmuls / mt) -------
                n2s = tp.tile([P, DT, 512], BF16, tag="xq", bufs=1, name="n2s")
                for d in range(DT):
                    eng = nc.vector if d % 2 == 0 else nc.gpsimd
                    eng.tensor_tensor(out=n2s[:, d, :], in0=n2r[:, d, :],
                                      in1=n2i[:, d, :], op=ALU.add)
                g1r = ap_.tile([P, MLP // P, TQ], BF16, tag="nbig1")
                g1i = ap_.tile([P, MLP // P, TQ], BF16, tag="nbig2")
                for j in range(4):
                    pk = loadpack(wf1_d[j], f"wf1_{j}")
                    f1a, f1b, f1s = (pk[:, i * 2048:(i + 1) * 2048].rearrange(
                        "p (k n) -> p k n", k=DT) for i in range(3))
                    for ml in range(4):
                        mt = j * 4 + ml
                        p12 = pair_ps("psf1")
                        p3 = big_ps("psf1b")
                        for sec, r_, ps_ in ((f1a, n2r, p12[:, 0, :]),
                                             (f1b, n2i, p12[:, 1, :]),
                                             (f1s, n2s, p3[:, 0, :])):
                            for dk in range(DT):
                                nc.tensor.matmul(
                                    ps_, sec[:, dk, ml * P:(ml + 1) * P],
                                    r_[:, dk, :],
                                    start=(dk == 0), stop=(dk == DT - 1))
                        t1s = tp.tile([P, 512], F32, tag="xch", bufs=3,
                                      name="t1s")
                        rpre = tp.tile([P, 512], F32, tag="xch", bufs=3,
                                       name="rpre")
                        ipre = tp.tile([P, 512], F32, tag="xch", bufs=3,
                                       name="ipre")
                        nc.scalar.activation(out=t1s, in_=p12[:, 0, :],
                                             func=AF.Copy)
                        nc.vector.tensor_tensor(
                            out=rpre, in0=t1s, in1=p12[:, 1, :],
                            op=ALU.subtract)
                        nc.vector.scalar_tensor_tensor(
                            out=ipre, in0=p3[:, 0, :],
                            scalar=b_f1im[:, mt:mt + 1], in1=t1s,
                            op0=ALU.add, op1=ALU.subtract)
                        nc.vector.tensor_tensor(
                            out=ipre, in0=ipre, in1=p12[:, 1, :],
                            op=ALU.subtract)
                        nc.scalar.activation(out=g1r[:, mt, :], in_=rpre,
                                             func=AF.Gelu_apprx_tanh,
                                             bias=b_f1re[:, mt:mt + 1])
                        nc.scalar.activation(out=g1i[:, mt, :], in_=ipre,
                                             func=AF.Gelu_apprx_tanh)

                # ---------------- MLP f2 (Karatsuba, single weight pass) ----
                # g1s = g1r + g1i, fragmented across now-dead tags
                g1s_parts = [ap_.tile([P, DT, 512], BF16, tag="n2r",
                                      name="g1sa"),
                             ap_.tile([P, DT, 512], BF16, tag="n2i",
                                      name="g1sb"),
                             tp.tile([P, DT, 512], BF16, tag="xq", bufs=1,
                                     name="g1sc"),
                             tp.tile([P, DT, 512], BF16, tag="xt", bufs=3,
                                     name="g1sd")]

                def g1s(kl):
                    return g1s_parts[kl // 4][:, kl % 4, :]

                wf2p = [loadpack(wf2_d[m], f"wf2_{m}", eng=nc.gpsimd)
                        for m in range(4)]
                for kl in range(16):
                    nc.gpsimd.tensor_tensor(out=g1s(kl), in0=g1r[:, kl, :],
                                            in1=g1i[:, kl, :], op=ALU.add)
                ov2 = out_d.ap().rearrange("c (o p) t -> p c o t", p=P)
                octiles = [atp.tile([P, 2, 2, 512], F32, tag=tg, bufs=1,
                                    name=f"oc{tg}")
                           for tg in ("Are", "Aim")]
                for m in range(4):
                    pk = wf2p[m]
                    f2a, f2b, f2s = (pk[:, i * 2048:(i + 1) * 2048].rearrange(
                        "p (k n) -> p k n", k=16) for i in range(3))
                    fps = pair_ps(f"psf2{m}")
                    p3 = big_ps(f"psf2b{m}")
                    for gi, (sec, r_) in enumerate(((f2a, g1r), (f2b, g1i))):
                        for kl in range(16):
                            nc.tensor.matmul(
                                fps[:, gi, :], sec[:, kl, :], r_[:, kl, :],
                                start=(kl == 0), stop=(kl == 15))
                    for kl in range(16):
                        nc.tensor.matmul(
                            p3[:, 0, :], f2s[:, kl, :], g1s(kl),
                            start=(kl == 0), stop=(kl == 15))
                    oct = octiles[m // 2]
                    mi = m % 2
                    t1s = tp.tile([P, 512], F32, tag="xch", bufs=3,
                                  name="t1s2")
                    tre = tp.tile([P, 512], F32, tag="xch", bufs=3,
                                  name="tre")
                    tim = tp.tile([P, 512], F32, tag="xch", bufs=3,
                                  name="tim")
                    nc.scalar.activation(out=t1s, in_=fps[:, 0, :],
                                         func=AF.Copy)
                    nc.vector.tensor_tensor(
                        out=tre, in0=t1s, in1=fps[:, 1, :],
                        op=ALU.subtract)
                    nc.vector.scalar_tensor_tensor(
                        out=oct[:, 0, mi, :], in0=tre,
                        scalar=b_f2re[:, m:m + 1], in1=x2r[:, m, :],
                        op0=ALU.add, op1=ALU.add)
                    if mi == 1:
                        dma(ov2[:, 0, 2 * (m // 2):2 * (m // 2) + 2, :],
                            oct[:, 0])
                    nc.vector.scalar_tensor_tensor(
                        out=tim, in0=p3[:, 0, :],
                        scalar=b_f2im[:, m:m + 1], in1=t1s,
                        op0=ALU.add, op1=ALU.subtract)
                    nc.vector.tensor_tensor(
                        out=tim, in0=tim, in1=fps[:, 1, :],
                        op=ALU.subtract)
                    nc.vector.tensor_tensor(
                        out=oct[:, 1, mi, :], in0=tim, in1=x2i[:, m, :],
                        op=ALU.add)
                    if mi == 1:
                        dma(ov2[:, 1, 2 * (m // 2):2 * (m // 2) + 2, :],
                            oct[:, 1])

            for _rep in range(reps):
                emit()

    _split_dma_waits(nc)
    return nc


def _split_dma_waits(nc):
    """Walrus's DIRECT2D DMA encoding takes one sync wait; move extra
    waits onto a preceding sequencer EventSemaphore on the same engine."""
    f = nc.m.functions[0]
    for blk in f.blocks:
        out = []
        for ins in blk.instructions:
            si = getattr(ins, 'sync_info', None)
            tn = type(ins).__name__
            lim = 1
            if si is not None and si.on_wait and len(si.on_wait) > lim:
                waits = list(si.on_wait)
                extra = waits[:-lim]
                si.on_wait = waits[-lim:]
                k = 0
                while extra:
                    ev = mybir.InstEventSemaphore(
                        name=f"{ins.name}_wsplit{k}",
                        engine=ins.engine,
                        ins=[], outs=[],
                        sync_info=mybir.SyncInfo(on_wait=extra[:2],
                                                 on_update=[]),
                    )
                    out.append(ev)
                    extra = extra[2:]
                    k += 1
            out.append(ins)
        blk.instructions = out


_NC_CACHE = None


def _get_nc():
    global _NC_CACHE
    if _NC_CACHE is None:
        _NC_CACHE = build_nc()
    return _NC_CACHE


TRACE = False
LAST_RESULT = None


def kernel(**inputs):
    global LAST_RESULT
    nc = _get_nc()
    in_maps = []
    for c in range(NCORES):
        in_maps.append(_prep_core(inputs, c // 2, c % 2))
    res = run_bass_kernel_spmd(nc, in_maps, list(range(NCORES)),
                               trace=TRACE)
    LAST_RESULT = res
    out = np.empty((2, B, T, D), np.float32)
    for c in range(NCORES):
        b, half = c // 2, c % 2
        y = res.results[c]["outT"]          # [2, D, TQ]
        out[:, b, half * TQ:(half + 1) * TQ, :] = y.transpose(0, 2, 1)
    return out
